# revision 18
# baseline (speedup 1.0000x reference)
"""LundNetTagger GNN on 8 Trainium2 NeuronCores (Bass/Tile).

Self-contained: kernel(**inputs) -> np.ndarray [1000, 2] float32.

Strategy: nodes are assigned to 100352 "slots" (8 cores x 98 windows x 128),
packed so each window receives <= 512 edges. Edges live on the core owning
their dst slot, in window-major order padded to 4x128-edge chunks per window.
EdgeConv cat[xi, xj-xi] is folded into split weights WA = W[:C]-W[C:],
WB = W[C:].

conv1 runs feature-major from a host-packed message tensor, keeping z in SBUF.
conv2 layer1 expands the xi term from a per-node table A2 = x1 @ WA2 via
host-precomputed transposed one-hot masks (no per-edge xi matmuls) and gathers
xj rows of x1. conv3 is fully table-based: z_e = A3[dst] + B3[src] with
A3/B3 = x2 @ WA3 / x2 @ WB3 computed during conv2's scatter; the gather then
yields z directly in [edge, channel] layout, GraphNorm stats come from a
Gram-matrix matmul (sum + sum-of-squares in one accumulation), and the scatter
consumes [edge, channel] tiles with zero transposes. One-hot masks for
scatter/expansion/pool are host-precomputed bf16 tables. GraphNorm stats are
global AllReduces; mean-aggregation is a collision-free one-hot matmul scatter
into PSUM per window.
"""
import numpy as np
import ml_dtypes

import concourse.bass as bass
import concourse.tile as tile
from concourse import bacc, mybir
from concourse.bass_utils import run_bass_kernel_spmd

BF16 = mybir.dt.bfloat16
F32 = mybir.dt.float32
AOP = mybir.AluOpType
AFT = mybir.ActivationFunctionType
AX = mybir.AxisListType

N_NODES = 100000
N_EDGES = 400000
N_GRAPHS = 1000
NC = 8
WIN = 128
NWIN = 98
SPC = WIN * NWIN          # 12544
NSLOTS = SPC * NC         # 100352
QUAD = NSLOTS // 4        # 25088
B = 4                     # chunks per window
EPW = B * WIN             # 512
E_PAD = NWIN * EPW        # 50176
EPS = 1e-5

NW_BLK = 7
BLK = NW_BLK * EPW        # 3584
NBLK = NWIN // NW_BLK     # 14
NCHUNK = BLK // 128       # 28
NSEG = BLK // 512         # 7
NCH_ALL = E_PAD // 128    # 392
ZW = 257                  # z3 chunk width (256 channels + ones column)


_cache = {}


# ============================ host-side packing ============================

def _pack(edge_index, batch):
    src = np.asarray(edge_index[0], dtype=np.int64)
    dst = np.asarray(edge_index[1], dtype=np.int64)
    batch = np.asarray(batch, dtype=np.int64)
    cnt = np.bincount(dst, minlength=N_NODES)

    nvirt = NSLOTS - N_NODES
    cnt_all = np.concatenate([cnt, np.zeros(nvirt, dtype=cnt.dtype)])
    order = np.argsort(-cnt_all, kind="stable")
    GW = NWIN * NC
    rounds = NSLOTS // GW
    win_of_rank = np.empty(NSLOTS, dtype=np.int64)
    for r in range(rounds):
        seg = np.arange(GW) if r % 2 == 0 else np.arange(GW - 1, -1, -1)
        win_of_rank[r * GW:(r + 1) * GW] = seg
    win_of_node = np.empty(NSLOTS, dtype=np.int64)
    win_of_node[order] = win_of_rank
    wsum = np.bincount(win_of_node, weights=cnt_all.astype(np.float64),
                       minlength=GW).astype(np.int64)

    cap = EPW
    members_of = [list(np.where(win_of_node == w)[0]) for w in range(GW)]
    for _ in range(2000):
        over = np.where(wsum > cap)[0]
        if len(over) == 0:
            break
        w = int(over[0])
        # smallest-count >0 node in w
        mem = members_of[w]
        cs = [(int(cnt_all[n]), n) for n in mem if cnt_all[n] > 0]
        cs.sort()
        moved = False
        for c1, n in cs:
            # find target window with a smaller-count node to swap
            worder2 = np.argsort(wsum)
            for tw in worder2[:64]:
                tw = int(tw)
                if tw == w:
                    continue
                tmem = members_of[tw]
                best = None
                for m in tmem:
                    c2 = int(cnt_all[m])
                    if c2 < c1 and wsum[tw] + c1 - c2 <= cap:
                        if best is None or c2 < best[0]:
                            best = (c2, m)
                        if c2 == 0:
                            break
                if best is not None:
                    c2, m = best
                    members_of[tw].remove(m)
                    members_of[tw].append(n)
                    members_of[w].remove(n)
                    members_of[w].append(m)
                    win_of_node[n] = tw
                    win_of_node[m] = w
                    wsum[tw] += c1 - c2
                    wsum[w] -= c1 - c2
                    moved = True
                    break
            if moved:
                break
        if not moved:
            raise RuntimeError("packing fixup stuck")
    assert wsum.max() <= cap, f"window packing failed: max={wsum.max()}"

    worder = np.argsort(-wsum, kind="stable")
    core_load = np.zeros(NC, dtype=np.int64)
    core_nwin = np.zeros(NC, dtype=np.int64)
    core_of_win = np.empty(GW, dtype=np.int64)
    for w in worder:
        cands = np.where(core_nwin < NWIN)[0]
        c = cands[np.argmin(core_load[cands])]
        core_of_win[w] = c
        core_load[c] += wsum[w]
        core_nwin[c] += 1

    win_lists = [[] for _ in range(NC)]
    for w in range(GW):
        win_lists[core_of_win[w]].append(w)
    for c in range(NC):
        wl = win_lists[c]
        j = int(np.argmin(wsum[wl]))
        assert wsum[wl[j]] < cap, "no sentinel room"
        wl[j], wl[-1] = wl[-1], wl[j]

    slot_of_node = np.empty(NSLOTS, dtype=np.int64)
    for c in range(NC):
        for wi, w in enumerate(win_lists[c]):
            mem = np.sort(np.array(members_of[w], dtype=np.int64))
            assert len(mem) == WIN
            slot_of_node[mem] = c * SPC + wi * WIN + np.arange(WIN)
    node_of_slot = np.empty(NSLOTS, dtype=np.int64)
    node_of_slot[slot_of_node] = np.arange(NSLOTS)
    cnt_of_slot = cnt_all[node_of_slot]

    qzero = []
    for q in range(4):
        z = np.where(cnt_of_slot[q * QUAD:(q + 1) * QUAD] == 0)[0]
        assert len(z) > 0
        qzero.append(int(z[0]))  # local to quadrant
    czero = []
    for c in range(NC):
        z = np.where(cnt_of_slot[c * SPC:(c + 1) * SPC] == 0)[0]
        assert len(z) > 0
        czero.append(int(z[0]))  # local to core

    dslot = slot_of_node[dst]
    sslot = slot_of_node[src]
    ecore = dslot // SPC
    ewin = (dslot % SPC) // WIN
    key = ecore * (NWIN * WIN) + ewin * WIN + (dslot % WIN)
    eorder = np.argsort(key, kind="stable")
    dsl, ssl = dslot[eorder], sslot[eorder]
    ec, ew = ecore[eorder], ewin[eorder]

    cw = ec * NWIN + ew
    cw_cnt = np.bincount(cw, minlength=NC * NWIN)
    assert cw_cnt.max() <= EPW

    xi_idx = np.zeros((NC, E_PAD), dtype=np.int64)
    xj_idx = np.zeros((NC, E_PAD), dtype=np.int64)
    dstwin = np.full((NC, E_PAD), -1.0, dtype=np.float32)
    valid = np.zeros((NC, E_PAD), dtype=bool)

    ofs = (np.arange(NC * NWIN) % NWIN) * EPW
    start = np.concatenate([[0], np.cumsum(cw_cnt)[:-1]])
    within = np.arange(N_EDGES) - start[cw]
    pos = ofs[cw] + within
    xi_idx[ec, pos] = dsl % SPC
    xj_idx[ec, pos] = ssl
    dstwin[ec, pos] = (dsl % WIN).astype(np.float32)
    valid[ec, pos] = True
    for c in range(NC):
        xi_idx[c, ~valid[c]] = czero[c]
    pad_cnt = (~valid).sum(axis=1).astype(np.float32)
    assert np.all(~valid[:, -1]), "sentinel column must be padding"

    gzero = qzero[0]  # global slot with zero row
    xj_glob = np.where(valid, xj_idx, gzero).astype(np.int32)

    inv_cnt = (1.0 / np.maximum(cnt_of_slot.reshape(NC, SPC), 1.0)).astype(np.float32)

    g_of_slot = np.full(NSLOTS, -1, dtype=np.int64)
    real = node_of_slot < N_NODES
    g_of_slot[real] = batch[node_of_slot[real]]
    NGW = 8
    Bg = 0
    pools = [[None] * NGW for _ in range(NC)]
    for c in range(NC):
        gl = g_of_slot[c * SPC:(c + 1) * SPC]
        for gw in range(NGW):
            m = np.where((gl >= gw * 128) & (gl < (gw + 1) * 128))[0]
            pools[c][gw] = m
            Bg = max(Bg, (len(m) + 127) // 128)
    NPG = Bg * 128
    pool_idx = np.zeros((NC, NGW, NPG), dtype=np.int16)
    pool_gwl = np.full((NC, NGW, NPG), -1.0, dtype=np.float32)
    for c in range(NC):
        for gw in range(NGW):
            m = pools[c][gw]
            pool_idx[c, gw, :len(m)] = m.astype(np.int16)
            pool_idx[c, gw, len(m):] = czero[c]
            pool_gwl[c, gw, :len(m)] = (g_of_slot[c * SPC + m] - gw * 128).astype(np.float32)

    gcnt = np.bincount(batch, minlength=N_GRAPHS).astype(np.float32)
    inv_g = np.zeros(1024, dtype=np.float32)
    inv_g[:N_GRAPHS] = 1.0 / np.maximum(gcnt, 1.0)

    return dict(slot_of_node=slot_of_node, node_of_slot=node_of_slot,
                xj_glob=xj_glob, dstwin=dstwin, pad_cnt=pad_cnt,
                inv_cnt=inv_cnt, valid=valid, eorder=eorder, ec=ec, pos=pos,
                pool_idx=pool_idx, pool_gwl=pool_gwl, inv_g=inv_g, Bg=Bg,
                qzero=qzero)


def _wrap_idx(a):
    """[.., n] int -> [.., 128, n//16]: element i -> partition i%16 col i//16,
    replicated to 8 groups of 16 partitions."""
    n = a.shape[-1]
    assert n % 16 == 0
    w = a.reshape(*a.shape[:-1], n // 16, 16)
    w = np.swapaxes(w, -1, -2)
    w = np.broadcast_to(w[..., None, :, :], (*a.shape[:-1], 8, 16, n // 16))
    return np.ascontiguousarray(w).reshape(*a.shape[:-1], 128, n // 16).astype(np.int16)


def _bf(x):
    return np.ascontiguousarray(np.asarray(x, dtype=np.float32)).astype(ml_dtypes.bfloat16)


def _tile_w(w):
    K, M = w.shape
    nk, nm = (K + 127) // 128, (M + 127) // 128
    out = np.zeros((nk, nm, 128, 128), dtype=ml_dtypes.bfloat16)
    for i in range(nk):
        for j in range(nm):
            blk = np.asarray(w, dtype=np.float32)[i * 128:(i + 1) * 128, j * 128:(j + 1) * 128]
            out[i, j, :blk.shape[0], :blk.shape[1]] = _bf(blk)
    return out


# ============================ device kernel ============================

EHALF = E_PAD // 2        # 25088
NSEG_H = EHALF // 512     # 49


def _build(Bg, debug=False, phases=4):
    nc = bacc.Bacc("TRN2", target_bir_lowering=False, debug=False, num_devices=NC)

    def din(name, shape, dt):
        return nc.dram_tensor(name, shape, dt, kind="ExternalInput").ap()

    t_msgT = din("msgT", [48, EHALF], BF16)
    t_xjw = din("xjw", [4, 128, E_PAD // 16], mybir.dt.int16)
    t_pidxw = din("pidxw", [8, 128, Bg * 8], mybir.dt.int16)
    t_oh = din("ohtab", [128, E_PAD], BF16)
    t_ohT = din("ohTtab", [128, E_PAD], BF16)
    t_invcnt = din("invcnt", [128, NWIN], F32)
    t_padcnt = din("padcnt", [128, 1], F32)
    t_ident = din("ident", [128, 128], BF16)
    t_eyef = din("eyef", [128, 128], F32)
    t_c1w = din("c1w", [3, 128, 128], BF16)
    t_c1b = din("c1b", [3, 128, 1], F32)
    t_c1gn = din("c1gn", [3, 3, 128, 1], F32)
    t_wa2 = din("wa2", [128, 256], BF16)
    t_c2wb = din("c2wb", [2, 128, 128], BF16)
    t_c2w2 = din("c2w2", [2, 2, 128, 128], BF16)
    t_c2b = din("c2b", [2, 2, 128, 1], F32)
    t_c2gn = din("c2gn", [2, 3, 2, 128, 1], F32)
    t_wa3 = din("wa3", [2, 128, 256], BF16)
    t_wb3 = din("wb3", [2, 128, 256], BF16)
    t_c3b = din("c3b", [2, 128, 1], F32)
    t_c3gn = din("c3gn", [3, 2, 128, 1], F32)
    t_lw1 = din("lw1", [2, 2, 128, 128], BF16)
    t_lb1 = din("lb1", [2, 128, 1], F32)
    t_lw2 = din("lw2", [2, 128, 2], BF16)
    t_lb2 = din("lb2", [2, 1], F32)
    t_poh = din("poolohtab", [128, 8 * Bg * 128], BF16)
    t_invg = din("invg", [128, 8], F32)

    o_out = nc.dram_tensor("out", [2, N_GRAPHS], F32, kind="ExternalOutput").ap()
    dbg = {}
    if debug:
        dbg["x1"] = nc.dram_tensor("dbg_x1", [NSLOTS, 128], BF16, kind="ExternalOutput").ap()
        dbg["x2"] = nc.dram_tensor("dbg_x2", [SPC, 256], BF16, kind="ExternalOutput").ap()
        dbg["x3"] = nc.dram_tensor("dbg_x3", [SPC, 256], BF16, kind="ExternalOutput").ap()
        dbg["pool"] = nc.dram_tensor("dbg_pool", [1024, 256], F32, kind="ExternalOutput").ap()

    with tile.TileContext(nc) as tc:
        with tc.tile_pool(name="dram", bufs=1, space="DRAM") as dram, \
             tc.tile_pool(name="cp", bufs=1) as cp:
            z_scr = [dram.tile([2, 128, E_PAD], BF16, tag=f"zscr{i}", name=f"zscr{i}") for i in range(2)]
            z3_scr = dram.tile([128, NCH_ALL * ZW], BF16)
            tab1_loc = dram.tile([SPC, 128], BF16)
            tab1 = dram.tile([NSLOTS, 128], BF16)
            a2_loc = dram.tile([SPC, 256], BF16)
            a3_loc = dram.tile([SPC, 256], BF16)
            b3_loc = dram.tile([SPC, 256], BF16)
            b3_full = dram.tile([NSLOTS, 256], BF16)
            tab3_loc = dram.tile([SPC, 256], BF16)
            st_in = dram.tile([128, 8], F32)
            st_out = dram.tile([128, 8], F32)
            pool_in = dram.tile([1024, 256], F32)
            pool_out = dram.tile([1024, 256], F32)

            ident = cp.tile([128, 128], BF16)
            nc.sync.dma_start(ident[:], t_ident[:])
            eyef = cp.tile([128, 128], F32)
            nc.sync.dma_start(eyef[:], t_eyef[:])
            invcnt = cp.tile([128, NWIN], F32)
            nc.sync.dma_start(invcnt[:], t_invcnt[:])
            padcnt = cp.tile([128, 1], F32)
            nc.sync.dma_start(padcnt[:], t_padcnt[:])

            # ---------- helpers ----------
            def allreduce_stats(s_acc, q_acc, n_mb, sb):
                st = sb.tile([128, 8], F32, tag="st_")
                nc.vector.memset(st[:], 0.0)
                nc.vector.tensor_copy(st[:, 0:n_mb], s_acc[:])
                nc.vector.tensor_copy(st[:, 4:4 + n_mb], q_acc[:])
                nc.sync.dma_start(st_in[:], st[:])
                nc.gpsimd.collective_compute(
                    "AllReduce", AOP.add, replica_groups=[list(range(NC))],
                    ins=[st_in.opt()], outs=[st_out.opt()])
                stg = sb.tile([128, 8], F32, tag="stg_")
                nc.sync.dma_start(stg[:], st_out[:])
                return stg

            def affine_from_stats(stg, n_mb, b_lin, gn, sb):
                A, Cc = [], []
                for mb in range(n_mb):
                    s = stg[:, mb:mb + 1]
                    q = stg[:, 4 + mb:5 + mb]
                    g, bgn, ms = gn[0][mb], gn[1][mb], gn[2][mb]
                    bl = b_lin[mb]
                    m = sb.tile([128, 1], F32, tag="af_m")
                    nc.vector.tensor_scalar(m[:], s, 1.0 / N_EDGES, None, AOP.mult)
                    nc.vector.tensor_tensor(m[:], m[:], bl, op=AOP.add)
                    e2 = sb.tile([128, 1], F32, tag="af_e2")
                    nc.vector.tensor_scalar(e2[:], q, 1.0 / N_EDGES, None, AOP.mult)
                    tmp = sb.tile([128, 1], F32, tag="af_t")
                    nc.vector.tensor_tensor(tmp[:], m[:], bl, op=AOP.mult)
                    nc.vector.tensor_scalar(tmp[:], tmp[:], 2.0, None, AOP.mult)
                    nc.vector.tensor_tensor(e2[:], e2[:], tmp[:], op=AOP.add)
                    nc.vector.tensor_tensor(tmp[:], bl, bl, op=AOP.mult)
                    nc.vector.tensor_tensor(e2[:], e2[:], tmp[:], op=AOP.subtract)
                    msm = sb.tile([128, 1], F32, tag="af_msm")
                    nc.vector.tensor_tensor(msm[:], ms, m[:], op=AOP.mult)
                    var = sb.tile([128, 1], F32, tag="af_v")
                    nc.vector.tensor_tensor(var[:], msm[:], msm[:], op=AOP.mult)
                    nc.vector.tensor_tensor(tmp[:], msm[:], m[:], op=AOP.mult)
                    nc.vector.tensor_scalar(tmp[:], tmp[:], 2.0, None, AOP.mult)
                    nc.vector.tensor_tensor(var[:], var[:], tmp[:], op=AOP.subtract)
                    nc.vector.tensor_tensor(var[:], var[:], e2[:], op=AOP.add)
                    a = sb.tile([128, 1], F32, tag="af_a")
                    nc.vector.tensor_scalar(var[:], var[:], EPS, None, AOP.add)
                    nc.scalar.activation(a[:], var[:], AFT.Sqrt)
                    nc.vector.reciprocal(a[:], a[:])
                    nc.vector.tensor_tensor(a[:], a[:], g, op=AOP.mult)
                    cc = sb.tile([128, 1], F32, tag="af_c")
                    nc.vector.tensor_tensor(cc[:], bl, msm[:], op=AOP.subtract)
                    nc.vector.tensor_tensor(cc[:], cc[:], a[:], op=AOP.mult)
                    nc.vector.tensor_tensor(cc[:], cc[:], bgn, op=AOP.add)
                    A.append(a)
                    Cc.append(cc)
                return A, Cc

            sqscr = cp.tile([128, BLK], BF16)

            def load_xjw(pool):
                xw = pool.tile([128, 4 * (E_PAD // 16)], mybir.dt.int16, tag="xjw",
                               name="xjw")
                for q in range(4):
                    nc.sync.dma_start(
                        xw[:, q * (E_PAD // 16):(q + 1) * (E_PAD // 16)], t_xjw[q])
                return xw

            def acc_stats(ps_ap, s_col, q_col, sb, n=512, sq_scalar=False):
                t1 = sb.tile([128, 1], F32, tag="rs_t1")
                nc.vector.reduce_sum(out=t1[:], in_=ps_ap, axis=AX.X)
                nc.vector.tensor_tensor(s_col, s_col, t1[:], op=AOP.add)
                qa = sb.tile([128, 1], F32, tag="rs_qa")
                if sq_scalar:
                    nc.scalar.activation(sqscr[:, :n], ps_ap, AFT.Square,
                                         accum_out=qa[:])
                else:
                    nc.vector.tensor_tensor(sqscr[:, :n], ps_ap, ps_ap, op=AOP.mult)
                    nc.vector.reduce_sum(out=qa[:], in_=sqscr[:, :n], axis=AX.X)
                nc.vector.tensor_tensor(q_col, q_col, qa[:], op=AOP.add)

            def quad_gather(gq, tab_full, b, elem, xw):
                """4 quadrant dma_gathers of one block's rows + merge into gq[0].
                SWDGE descriptor ring caps a single gather at 1024 rows."""
                n16 = E_PAD // 16
                groups = [(0, 1024), (1024, 1024), (2048, 1024), (3072, 512)]
                for q in range(4):
                    for g0, gn in groups:
                        e0 = b * BLK + g0
                        c0 = (g0 // 128) * elem
                        c1 = ((g0 + gn) // 128) * elem
                        nc.gpsimd.dma_gather(
                            out_ap=gq[q][:, c0:c1].rearrange(
                                "p (j c) -> p j c", c=elem),
                            in_ap=tab_full[q * QUAD:(q + 1) * QUAD, :],
                            idxs_ap=xw[:, q * n16 + e0 // 16:q * n16 + (e0 + gn) // 16],
                            num_idxs=gn, num_idxs_reg=gn, elem_size=elem)
                for q in range(1, 4):
                    nc.vector.tensor_tensor(gq[0][:], gq[0][:], gq[q][:], op=AOP.add)
                return gq[0]

            def sentinel_correct(s_acc, q_acc, zsent_cols, n_mb, sb):
                for mb in range(n_mb):
                    zs = zsent_cols[mb]
                    t1 = sb.tile([128, 1], F32, tag="sc_t1")
                    nc.vector.tensor_tensor(t1[:], zs, padcnt[:], op=AOP.mult)
                    nc.vector.tensor_tensor(s_acc[:, mb:mb + 1], s_acc[:, mb:mb + 1],
                                            t1[:], op=AOP.subtract)
                    nc.vector.tensor_tensor(t1[:], zs, zs, op=AOP.mult)
                    nc.vector.tensor_tensor(t1[:], t1[:], padcnt[:], op=AOP.mult)
                    nc.vector.tensor_tensor(q_acc[:, mb:mb + 1], q_acc[:, mb:mb + 1],
                                            t1[:], op=AOP.subtract)

            def load_vec(t_ap, sb, tag):
                v = sb.tile([128, 1], F32, tag=tag)
                nc.sync.dma_start(v[:], t_ap)
                return v[:]

            # ======================= CONV 1 =======================
            with tc.tile_pool(name="c1sb", bufs=2) as sb, \
                 tc.tile_pool(name="c1zb", bufs=1) as zbp:
                c1b = [[load_vec(t_c1b[i], sb, f"c1b{i}")] for i in range(3)]
                c1gn = [[[load_vec(t_c1gn[i, j], sb, f"c1gn{i}{j}")] for j in range(3)]
                        for i in range(3)]
                zbuf = zbp.tile([128, E_PAD], BF16)
                with tc.tile_pool(name="c1big", bufs=2) as bp, \
                     tc.tile_pool(name="c1ps", bufs=2, space="PSUM") as ps, \
                     tc.tile_pool(name="msgp", bufs=1) as msgp:
                    c1w = []
                    for i in range(3):
                        w = sb.tile([128, 128], BF16, tag=f"c1w{i}")
                        nc.sync.dma_start(w[:], t_c1w[i])
                        c1w.append(w)
                    msgT = msgp.tile([48, EHALF], BF16)
                    nc.sync.dma_start(msgT[:], t_msgT[:])

                    def z1_psum(h, s):
                        zp = ps.tile([128, 512], F32, tag="zp")
                        nc.tensor.matmul(zp[:], c1w[0][32 * h:32 * h + 10, :],
                                         msgT[32 * h:32 * h + 10, s * 512:(s + 1) * 512],
                                         start=True, stop=True)
                        return zp

                    s1 = sb.tile([128, 1], F32, tag="s1")
                    q1 = sb.tile([128, 1], F32, tag="q1")
                    nc.vector.memset(s1[:], 0.0)
                    nc.vector.memset(q1[:], 0.0)
                    for h in range(2):
                        for s in range(NSEG_H):
                            zp = z1_psum(h, s)
                            acc_stats(zp[:], s1[:, 0:1], q1[:, 0:1], sb,
                                      sq_scalar=True)
                    stg = allreduce_stats(s1, q1, 1, sb)
                    A1, C1 = affine_from_stats(stg, 1, c1b[0], c1gn[0], sb)

                    # L2: recompute z1, relu, z2 = W2 @ h1 -> zbuf (SBUF), batched stats
                    s2 = sb.tile([128, 1], F32, tag="s2")
                    q2 = sb.tile([128, 1], F32, tag="q2")
                    nc.vector.memset(s2[:], 0.0)
                    nc.vector.memset(q2[:], 0.0)
                    for h in range(2):
                        for b in range(NSEG_H // 7):
                            h1 = bp.tile([128, BLK], BF16, tag="h1")
                            for s in range(7):
                                zp = z1_psum(h, b * 7 + s)
                                nc.scalar.activation(h1[:, s * 512:(s + 1) * 512], zp[:],
                                                     AFT.Relu, bias=C1[0], scale=A1[0])
                            col0 = h * EHALF + b * BLK
                            for s in range(7):
                                zp = ps.tile([128, 512], F32, tag="zp")
                                nc.tensor.matmul(zp[:], c1w[1][:],
                                                 h1[:, s * 512:(s + 1) * 512],
                                                 start=True, stop=True)
                                nc.vector.tensor_copy(
                                    zbuf[:, col0 + s * 512:col0 + (s + 1) * 512], zp[:])
                            acc_stats(zbuf[:, col0:col0 + BLK], s2[:, 0:1], q2[:, 0:1],
                                      sb, n=BLK)
                    zs2 = sb.tile([128, 1], F32, tag="zs2")
                    nc.vector.tensor_copy(zs2[:], zbuf[:, E_PAD - 1:E_PAD])
                    sentinel_correct(s2, q2, [zs2[:]], 1, sb)
                    stg2 = allreduce_stats(s2, q2, 1, sb)
                    A2, C2 = affine_from_stats(stg2, 1, c1b[1], c1gn[1], sb)

                    # L3: h2 = relu(aff(z2)), z3 = W3 @ h2 -> zbuf in place
                    s3 = sb.tile([128, 1], F32, tag="s3")
                    q3 = sb.tile([128, 1], F32, tag="q3")
                    nc.vector.memset(s3[:], 0.0)
                    nc.vector.memset(q3[:], 0.0)
                    for b in range(NBLK):
                        h2 = bp.tile([128, BLK], BF16, tag="h2")
                        nc.scalar.activation(h2[:], zbuf[:, b * BLK:(b + 1) * BLK],
                                             AFT.Relu, bias=C2[0], scale=A2[0])
                        for s in range(7):
                            zp = ps.tile([128, 512], F32, tag="zp")
                            nc.tensor.matmul(zp[:], c1w[2][:],
                                             h2[:, s * 512:(s + 1) * 512],
                                             start=True, stop=True)
                            nc.vector.tensor_copy(
                                zbuf[:, b * BLK + s * 512:b * BLK + (s + 1) * 512], zp[:])
                        acc_stats(zbuf[:, b * BLK:(b + 1) * BLK], s3[:, 0:1], q3[:, 0:1],
                                  sb, n=BLK)
                    zs3 = sb.tile([128, 1], F32, tag="zs3")
                    nc.vector.tensor_copy(zs3[:], zbuf[:, E_PAD - 1:E_PAD])
                    sentinel_correct(s3, q3, [zs3[:]], 1, sb)
                    stg3 = allreduce_stats(s3, q3, 1, sb)
                    A3, C3 = affine_from_stats(stg3, 1, c1b[2], c1gn[2], sb)

                # scatter: h3 = relu(aff(z3)); transpose; one-hot matmul; + A2 table
                with tc.tile_pool(name="s1sb", bufs=2) as sp, \
                     tc.tile_pool(name="s1oh", bufs=2) as ohp, \
                     tc.tile_pool(name="s1tp", bufs=2, space="PSUM") as ps_tp, \
                     tc.tile_pool(name="s1sc", bufs=2, space="PSUM") as ps_sc:
                    wa2 = sp.tile([128, 256], BF16, tag="wa2")
                    nc.sync.dma_start(wa2[:], t_wa2[:])
                    for b in range(NBLK):
                        h3 = sp.tile([128, BLK], BF16, tag="h3")
                        nc.scalar.activation(h3[:], zbuf[:, b * BLK:(b + 1) * BLK],
                                             AFT.Relu, bias=C3[0], scale=A3[0])
                        ohb = ohp.tile([128, BLK], BF16, tag="ohb")
                        nc.sync.dma_start(ohb[:], t_oh[:, b * BLK:(b + 1) * BLK])
                        for w in range(NW_BLK):
                            gw = b * NW_BLK + w
                            tpp = ps_tp.tile([128, 512], BF16, tag="tpp", space="PSUM")
                            for cb in range(B):
                                nc.tensor.transpose(
                                    tpp[:, cb * 128:(cb + 1) * 128],
                                    h3[:, (w * B + cb) * 128:(w * B + cb + 1) * 128],
                                    ident[:])
                            hE = sp.tile([128, 512], BF16, tag="hE")
                            nc.vector.tensor_copy(hE[:], tpp[:])
                            sc = ps_sc.tile([128, 128], F32, tag="sc", space="PSUM")
                            for cb in range(B):
                                nc.tensor.matmul(
                                    sc[:], ohb[:, (w * B + cb) * 128:(w * B + cb + 1) * 128],
                                    hE[:, cb * 128:(cb + 1) * 128],
                                    start=(cb == 0), stop=(cb == B - 1))
                            nt = sp.tile([128, 128], BF16, tag="nt")
                            nc.vector.tensor_scalar(nt[:], sc[:], invcnt[:, gw:gw + 1],
                                                    None, AOP.mult)
                            nc.sync.dma_start(tab1_loc[gw * WIN:(gw + 1) * WIN, :], nt[:])
                            # A2 table: ntT then (x1_win) @ WA2
                            ntp = ps_tp.tile([128, 128], BF16, tag="ntp", space="PSUM")
                            nc.tensor.transpose(ntp[:], nt[:], ident[:])
                            ntT = sp.tile([128, 128], BF16, tag="ntT")
                            nc.vector.tensor_copy(ntT[:], ntp[:])
                            a2p = ps_sc.tile([128, 256], F32, tag="a2p", space="PSUM")
                            nc.tensor.matmul(a2p[:], ntT[:], wa2[:], start=True, stop=True)
                            a2t = sp.tile([128, 256], BF16, tag="a2t")
                            nc.vector.tensor_copy(a2t[:], a2p[:])
                            nc.sync.dma_start(a2_loc[gw * WIN:(gw + 1) * WIN, :], a2t[:])

            nc.gpsimd.collective_compute(
                "AllGather", AOP.bypass, replica_groups=[list(range(NC))],
                ins=[tab1_loc.opt()], outs=[tab1.opt()])
            if debug:
                nc.sync.dma_start(dbg["x1"][:], tab1[:])

            # ======================= CONV 2 =======================
            if phases >= 2:
              with tc.tile_pool(name="c2sb", bufs=2) as sb:
                  c2b = [[load_vec(t_c2b[i, mb], sb, f"c2b{i}{mb}") for mb in range(2)]
                         for i in range(2)]
                  c2gn = [[[load_vec(t_c2gn[i, j, mb], sb, f"c2gn{i}{j}{mb}")
                            for mb in range(2)] for j in range(3)] for i in range(2)]
                  # ---- pass 1: z1 = A2[dst] (one-hot expand) + WB2 @ x1[src] ----
                  sA = sb.tile([128, 2], F32, tag="c2sA")
                  qA = sb.tile([128, 2], F32, tag="c2qA")
                  nc.vector.memset(sA[:], 0.0)
                  nc.vector.memset(qA[:], 0.0)
                  with tc.tile_pool(name="g2g", bufs=2) as g2, \
                       tc.tile_pool(name="g2q", bufs=1) as gqp, \
                       tc.tile_pool(name="g2z", bufs=2) as zwp, \
                       tc.tile_pool(name="g2ps", bufs=2, space="PSUM") as ps, \
                       tc.tile_pool(name="g2tp", bufs=2, space="PSUM") as ps_tp:
                      wbs = []
                      for mo in range(2):
                          wtb = sb.tile([128, 128], BF16, tag=f"c2wb{mo}")
                          nc.sync.dma_start(wtb[:], t_c2wb[mo])
                          wbs.append(wtb)
                      xw2 = load_xjw(g2)
                      for b in range(NBLK):
                          gq = [gqp.tile([128, NCHUNK * 128], BF16, tag=f"gq{q}",
                                         name=f"gq{q}") for q in range(4)]
                          gxj = quad_gather(gq, tab1, b, 128, xw2)
                          ohTb = g2.tile([128, BLK], BF16, tag="ohTb")
                          nc.sync.dma_start(ohTb[:], t_ohT[:, b * BLK:(b + 1) * BLK])
                          zsb = [zwp.tile([128, BLK], BF16, tag=f"zsb{h}", name=f"zsb{h}")
                                 for h in range(2)]
                          for w in range(NW_BLK):
                              gw = b * NW_BLK + w
                              a2w = g2.tile([128, 256], BF16, tag="a2w")
                              nc.sync.dma_start(a2w[:], a2_loc[gw * WIN:(gw + 1) * WIN, :])
                              tpp = ps_tp.tile([128, 512], BF16, tag="xtp", space="PSUM")
                              for cb in range(B):
                                  nc.tensor.transpose(
                                      tpp[:, cb * 128:(cb + 1) * 128],
                                      gxj[:, (w * B + cb) * 128:(w * B + cb + 1) * 128],
                                      ident[:])
                              xjT = g2.tile([128, 512], BF16, tag="xjT")
                              nc.vector.tensor_copy(xjT[:], tpp[:])
                              for h in range(2):
                                  zp = ps.tile([128, 512], F32, tag="zp")
                                  nc.tensor.matmul(zp[:], a2w[:, h * 128:(h + 1) * 128],
                                                   ohTb[:, w * 512:(w + 1) * 512],
                                                   start=True, stop=False)
                                  nc.tensor.matmul(zp[:], wbs[h][:], xjT[:],
                                                   start=False, stop=True)
                                  nc.scalar.copy(zsb[h][:, w * 512:(w + 1) * 512], zp[:])
                          for h in range(2):
                              acc_stats(zsb[h][:], sA[:, h:h + 1], qA[:, h:h + 1],
                                        sb, n=BLK)
                              nc.sync.dma_start(z_scr[0][h, :, b * BLK:(b + 1) * BLK],
                                                zsb[h][:])
                  stg = allreduce_stats(sA, qA, 2, sb)
                  A1, C1 = affine_from_stats(stg, 2, c2b[0], c2gn[0], sb)

                  # ---- layer 2 ----
                  s2 = sb.tile([128, 2], F32, tag="c2s2")
                  q2 = sb.tile([128, 2], F32, tag="c2q2")
                  nc.vector.memset(s2[:], 0.0)
                  nc.vector.memset(q2[:], 0.0)
                  zlast = [None, None]
                  with tc.tile_pool(name="c2mid", bufs=2) as mp, \
                       tc.tile_pool(name="c2ps", bufs=2, space="PSUM") as ps:
                      w2s = []
                      for ki in range(2):
                          for mo in range(2):
                              w = sb.tile([128, 128], BF16, tag=f"c2w2{ki}{mo}")
                              nc.sync.dma_start(w[:], t_c2w2[ki, mo])
                              w2s.append(w)
                      for b in range(NBLK):
                          h1 = []
                          for mb in range(2):
                              z = mp.tile([128, BLK], BF16, tag=f"c2z1r{mb}")
                              nc.sync.dma_start(z[:], z_scr[0][mb, :, b * BLK:(b + 1) * BLK])
                              hh = mp.tile([128, BLK], BF16, tag=f"c2h1{mb}")
                              nc.vector.tensor_scalar(hh[:], z[:], A1[mb], C1[mb],
                                                      AOP.mult, AOP.add)
                              nc.vector.tensor_scalar(hh[:], hh[:], 0.0, None, AOP.max)
                              h1.append(hh)
                          for mo in range(2):
                              zw = mp.tile([128, BLK], BF16, tag=f"c2z2w{mo}")
                              for s in range(NSEG):
                                  zp = ps.tile([128, 512], F32, tag="c2zp")
                                  for ki in range(2):
                                      nc.tensor.matmul(zp[:], w2s[ki * 2 + mo][:],
                                                       h1[ki][:, s * 512:(s + 1) * 512],
                                                       start=(ki == 0), stop=(ki == 1))
                                  nc.scalar.copy(zw[:, s * 512:(s + 1) * 512], zp[:])
                              acc_stats(zw[:], s2[:, mo:mo + 1], q2[:, mo:mo + 1],
                                        sb, n=BLK)
                              nc.sync.dma_start(z_scr[1][mo, :, b * BLK:(b + 1) * BLK], zw[:])
                              zlast[mo] = zw
                      zsent = []
                      for mo in range(2):
                          zc = sb.tile([128, 1], F32, tag=f"c2zs{mo}")
                          nc.vector.tensor_copy(zc[:], zlast[mo][:, BLK - 1:BLK])
                          zsent.append(zc[:])
                  sentinel_correct(s2, q2, zsent, 2, sb)
                  stg2 = allreduce_stats(s2, q2, 2, sb)
                  A2, C2 = affine_from_stats(stg2, 2, c2b[1], c2gn[1], sb)

                  # ---- scatter + A3/B3 tables ----
                  with tc.tile_pool(name="s2sb", bufs=2) as sp, \
                       tc.tile_pool(name="s2oh", bufs=2) as ohp, \
                       tc.tile_pool(name="s2tp", bufs=2, space="PSUM") as ps_tp, \
                       tc.tile_pool(name="s2sc", bufs=2, space="PSUM") as ps_sc:
                      wab3 = sp.tile([128, 1024], BF16, tag="wab3")
                      for ki in range(2):
                          nc.sync.dma_start(wab3[:, ki * 512:ki * 512 + 256], t_wa3[ki])
                          nc.sync.dma_start(wab3[:, ki * 512 + 256:ki * 512 + 512],
                                            t_wb3[ki])
                      for b in range(NBLK):
                          hs = []
                          for mb in range(2):
                              z = sp.tile([128, BLK], BF16, tag=f"s2z{mb}")
                              nc.sync.dma_start(z[:], z_scr[1][mb, :, b * BLK:(b + 1) * BLK])
                              h = sp.tile([128, BLK], BF16, tag=f"s2h{mb}")
                              nc.scalar.activation(h[:], z[:], AFT.Relu,
                                                   bias=C2[mb], scale=A2[mb])
                              hs.append(h)
                          ohb = ohp.tile([128, BLK], BF16, tag="ohb2")
                          nc.sync.dma_start(ohb[:], t_oh[:, b * BLK:(b + 1) * BLK])
                          for w in range(NW_BLK):
                              gw = b * NW_BLK + w
                              tpp = ps_tp.tile([128, 1024], BF16, tag="tpp2", space="PSUM")
                              for cb in range(B):
                                  for mb in range(2):
                                      nc.tensor.transpose(
                                          tpp[:, (cb * 2 + mb) * 128:(cb * 2 + mb + 1) * 128],
                                          hs[mb][:, (w * B + cb) * 128:(w * B + cb + 1) * 128],
                                          ident[:])
                              hE = sp.tile([128, 1024], BF16, tag="hE2")
                              nc.vector.tensor_copy(hE[:], tpp[:])
                              sc = ps_sc.tile([128, 256], F32, tag="sc2", space="PSUM")
                              for cb in range(B):
                                  nc.tensor.matmul(
                                      sc[:], ohb[:, (w * B + cb) * 128:(w * B + cb + 1) * 128],
                                      hE[:, cb * 256:(cb + 1) * 256],
                                      start=(cb == 0), stop=(cb == B - 1))
                              nt = sp.tile([128, 256], BF16, tag="nt2")
                              nc.vector.tensor_scalar(nt[:], sc[:], invcnt[:, gw:gw + 1],
                                                      None, AOP.mult)
                              if debug:
                                  nc.sync.dma_start(dbg["x2"][gw * WIN:(gw + 1) * WIN, :], nt[:])
                              ntp = ps_tp.tile([128, 256], BF16, tag="ntp2", space="PSUM")
                              for ki in range(2):
                                  nc.tensor.transpose(ntp[:, ki * 128:(ki + 1) * 128],
                                                      nt[:, ki * 128:(ki + 1) * 128],
                                                      ident[:])
                              ntT = sp.tile([128, 256], BF16, tag="ntT2")
                              nc.vector.tensor_copy(ntT[:], ntp[:])
                              abp = ps_sc.tile([128, 512], F32, tag="abp", space="PSUM")
                              for ki in range(2):
                                  nc.tensor.matmul(abp[:], ntT[:, ki * 128:(ki + 1) * 128],
                                                   wab3[:, ki * 512:(ki + 1) * 512],
                                                   start=(ki == 0), stop=(ki == 1))
                              abt = sp.tile([128, 512], BF16, tag="abt")
                              nc.vector.tensor_copy(abt[:], abp[:])
                              nc.sync.dma_start(a3_loc[gw * WIN:(gw + 1) * WIN, :],
                                                abt[:, 0:256])
                              nc.sync.dma_start(b3_loc[gw * WIN:(gw + 1) * WIN, :],
                                                abt[:, 256:512])

            nc.gpsimd.collective_compute(
                "AllGather", AOP.bypass, replica_groups=[list(range(NC))],
                ins=[b3_loc.opt()], outs=[b3_full.opt()])

            # ======================= CONV 3 =======================
            if phases >= 3:
              with tc.tile_pool(name="c3sb", bufs=2) as sb:
                  c3b = [load_vec(t_c3b[mb], sb, f"c3b{mb}") for mb in range(2)]
                  c3gn = [[load_vec(t_c3gn[j, mb], sb, f"c3gn{j}{mb}") for mb in range(2)]
                          for j in range(3)]
                  G1 = sb.tile([128, ZW], F32, tag="G1")
                  G2 = sb.tile([128, ZW], F32, tag="G2")
                  nc.vector.memset(G1[:], 0.0)
                  nc.vector.memset(G2[:], 0.0)
                  # ---- pass 1: z = A3[dst] + B3[src]; Gram stats; spill z ----
                  with tc.tile_pool(name="c3g", bufs=2) as g3, \
                       tc.tile_pool(name="c3q", bufs=1) as gqp3, \
                       tc.tile_pool(name="c3zt", bufs=2) as ztp, \
                       tc.tile_pool(name="c3ps", bufs=4, space="PSUM") as ps, \
                       tc.tile_pool(name="c3gp", bufs=2, space="PSUM") as psg:
                      xw3 = load_xjw(g3)
                      for b in range(NBLK):
                          gq = [gqp3.tile([128, NCHUNK * 256], BF16, tag=f"g3q{q}",
                                          name=f"g3q{q}") for q in range(4)]
                          gb = quad_gather(gq, b3_full, b, 256, xw3)
                          zt = ztp.tile([128, NCHUNK * ZW], BF16, tag="zt")
                          ones_ap = zt[:].rearrange("p (c k) -> p c k", k=ZW)[:, :, 256:257]
                          nc.vector.memset(ones_ap, 1.0)
                          ohTb = g3.tile([128, BLK], BF16, tag="ohTb3")
                          nc.sync.dma_start(ohTb[:], t_ohT[:, b * BLK:(b + 1) * BLK])
                          for w in range(NW_BLK):
                              gw = b * NW_BLK + w
                              a3w = g3.tile([128, 256], BF16, tag="a3w")
                              nc.sync.dma_start(a3w[:], a3_loc[gw * WIN:(gw + 1) * WIN, :])
                              for cb in range(B):
                                  ch = w * B + cb
                                  ap_ = ps.tile([128, 256], F32, tag="aexp")
                                  nc.tensor.matmul(ap_[:],
                                                   ohTb[:, ch * 128:(ch + 1) * 128],
                                                   a3w[:], start=True, stop=True)
                                  nc.vector.tensor_tensor(
                                      zt[:, ch * ZW:ch * ZW + 256],
                                      gb[:, ch * 256:(ch + 1) * 256],
                                      ap_[:], op=AOP.add)
                          G1p = psg.tile([128, ZW], F32, tag="G1p", space="PSUM")
                          G2p = psg.tile([128, ZW], F32, tag="G2p", space="PSUM")
                          for ch in range(NCHUNK):
                              nc.tensor.matmul(G1p[:], zt[:, ch * ZW:ch * ZW + 128],
                                               zt[:, ch * ZW:ch * ZW + ZW],
                                               start=(ch == 0), stop=(ch == NCHUNK - 1),
                                               skip_group_check=True)
                              nc.tensor.matmul(G2p[:], zt[:, ch * ZW + 128:ch * ZW + 256],
                                               zt[:, ch * ZW:ch * ZW + ZW],
                                               start=(ch == 0), stop=(ch == NCHUNK - 1),
                                               skip_group_check=True)
                          nc.vector.tensor_tensor(G1[:], G1[:], G1p[:], op=AOP.add)
                          nc.vector.tensor_tensor(G2[:], G2[:], G2p[:], op=AOP.add)
                          nc.sync.dma_start(
                              z3_scr[:, b * NCHUNK * ZW:(b + 1) * NCHUNK * ZW], zt[:])
                  # stats: sums = G[:,256]; sumsq = diag
                  sA = sb.tile([128, 2], F32, tag="c3sA")
                  qA = sb.tile([128, 2], F32, tag="c3qA")
                  nc.vector.tensor_copy(sA[:, 0:1], G1[:, 256:257])
                  nc.vector.tensor_copy(sA[:, 1:2], G2[:, 256:257])
                  dtmp = sb.tile([128, 128], F32, tag="dtmp")
                  nc.vector.tensor_tensor(dtmp[:], G1[:, 0:128], eyef[:], op=AOP.mult)
                  nc.vector.reduce_sum(out=qA[:, 0:1], in_=dtmp[:], axis=AX.X)
                  nc.vector.tensor_tensor(dtmp[:], G2[:, 128:256], eyef[:], op=AOP.mult)
                  nc.vector.reduce_sum(out=qA[:, 1:2], in_=dtmp[:], axis=AX.X)
                  stg = allreduce_stats(sA, qA, 2, sb)
                  A1, C1 = affine_from_stats(stg, 2, c3b, c3gn, sb)
                  # broadcast affine rows: a_bc/c_bc [128, ZW] bf16
                  a_bc = sb.tile([128, ZW], BF16, tag="a_bc")
                  c_bc = sb.tile([128, ZW], BF16, tag="c_bc")
                  with tc.tile_pool(name="c3bp", bufs=2, space="PSUM") as psb:
                      for dst_t, vals in ((a_bc, A1), (c_bc, C1)):
                          nc.vector.memset(dst_t[:], 0.0)
                          for mb in range(2):
                              tp = psb.tile([128, 128], F32, tag="bcp", space="PSUM")
                              nc.tensor.transpose(
                                  tp[:], vals[mb][:, 0:1].to_broadcast([128, 128]),
                                  eyef[:])
                              nc.vector.tensor_copy(dst_t[:, mb * 128:(mb + 1) * 128], tp[:])
                  # ---- pass 2: h = relu(a*z + c); scatter ----
                  with tc.tile_pool(name="c3p2", bufs=2) as p2, \
                       tc.tile_pool(name="c3oh", bufs=2) as ohp, \
                       tc.tile_pool(name="c3sc", bufs=2, space="PSUM") as ps_sc:
                      for b in range(NBLK):
                          zt = p2.tile([128, NCHUNK * ZW], BF16, tag="zt2")
                          nc.sync.dma_start(
                              zt[:], z3_scr[:, b * NCHUNK * ZW:(b + 1) * NCHUNK * ZW])
                          h = p2.tile([128, NCHUNK * ZW], BF16, tag="h3b")
                          nc.vector.tensor_tensor(
                              out=h[:].rearrange("p (c k) -> p c k", k=ZW),
                              in0=zt[:].rearrange("p (c k) -> p c k", k=ZW),
                              in1=a_bc[:, None, :].to_broadcast((128, NCHUNK, ZW)),
                              op=AOP.mult)
                          nc.vector.tensor_tensor(
                              out=h[:].rearrange("p (c k) -> p c k", k=ZW),
                              in0=h[:].rearrange("p (c k) -> p c k", k=ZW),
                              in1=c_bc[:, None, :].to_broadcast((128, NCHUNK, ZW)),
                              op=AOP.add)
                          nc.scalar.activation(h[:], h[:], AFT.Relu)
                          ohb = ohp.tile([128, BLK], BF16, tag="ohb3")
                          nc.sync.dma_start(ohb[:], t_oh[:, b * BLK:(b + 1) * BLK])
                          for w in range(NW_BLK):
                              gw = b * NW_BLK + w
                              sc = ps_sc.tile([128, 256], F32, tag="sc3", space="PSUM")
                              for cb in range(B):
                                  ch = w * B + cb
                                  nc.tensor.matmul(
                                      sc[:], ohb[:, ch * 128:(ch + 1) * 128],
                                      h[:, ch * ZW:ch * ZW + 256],
                                      start=(cb == 0), stop=(cb == B - 1))
                              nt = p2.tile([128, 256], BF16, tag="nt3")
                              nc.vector.tensor_scalar(nt[:], sc[:], invcnt[:, gw:gw + 1],
                                                      None, AOP.mult)
                              nc.sync.dma_start(tab3_loc[gw * WIN:(gw + 1) * WIN, :], nt[:])

            if debug:
                nc.sync.dma_start(dbg["x3"][:], tab3_loc[:])

            # ======================= POOL + HEAD =======================
            if phases >= 4:
              with tc.tile_pool(name="p_sb", bufs=2) as sb, \
                 tc.tile_pool(name="p_ps", bufs=2, space="PSUM") as ps:
                  for gw in range(8):
                      pidx = sb.tile([128, Bg * 8], mybir.dt.int16, tag="p_idx")
                      nc.sync.dma_start(pidx[:], t_pidxw[gw])
                      poh = sb.tile([128, Bg * 128], BF16, tag="p_poh")
                      nc.sync.dma_start(poh[:],
                                        t_poh[:, gw * Bg * 128:(gw + 1) * Bg * 128])
                      gp = sb.tile([128, Bg * 256], BF16, tag="p_gp")
                      g0 = 0
                      while g0 < Bg * 128:
                          gn = min(1024, Bg * 128 - g0)
                          nc.gpsimd.dma_gather(
                              out_ap=gp[:, (g0 // 128) * 256:((g0 + gn) // 128) * 256]
                              .rearrange("p (j c) -> p j c", c=256),
                              in_ap=tab3_loc[:],
                              idxs_ap=pidx[:, g0 // 16:(g0 + gn) // 16],
                              num_idxs=gn, num_idxs_reg=gn, elem_size=256)
                          g0 += gn
                      pp = ps.tile([128, 256], F32, tag="p_pp", space="PSUM")
                      for c in range(Bg):
                          nc.tensor.matmul(pp[:], poh[:, c * 128:(c + 1) * 128],
                                           gp[:, c * 256:(c + 1) * 256],
                                           start=(c == 0), stop=(c == Bg - 1))
                      pf = sb.tile([128, 256], F32, tag="p_pf")
                      nc.vector.tensor_copy(pf[:], pp[:])
                      nc.sync.dma_start(pool_in[gw * 128:(gw + 1) * 128, :], pf[:])
                  nc.gpsimd.collective_compute(
                      "AllReduce", AOP.add, replica_groups=[list(range(NC))],
                      ins=[pool_in.opt()], outs=[pool_out.opt()])
                  if debug:
                      nc.sync.dma_start(dbg["pool"][:], pool_out[:])

                  invg = sb.tile([128, 8], F32, tag="p_invg")
                  nc.sync.dma_start(invg[:], t_invg[:])
                  lw1 = []
                  for ki in range(2):
                      for mo in range(2):
                          w = sb.tile([128, 128], BF16, tag=f"p_lw1{ki}{mo}")
                          nc.sync.dma_start(w[:], t_lw1[ki, mo])
                          lw1.append(w)
                  lw2 = []
                  for ki in range(2):
                      w = sb.tile([128, 2], BF16, tag=f"p_lw2{ki}")
                      nc.sync.dma_start(w[:], t_lw2[ki])
                      lw2.append(w)
                  lb1 = [load_vec(t_lb1[mb], sb, f"p_lb1{mb}") for mb in range(2)]
                  lb2 = sb.tile([2, 1], F32, tag="p_lb2")
                  nc.sync.dma_start(lb2[:], t_lb2[:])
                  ofin = sb.tile([2, 1024], F32, tag="p_out")
                  for gw in range(8):
                      g = sb.tile([128, 256], F32, tag="p_g")
                      nc.sync.dma_start(g[:], pool_out[gw * 128:(gw + 1) * 128, :])
                      gm = sb.tile([128, 256], BF16, tag="p_gm")
                      nc.vector.tensor_scalar(gm[:], g[:], invg[:, gw:gw + 1], None, AOP.mult)
                      gT = sb.tile([128, 2 * 128], BF16, tag="p_gT")
                      for kb in range(2):
                          tp = ps.tile([128, 128], BF16, tag="p_tp", space="PSUM")
                          nc.tensor.transpose(tp[:], gm[:, kb * 128:(kb + 1) * 128], ident[:])
                          nc.vector.tensor_copy(gT[:, kb * 128:(kb + 1) * 128], tp[:])
                      hT = sb.tile([128, 2 * 128], BF16, tag="p_hT")
                      for mo in range(2):
                          hp = ps.tile([128, 128], F32, tag="p_hp", space="PSUM")
                          for ki in range(2):
                              nc.tensor.matmul(hp[:], lw1[ki * 2 + mo][:],
                                               gT[:, ki * 128:(ki + 1) * 128],
                                               start=(ki == 0), stop=(ki == 1))
                          nc.scalar.activation(hT[:, mo * 128:(mo + 1) * 128], hp[:],
                                               AFT.Relu, bias=lb1[mo])
                      op_ = ps.tile([2, 128], F32, tag="p_op", space="PSUM")
                      for ki in range(2):
                          nc.tensor.matmul(op_[:], lw2[ki][:],
                                           hT[:, ki * 128:(ki + 1) * 128],
                                           start=(ki == 0), stop=(ki == 1))
                      nc.vector.tensor_scalar(ofin[:, gw * 128:(gw + 1) * 128],
                                              op_[:], lb2[:], None, AOP.add)
                  nc.sync.dma_start(o_out[:], ofin[:, :N_GRAPHS])

    nc.compile()
    return nc


# ============================ entry point ============================


def kernel(**inputs):
    x = np.asarray(inputs["x"], dtype=np.float32)
    edge_index = np.asarray(inputs["edge_index"])
    batch = np.asarray(inputs["batch"])

    meta = _pack(edge_index, batch)
    Bg = meta["Bg"]

    import os as _os
    phases = int(_os.environ.get("KPHASES", "4"))
    key = ("mod", Bg, phases, _DEBUG[0])
    if key not in _cache:
        _cache[key] = _build(Bg, debug=bool(inputs.get("_debug", False)) or _DEBUG[0],
                             phases=phases)
    nc = _cache[key]

    # ---- per-core input arrays ----
    src = np.asarray(edge_index[0], dtype=np.int64)
    dst = np.asarray(edge_index[1], dtype=np.int64)

    # conv1 msgT: [core, 48, E_PAD//2] bf16; edge e<EHALF -> rows 0..9 col e,
    # e>=EHALF -> rows 32..41 col e-EHALF
    xi_v = x[dst]
    xj_v = x[src]
    msg = np.concatenate([xi_v, xj_v - xi_v], axis=1)       # [E, 10]
    msg_full = np.zeros((NC, E_PAD, 10), dtype=np.float32)
    ec, pos = meta["ec"], meta["pos"]
    msg_full[ec, pos] = msg[meta["eorder"]]
    msgT = np.zeros((NC, 48, EHALF), dtype=ml_dtypes.bfloat16)
    msgT[:, :10, :] = _bf(msg_full[:, :EHALF].transpose(0, 2, 1))
    msgT[:, 32:42, :] = _bf(msg_full[:, EHALF:].transpose(0, 2, 1))

    # one-hot tables from dstwin
    dstwin = meta["dstwin"]  # [NC, E_PAD], float; -1 for padding
    dw = dstwin.reshape(NC, NCH_ALL, 128).astype(np.int32)
    nn_ = np.arange(128, dtype=np.int32)
    oh_in = np.empty((NC, 128, E_PAD), dtype=ml_dtypes.bfloat16)
    ohT_in = np.empty((NC, 128, E_PAD), dtype=ml_dtypes.bfloat16)
    for c in range(NC):
        m = (dw[c][:, :, None] == nn_[None, None, :])     # [392, 128e, 128n]
        oh_in[c] = m.transpose(1, 0, 2).reshape(128, E_PAD).astype(ml_dtypes.bfloat16)
        ohT_in[c] = m.transpose(2, 0, 1).reshape(128, E_PAD).astype(ml_dtypes.bfloat16)

    invcnt_in = np.ascontiguousarray(
        meta["inv_cnt"].reshape(NC, NWIN, 128).transpose(0, 2, 1)).astype(np.float32)
    padcnt_in = np.repeat(meta["pad_cnt"][:, None], 128, axis=1)[:, :, None].astype(np.float32)

    ident_in = np.eye(128, dtype=np.float32).astype(ml_dtypes.bfloat16)
    eyef_in = np.eye(128, dtype=np.float32)

    xq = meta["xj_glob"].astype(np.int64)        # [NC, E_PAD], pad -> gzero
    qz = meta["qzero"]
    xjw_in = np.empty((NC, 4, 128, E_PAD // 16), dtype=np.int16)
    for q in range(4):
        inq = (xq >= q * QUAD) & (xq < (q + 1) * QUAD)
        idx_q = np.where(inq, xq - q * QUAD, qz[q])
        xjw_in[:, q] = _wrap_idx(idx_q)

    # weights
    c1w = np.zeros((3, 128, 128), dtype=ml_dtypes.bfloat16)
    c1w[0, :10, :] = _bf(inputs["c1_w1"])
    c1w[0, 32:42, :] = _bf(inputs["c1_w1"])
    c1w[1] = _bf(inputs["c1_w2"])
    c1w[2] = _bf(inputs["c1_w3"])
    c1b = np.stack([np.asarray(inputs[f"c1_b{i}"], dtype=np.float32).reshape(128, 1)
                    for i in (1, 2, 3)])
    c1gn = np.stack([np.asarray(inputs[f"c1_gn{i}"], dtype=np.float32).reshape(3, 128, 1)
                     for i in (1, 2, 3)])

    w2a = np.asarray(inputs["c2_w1"], dtype=np.float32)   # [256, 256]
    WA2 = w2a[:128] - w2a[128:]
    WB2 = w2a[128:]
    wa2 = _bf(WA2)                                        # [128, 256]
    c2wb = _tile_w(WB2)[0]                                # [2(mo), 128, 128]
    c2w2 = _tile_w(np.asarray(inputs["c2_w2"], dtype=np.float32))  # [2,2,128,128]
    c2b = np.stack([np.asarray(inputs["c2_b1"], dtype=np.float32).reshape(2, 128, 1),
                    np.asarray(inputs["c2_b2"], dtype=np.float32).reshape(2, 128, 1)])
    c2gn = np.stack([np.asarray(inputs["c2_gn1"], dtype=np.float32).reshape(3, 2, 128, 1),
                     np.asarray(inputs["c2_gn2"], dtype=np.float32).reshape(3, 2, 128, 1)])

    w3a = np.asarray(inputs["c3_w1"], dtype=np.float32)   # [512, 256]
    WA3 = w3a[:256] - w3a[256:]
    WB3 = w3a[256:]
    wa3 = _bf(WA3).reshape(2, 128, 256)
    wb3 = _bf(WB3).reshape(2, 128, 256)
    c3b = np.asarray(inputs["c3_b1"], dtype=np.float32).reshape(2, 128, 1)
    c3gn = np.asarray(inputs["c3_gn1"], dtype=np.float32).reshape(3, 2, 128, 1)

    lw1 = _tile_w(np.asarray(inputs["lin_w1"], dtype=np.float32))
    lb1 = np.asarray(inputs["lin_b1"], dtype=np.float32).reshape(2, 128, 1)
    lw2_f = np.asarray(inputs["lin_w2"], dtype=np.float32)  # [256, 2]
    lw2 = np.stack([_bf(lw2_f[:128]), _bf(lw2_f[128:])])    # [2, 128, 2]
    lb2 = np.asarray(inputs["lin_b2"], dtype=np.float32).reshape(2, 1)

    pidxw_in = _wrap_idx(meta["pool_idx"].astype(np.int64))  # [NC, 8, 128, NPG//16]
    pgwl = meta["pool_gwl"].reshape(NC, 8, Bg, 128)        # [c, gw, cs, p]
    gg = np.arange(128, dtype=np.float32)
    poh_in = np.empty((NC, 128, 8 * Bg * 128), dtype=ml_dtypes.bfloat16)
    for c in range(NC):
        m = (pgwl[c][:, :, :, None] == gg[None, None, None, :])  # [8, Bg, 128p, 128g]
        poh_in[c] = m.transpose(2, 0, 1, 3).reshape(128, 8 * Bg * 128).astype(
            ml_dtypes.bfloat16)
    invg_in = np.broadcast_to(
        meta["inv_g"].reshape(8, 128).T[None], (NC, 128, 8)).astype(np.float32)
    invg_in = np.ascontiguousarray(invg_in)

    in_maps = []
    for c in range(NC):
        im = {
            "msgT": msgT[c],
            "xjw": xjw_in[c],
            "pidxw": pidxw_in[c],
            "ohtab": oh_in[c],
            "ohTtab": ohT_in[c],
            "invcnt": invcnt_in[c],
            "padcnt": padcnt_in[c],
            "ident": ident_in,
            "eyef": eyef_in,
            "c1w": c1w, "c1b": c1b, "c1gn": c1gn,
            "wa2": wa2, "c2wb": c2wb, "c2w2": c2w2, "c2b": c2b, "c2gn": c2gn,
            "wa3": wa3, "wb3": wb3, "c3b": c3b, "c3gn": c3gn,
            "lw1": lw1, "lb1": lb1, "lw2": lw2, "lb2": lb2,
            "poolohtab": poh_in[c],
            "invg": invg_in[c],
        }
        in_maps.append(im)

    res = run_bass_kernel_spmd(nc, in_maps, core_ids=list(range(NC)),
                               trace=_TRACE[0])
    kernel.last_result = res
    kernel.last_meta = meta
    out = res.results[0]["out"]            # [2, 1000]
    return np.ascontiguousarray(out.T).astype(np.float32)


_DEBUG = [False]
_TRACE = [False]


# revision 19
# speedup vs baseline: 1.8875x; 1.8875x over previous
"""LundNetTagger GNN on 8 Trainium2 NeuronCores (Bass/Tile).

Self-contained: kernel(**inputs) -> np.ndarray [1000, 2] float32.

Strategy: nodes are assigned to 100352 "slots" (8 cores x 98 windows x 128),
packed so each window receives <= 512 edges. Edges live on the core owning
their dst slot, in window-major order padded to 4x128-edge chunks per window.
EdgeConv cat[xi, xj-xi] is folded into split weights WA = W[:C]-W[C:],
WB = W[C:].

conv1 runs feature-major from a host-packed message tensor, keeping z in SBUF.
conv2 layer1 expands the xi term from a per-node table A2 = x1 @ WA2 via
host-precomputed transposed one-hot masks (no per-edge xi matmuls) and gathers
xj rows of x1. conv3 is fully table-based: z_e = A3[dst] + B3[src] with
A3/B3 = x2 @ WA3 / x2 @ WB3 computed during conv2's scatter; the gather then
yields z directly in [edge, channel] layout, GraphNorm stats come from a
Gram-matrix matmul (sum + sum-of-squares in one accumulation), and the scatter
consumes [edge, channel] tiles with zero transposes. One-hot masks for
scatter/expansion/pool are host-precomputed bf16 tables. GraphNorm stats are
global AllReduces; mean-aggregation is a collision-free one-hot matmul scatter
into PSUM per window.
"""
import numpy as np
import ml_dtypes

import concourse.bass as bass
import concourse.tile as tile
from concourse import bacc, mybir
from concourse.bass_utils import run_bass_kernel_spmd

BF16 = mybir.dt.bfloat16
F32 = mybir.dt.float32
AOP = mybir.AluOpType
AFT = mybir.ActivationFunctionType
AX = mybir.AxisListType

N_NODES = 100000
N_EDGES = 400000
N_GRAPHS = 1000
NC = 8
WIN = 128
NWIN = 98
SPC = WIN * NWIN          # 12544
NSLOTS = SPC * NC         # 100352
QUAD = NSLOTS // 4        # 25088
B = 4                     # chunks per window
EPW = B * WIN             # 512
E_PAD = NWIN * EPW        # 50176
EPS = 1e-5

NW_BLK = 7
BLK = NW_BLK * EPW        # 3584
NBLK = NWIN // NW_BLK     # 14
NCHUNK = BLK // 128       # 28
NSEG = BLK // 512         # 7
NCH_ALL = E_PAD // 128    # 392
ZW = 257                  # z3 chunk width (256 channels + ones column)


_cache = {}


# ============================ host-side packing ============================

def _pack(edge_index, batch):
    src = np.asarray(edge_index[0], dtype=np.int64)
    dst = np.asarray(edge_index[1], dtype=np.int64)
    batch = np.asarray(batch, dtype=np.int64)
    cnt = np.bincount(dst, minlength=N_NODES)

    nvirt = NSLOTS - N_NODES
    cnt_all = np.concatenate([cnt, np.zeros(nvirt, dtype=cnt.dtype)])
    order = np.argsort(-cnt_all, kind="stable")
    GW = NWIN * NC
    rounds = NSLOTS // GW
    win_of_rank = np.empty(NSLOTS, dtype=np.int64)
    for r in range(rounds):
        seg = np.arange(GW) if r % 2 == 0 else np.arange(GW - 1, -1, -1)
        win_of_rank[r * GW:(r + 1) * GW] = seg
    win_of_node = np.empty(NSLOTS, dtype=np.int64)
    win_of_node[order] = win_of_rank
    wsum = np.bincount(win_of_node, weights=cnt_all.astype(np.float64),
                       minlength=GW).astype(np.int64)

    cap = EPW
    members_of = [list(np.where(win_of_node == w)[0]) for w in range(GW)]
    for _ in range(2000):
        over = np.where(wsum > cap)[0]
        if len(over) == 0:
            break
        w = int(over[0])
        # smallest-count >0 node in w
        mem = members_of[w]
        cs = [(int(cnt_all[n]), n) for n in mem if cnt_all[n] > 0]
        cs.sort()
        moved = False
        for c1, n in cs:
            # find target window with a smaller-count node to swap
            worder2 = np.argsort(wsum)
            for tw in worder2[:64]:
                tw = int(tw)
                if tw == w:
                    continue
                tmem = members_of[tw]
                best = None
                for m in tmem:
                    c2 = int(cnt_all[m])
                    if c2 < c1 and wsum[tw] + c1 - c2 <= cap:
                        if best is None or c2 < best[0]:
                            best = (c2, m)
                        if c2 == 0:
                            break
                if best is not None:
                    c2, m = best
                    members_of[tw].remove(m)
                    members_of[tw].append(n)
                    members_of[w].remove(n)
                    members_of[w].append(m)
                    win_of_node[n] = tw
                    win_of_node[m] = w
                    wsum[tw] += c1 - c2
                    wsum[w] -= c1 - c2
                    moved = True
                    break
            if moved:
                break
        if not moved:
            raise RuntimeError("packing fixup stuck")
    assert wsum.max() <= cap, f"window packing failed: max={wsum.max()}"

    worder = np.argsort(-wsum, kind="stable")
    core_load = np.zeros(NC, dtype=np.int64)
    core_nwin = np.zeros(NC, dtype=np.int64)
    core_of_win = np.empty(GW, dtype=np.int64)
    for w in worder:
        cands = np.where(core_nwin < NWIN)[0]
        c = cands[np.argmin(core_load[cands])]
        core_of_win[w] = c
        core_load[c] += wsum[w]
        core_nwin[c] += 1

    win_lists = [[] for _ in range(NC)]
    for w in range(GW):
        win_lists[core_of_win[w]].append(w)
    for c in range(NC):
        wl = win_lists[c]
        j = int(np.argmin(wsum[wl]))
        assert wsum[wl[j]] < cap, "no sentinel room"
        wl[j], wl[-1] = wl[-1], wl[j]

    slot_of_node = np.empty(NSLOTS, dtype=np.int64)
    for c in range(NC):
        for wi, w in enumerate(win_lists[c]):
            mem = np.sort(np.array(members_of[w], dtype=np.int64))
            assert len(mem) == WIN
            slot_of_node[mem] = c * SPC + wi * WIN + np.arange(WIN)
    node_of_slot = np.empty(NSLOTS, dtype=np.int64)
    node_of_slot[slot_of_node] = np.arange(NSLOTS)
    cnt_of_slot = cnt_all[node_of_slot]

    qzero = []
    for q in range(4):
        z = np.where(cnt_of_slot[q * QUAD:(q + 1) * QUAD] == 0)[0]
        assert len(z) > 0
        qzero.append(int(z[0]))  # local to quadrant
    czero = []
    for c in range(NC):
        z = np.where(cnt_of_slot[c * SPC:(c + 1) * SPC] == 0)[0]
        assert len(z) > 0
        czero.append(int(z[0]))  # local to core

    dslot = slot_of_node[dst]
    sslot = slot_of_node[src]
    ecore = dslot // SPC
    ewin = (dslot % SPC) // WIN
    key = ecore * (NWIN * WIN) + ewin * WIN + (dslot % WIN)
    eorder = np.argsort(key, kind="stable")
    dsl, ssl = dslot[eorder], sslot[eorder]
    ec, ew = ecore[eorder], ewin[eorder]

    cw = ec * NWIN + ew
    cw_cnt = np.bincount(cw, minlength=NC * NWIN)
    assert cw_cnt.max() <= EPW

    xi_idx = np.zeros((NC, E_PAD), dtype=np.int64)
    xj_idx = np.zeros((NC, E_PAD), dtype=np.int64)
    dstwin = np.full((NC, E_PAD), -1.0, dtype=np.float32)
    valid = np.zeros((NC, E_PAD), dtype=bool)

    ofs = (np.arange(NC * NWIN) % NWIN) * EPW
    start = np.concatenate([[0], np.cumsum(cw_cnt)[:-1]])
    within = np.arange(N_EDGES) - start[cw]
    pos = ofs[cw] + within
    xi_idx[ec, pos] = dsl % SPC
    xj_idx[ec, pos] = ssl
    dstwin[ec, pos] = (dsl % WIN).astype(np.float32)
    valid[ec, pos] = True
    for c in range(NC):
        xi_idx[c, ~valid[c]] = czero[c]
    pad_cnt = (~valid).sum(axis=1).astype(np.float32)
    assert np.all(~valid[:, -1]), "sentinel column must be padding"

    gzero = qzero[0]  # global slot with zero row
    xj_glob = np.where(valid, xj_idx, gzero).astype(np.int32)

    inv_cnt = (1.0 / np.maximum(cnt_of_slot.reshape(NC, SPC), 1.0)).astype(np.float32)

    g_of_slot = np.full(NSLOTS, -1, dtype=np.int64)
    real = node_of_slot < N_NODES
    g_of_slot[real] = batch[node_of_slot[real]]
    NGW = 8
    Bg = 0
    pools = [[None] * NGW for _ in range(NC)]
    for c in range(NC):
        gl = g_of_slot[c * SPC:(c + 1) * SPC]
        for gw in range(NGW):
            m = np.where((gl >= gw * 128) & (gl < (gw + 1) * 128))[0]
            pools[c][gw] = m
            Bg = max(Bg, (len(m) + 127) // 128)
    NPG = Bg * 128
    pool_idx = np.zeros((NC, NGW, NPG), dtype=np.int16)
    pool_gwl = np.full((NC, NGW, NPG), -1.0, dtype=np.float32)
    for c in range(NC):
        for gw in range(NGW):
            m = pools[c][gw]
            pool_idx[c, gw, :len(m)] = m.astype(np.int16)
            pool_idx[c, gw, len(m):] = czero[c]
            pool_gwl[c, gw, :len(m)] = (g_of_slot[c * SPC + m] - gw * 128).astype(np.float32)

    gcnt = np.bincount(batch, minlength=N_GRAPHS).astype(np.float32)
    inv_g = np.zeros(1024, dtype=np.float32)
    inv_g[:N_GRAPHS] = 1.0 / np.maximum(gcnt, 1.0)

    return dict(slot_of_node=slot_of_node, node_of_slot=node_of_slot,
                xj_glob=xj_glob, dstwin=dstwin, pad_cnt=pad_cnt,
                inv_cnt=inv_cnt, valid=valid, eorder=eorder, ec=ec, pos=pos,
                pool_idx=pool_idx, pool_gwl=pool_gwl, inv_g=inv_g, Bg=Bg,
                qzero=qzero)


def _wrap_idx(a):
    """[.., n] int -> [.., 128, n//16]: element i -> partition i%16 col i//16,
    replicated to 8 groups of 16 partitions."""
    n = a.shape[-1]
    assert n % 16 == 0
    w = a.reshape(*a.shape[:-1], n // 16, 16)
    w = np.swapaxes(w, -1, -2)
    w = np.broadcast_to(w[..., None, :, :], (*a.shape[:-1], 8, 16, n // 16))
    return np.ascontiguousarray(w).reshape(*a.shape[:-1], 128, n // 16).astype(np.int16)


def _bf(x):
    return np.ascontiguousarray(np.asarray(x, dtype=np.float32)).astype(ml_dtypes.bfloat16)


def _tile_w(w):
    K, M = w.shape
    nk, nm = (K + 127) // 128, (M + 127) // 128
    out = np.zeros((nk, nm, 128, 128), dtype=ml_dtypes.bfloat16)
    for i in range(nk):
        for j in range(nm):
            blk = np.asarray(w, dtype=np.float32)[i * 128:(i + 1) * 128, j * 128:(j + 1) * 128]
            out[i, j, :blk.shape[0], :blk.shape[1]] = _bf(blk)
    return out


# ============================ device kernel ============================

EHALF = E_PAD // 2        # 25088
NSEG_H = EHALF // 512     # 49


def _build(Bg, debug=False, phases=4):
    nc = bacc.Bacc("TRN2", target_bir_lowering=False, debug=False, num_devices=NC)

    def din(name, shape, dt):
        return nc.dram_tensor(name, shape, dt, kind="ExternalInput").ap()

    t_msgT = din("msgT", [48, EHALF], BF16)
    t_xj = din("xj_idx", [128, NCH_ALL], mybir.dt.int32)
    t_pidx = din("pool_idx", [8, 128, Bg], mybir.dt.int32)
    t_oh = din("ohtab", [128, E_PAD], BF16)
    t_ohT = din("ohTtab", [128, E_PAD], BF16)
    t_invcnt = din("invcnt", [128, NWIN], F32)
    t_padcnt = din("padcnt", [128, 1], F32)
    t_ident = din("ident", [128, 128], BF16)
    t_eyef = din("eyef", [128, 128], F32)
    t_c1w = din("c1w", [3, 128, 128], BF16)
    t_c1b = din("c1b", [3, 128, 1], F32)
    t_c1gn = din("c1gn", [3, 3, 128, 1], F32)
    t_wa2 = din("wa2", [128, 256], BF16)
    t_c2wb = din("c2wb", [2, 128, 128], BF16)
    t_c2w2 = din("c2w2", [2, 2, 128, 128], BF16)
    t_c2b = din("c2b", [2, 2, 128, 1], F32)
    t_c2gn = din("c2gn", [2, 3, 2, 128, 1], F32)
    t_wa3 = din("wa3", [2, 128, 256], BF16)
    t_wb3 = din("wb3", [2, 128, 256], BF16)
    t_c3b = din("c3b", [2, 128, 1], F32)
    t_c3gn = din("c3gn", [3, 2, 128, 1], F32)
    t_lw1 = din("lw1", [2, 2, 128, 128], BF16)
    t_lb1 = din("lb1", [2, 128, 1], F32)
    t_lw2 = din("lw2", [2, 128, 2], BF16)
    t_lb2 = din("lb2", [2, 1], F32)
    t_poh = din("poolohtab", [128, 8 * Bg * 128], BF16)
    t_invg = din("invg", [128, 8], F32)

    o_out = nc.dram_tensor("out", [2, N_GRAPHS], F32, kind="ExternalOutput").ap()
    dbg = {}
    if debug:
        dbg["x1"] = nc.dram_tensor("dbg_x1", [NSLOTS, 128], BF16, kind="ExternalOutput").ap()
        dbg["x2"] = nc.dram_tensor("dbg_x2", [SPC, 256], BF16, kind="ExternalOutput").ap()
        dbg["x3"] = nc.dram_tensor("dbg_x3", [SPC, 256], BF16, kind="ExternalOutput").ap()
        dbg["pool"] = nc.dram_tensor("dbg_pool", [1024, 256], F32, kind="ExternalOutput").ap()

    with tile.TileContext(nc) as tc:
        with tc.tile_pool(name="dram", bufs=1, space="DRAM") as dram, \
             tc.tile_pool(name="cp", bufs=1) as cp:
            z_scr = [dram.tile([2, 128, E_PAD], BF16, tag=f"zscr{i}", name=f"zscr{i}") for i in range(2)]
            z3_scr = dram.tile([128, NCH_ALL * ZW], BF16)
            tab1_loc = dram.tile([SPC, 128], BF16)
            tab1 = dram.tile([NSLOTS, 128], BF16)
            a2_loc = dram.tile([SPC, 256], BF16)
            a3_loc = dram.tile([SPC, 256], BF16)
            b3_loc = dram.tile([SPC, 256], BF16)
            b3_full = dram.tile([NSLOTS, 256], BF16)
            tab3_loc = dram.tile([SPC, 256], BF16)
            st_in = dram.tile([128, 8], F32)
            st_out = dram.tile([128, 8], F32)
            pool_in = dram.tile([1024, 256], F32)
            pool_out = dram.tile([1024, 256], F32)

            ident = cp.tile([128, 128], BF16)
            nc.sync.dma_start(ident[:], t_ident[:])
            eyef = cp.tile([128, 128], F32)
            nc.sync.dma_start(eyef[:], t_eyef[:])
            invcnt = cp.tile([128, NWIN], F32)
            nc.sync.dma_start(invcnt[:], t_invcnt[:])
            padcnt = cp.tile([128, 1], F32)
            nc.sync.dma_start(padcnt[:], t_padcnt[:])

            # ---------- helpers ----------
            def allreduce_stats(s_acc, q_acc, n_mb, sb):
                st = sb.tile([128, 8], F32, tag="st_")
                nc.vector.memset(st[:], 0.0)
                nc.vector.tensor_copy(st[:, 0:n_mb], s_acc[:])
                nc.vector.tensor_copy(st[:, 4:4 + n_mb], q_acc[:])
                nc.sync.dma_start(st_in[:], st[:])
                nc.gpsimd.collective_compute(
                    "AllReduce", AOP.add, replica_groups=[list(range(NC))],
                    ins=[st_in.opt()], outs=[st_out.opt()])
                stg = sb.tile([128, 8], F32, tag="stg_")
                nc.sync.dma_start(stg[:], st_out[:])
                return stg

            def affine_from_stats(stg, n_mb, b_lin, gn, sb):
                A, Cc = [], []
                for mb in range(n_mb):
                    s = stg[:, mb:mb + 1]
                    q = stg[:, 4 + mb:5 + mb]
                    g, bgn, ms = gn[0][mb], gn[1][mb], gn[2][mb]
                    bl = b_lin[mb]
                    m = sb.tile([128, 1], F32, tag="af_m")
                    nc.vector.tensor_scalar(m[:], s, 1.0 / N_EDGES, None, AOP.mult)
                    nc.vector.tensor_tensor(m[:], m[:], bl, op=AOP.add)
                    e2 = sb.tile([128, 1], F32, tag="af_e2")
                    nc.vector.tensor_scalar(e2[:], q, 1.0 / N_EDGES, None, AOP.mult)
                    tmp = sb.tile([128, 1], F32, tag="af_t")
                    nc.vector.tensor_tensor(tmp[:], m[:], bl, op=AOP.mult)
                    nc.vector.tensor_scalar(tmp[:], tmp[:], 2.0, None, AOP.mult)
                    nc.vector.tensor_tensor(e2[:], e2[:], tmp[:], op=AOP.add)
                    nc.vector.tensor_tensor(tmp[:], bl, bl, op=AOP.mult)
                    nc.vector.tensor_tensor(e2[:], e2[:], tmp[:], op=AOP.subtract)
                    msm = sb.tile([128, 1], F32, tag="af_msm")
                    nc.vector.tensor_tensor(msm[:], ms, m[:], op=AOP.mult)
                    var = sb.tile([128, 1], F32, tag="af_v")
                    nc.vector.tensor_tensor(var[:], msm[:], msm[:], op=AOP.mult)
                    nc.vector.tensor_tensor(tmp[:], msm[:], m[:], op=AOP.mult)
                    nc.vector.tensor_scalar(tmp[:], tmp[:], 2.0, None, AOP.mult)
                    nc.vector.tensor_tensor(var[:], var[:], tmp[:], op=AOP.subtract)
                    nc.vector.tensor_tensor(var[:], var[:], e2[:], op=AOP.add)
                    a = sb.tile([128, 1], F32, tag="af_a")
                    nc.vector.tensor_scalar(var[:], var[:], EPS, None, AOP.add)
                    nc.scalar.activation(a[:], var[:], AFT.Sqrt)
                    nc.vector.reciprocal(a[:], a[:])
                    nc.vector.tensor_tensor(a[:], a[:], g, op=AOP.mult)
                    cc = sb.tile([128, 1], F32, tag="af_c")
                    nc.vector.tensor_tensor(cc[:], bl, msm[:], op=AOP.subtract)
                    nc.vector.tensor_tensor(cc[:], cc[:], a[:], op=AOP.mult)
                    nc.vector.tensor_tensor(cc[:], cc[:], bgn, op=AOP.add)
                    A.append(a)
                    Cc.append(cc)
                return A, Cc

            sqscr = cp.tile([128, BLK], BF16)

            def acc_stats(ps_ap, s_col, q_col, sb, n=512, sq_scalar=False):
                t1 = sb.tile([128, 1], F32, tag="rs_t1")
                nc.vector.reduce_sum(out=t1[:], in_=ps_ap, axis=AX.X)
                nc.vector.tensor_tensor(s_col, s_col, t1[:], op=AOP.add)
                qa = sb.tile([128, 1], F32, tag="rs_qa")
                if sq_scalar:
                    nc.scalar.activation(sqscr[:, :n], ps_ap, AFT.Square,
                                         accum_out=qa[:])
                else:
                    nc.vector.tensor_tensor(sqscr[:, :n], ps_ap, ps_ap, op=AOP.mult)
                    nc.vector.reduce_sum(out=qa[:], in_=sqscr[:, :n], axis=AX.X)
                nc.vector.tensor_tensor(q_col, q_col, qa[:], op=AOP.add)


            def sentinel_correct(s_acc, q_acc, zsent_cols, n_mb, sb):
                for mb in range(n_mb):
                    zs = zsent_cols[mb]
                    t1 = sb.tile([128, 1], F32, tag="sc_t1")
                    nc.vector.tensor_tensor(t1[:], zs, padcnt[:], op=AOP.mult)
                    nc.vector.tensor_tensor(s_acc[:, mb:mb + 1], s_acc[:, mb:mb + 1],
                                            t1[:], op=AOP.subtract)
                    nc.vector.tensor_tensor(t1[:], zs, zs, op=AOP.mult)
                    nc.vector.tensor_tensor(t1[:], t1[:], padcnt[:], op=AOP.mult)
                    nc.vector.tensor_tensor(q_acc[:, mb:mb + 1], q_acc[:, mb:mb + 1],
                                            t1[:], op=AOP.subtract)

            def load_vec(t_ap, sb, tag):
                v = sb.tile([128, 1], F32, tag=tag)
                nc.sync.dma_start(v[:], t_ap)
                return v[:]

            # ======================= CONV 1 =======================
            with tc.tile_pool(name="c1sb", bufs=2) as sb, \
                 tc.tile_pool(name="c1zb", bufs=1) as zbp:
                c1b = [[load_vec(t_c1b[i], sb, f"c1b{i}")] for i in range(3)]
                c1gn = [[[load_vec(t_c1gn[i, j], sb, f"c1gn{i}{j}")] for j in range(3)]
                        for i in range(3)]
                zbuf = zbp.tile([128, E_PAD], BF16)
                with tc.tile_pool(name="c1big", bufs=2) as bp, \
                     tc.tile_pool(name="c1ps", bufs=2, space="PSUM") as ps, \
                     tc.tile_pool(name="msgp", bufs=1) as msgp:
                    c1w = []
                    for i in range(3):
                        w = sb.tile([128, 128], BF16, tag=f"c1w{i}")
                        nc.sync.dma_start(w[:], t_c1w[i])
                        c1w.append(w)
                    msgT = msgp.tile([48, EHALF], BF16)
                    nc.sync.dma_start(msgT[:], t_msgT[:])

                    def z1_psum(h, s):
                        zp = ps.tile([128, 512], F32, tag="zp")
                        nc.tensor.matmul(zp[:], c1w[0][32 * h:32 * h + 10, :],
                                         msgT[32 * h:32 * h + 10, s * 512:(s + 1) * 512],
                                         start=True, stop=True)
                        return zp

                    s1 = sb.tile([128, 1], F32, tag="s1")
                    q1 = sb.tile([128, 1], F32, tag="q1")
                    nc.vector.memset(s1[:], 0.0)
                    nc.vector.memset(q1[:], 0.0)
                    for h in range(2):
                        for s in range(NSEG_H):
                            zp = z1_psum(h, s)
                            acc_stats(zp[:], s1[:, 0:1], q1[:, 0:1], sb,
                                      sq_scalar=True)
                    stg = allreduce_stats(s1, q1, 1, sb)
                    A1, C1 = affine_from_stats(stg, 1, c1b[0], c1gn[0], sb)

                    # L2: recompute z1, relu, z2 = W2 @ h1 -> zbuf (SBUF), batched stats
                    s2 = sb.tile([128, 1], F32, tag="s2")
                    q2 = sb.tile([128, 1], F32, tag="q2")
                    nc.vector.memset(s2[:], 0.0)
                    nc.vector.memset(q2[:], 0.0)
                    for h in range(2):
                        for b in range(NSEG_H // 7):
                            h1 = bp.tile([128, BLK], BF16, tag="h1")
                            for s in range(7):
                                zp = z1_psum(h, b * 7 + s)
                                nc.scalar.activation(h1[:, s * 512:(s + 1) * 512], zp[:],
                                                     AFT.Relu, bias=C1[0], scale=A1[0])
                            col0 = h * EHALF + b * BLK
                            for s in range(7):
                                zp = ps.tile([128, 512], F32, tag="zp")
                                nc.tensor.matmul(zp[:], c1w[1][:],
                                                 h1[:, s * 512:(s + 1) * 512],
                                                 start=True, stop=True)
                                nc.vector.tensor_copy(
                                    zbuf[:, col0 + s * 512:col0 + (s + 1) * 512], zp[:])
                            acc_stats(zbuf[:, col0:col0 + BLK], s2[:, 0:1], q2[:, 0:1],
                                      sb, n=BLK)
                    zs2 = sb.tile([128, 1], F32, tag="zs2")
                    nc.vector.tensor_copy(zs2[:], zbuf[:, E_PAD - 1:E_PAD])
                    sentinel_correct(s2, q2, [zs2[:]], 1, sb)
                    stg2 = allreduce_stats(s2, q2, 1, sb)
                    A2, C2 = affine_from_stats(stg2, 1, c1b[1], c1gn[1], sb)

                    # L3: h2 = relu(aff(z2)), z3 = W3 @ h2 -> zbuf in place
                    s3 = sb.tile([128, 1], F32, tag="s3")
                    q3 = sb.tile([128, 1], F32, tag="q3")
                    nc.vector.memset(s3[:], 0.0)
                    nc.vector.memset(q3[:], 0.0)
                    for b in range(NBLK):
                        h2 = bp.tile([128, BLK], BF16, tag="h2")
                        nc.scalar.activation(h2[:], zbuf[:, b * BLK:(b + 1) * BLK],
                                             AFT.Relu, bias=C2[0], scale=A2[0])
                        for s in range(7):
                            zp = ps.tile([128, 512], F32, tag="zp")
                            nc.tensor.matmul(zp[:], c1w[2][:],
                                             h2[:, s * 512:(s + 1) * 512],
                                             start=True, stop=True)
                            nc.vector.tensor_copy(
                                zbuf[:, b * BLK + s * 512:b * BLK + (s + 1) * 512], zp[:])
                        acc_stats(zbuf[:, b * BLK:(b + 1) * BLK], s3[:, 0:1], q3[:, 0:1],
                                  sb, n=BLK)
                    zs3 = sb.tile([128, 1], F32, tag="zs3")
                    nc.vector.tensor_copy(zs3[:], zbuf[:, E_PAD - 1:E_PAD])
                    sentinel_correct(s3, q3, [zs3[:]], 1, sb)
                    stg3 = allreduce_stats(s3, q3, 1, sb)
                    A3, C3 = affine_from_stats(stg3, 1, c1b[2], c1gn[2], sb)

                # scatter: h3 = relu(aff(z3)); transpose; one-hot matmul; + A2 table
                with tc.tile_pool(name="s1sb", bufs=2) as sp, \
                     tc.tile_pool(name="s1oh", bufs=2) as ohp, \
                     tc.tile_pool(name="s1tp", bufs=2, space="PSUM") as ps_tp, \
                     tc.tile_pool(name="s1sc", bufs=2, space="PSUM") as ps_sc:
                    wa2 = sp.tile([128, 256], BF16, tag="wa2")
                    nc.sync.dma_start(wa2[:], t_wa2[:])
                    for b in range(NBLK):
                        h3 = sp.tile([128, BLK], BF16, tag="h3")
                        nc.scalar.activation(h3[:], zbuf[:, b * BLK:(b + 1) * BLK],
                                             AFT.Relu, bias=C3[0], scale=A3[0])
                        ohb = ohp.tile([128, BLK], BF16, tag="ohb")
                        nc.sync.dma_start(ohb[:], t_oh[:, b * BLK:(b + 1) * BLK])
                        for w in range(NW_BLK):
                            gw = b * NW_BLK + w
                            tpp = ps_tp.tile([128, 512], BF16, tag="tpp", space="PSUM")
                            for cb in range(B):
                                nc.tensor.transpose(
                                    tpp[:, cb * 128:(cb + 1) * 128],
                                    h3[:, (w * B + cb) * 128:(w * B + cb + 1) * 128],
                                    ident[:])
                            hE = sp.tile([128, 512], BF16, tag="hE")
                            nc.vector.tensor_copy(hE[:], tpp[:])
                            sc = ps_sc.tile([128, 128], F32, tag="sc", space="PSUM")
                            for cb in range(B):
                                nc.tensor.matmul(
                                    sc[:], ohb[:, (w * B + cb) * 128:(w * B + cb + 1) * 128],
                                    hE[:, cb * 128:(cb + 1) * 128],
                                    start=(cb == 0), stop=(cb == B - 1))
                            nt = sp.tile([128, 128], BF16, tag="nt")
                            nc.vector.tensor_scalar(nt[:], sc[:], invcnt[:, gw:gw + 1],
                                                    None, AOP.mult)
                            nc.sync.dma_start(tab1_loc[gw * WIN:(gw + 1) * WIN, :], nt[:])
                            # A2 table: ntT then (x1_win) @ WA2
                            ntp = ps_tp.tile([128, 128], BF16, tag="ntp", space="PSUM")
                            nc.tensor.transpose(ntp[:], nt[:], ident[:])
                            ntT = sp.tile([128, 128], BF16, tag="ntT")
                            nc.vector.tensor_copy(ntT[:], ntp[:])
                            a2p = ps_sc.tile([128, 256], F32, tag="a2p", space="PSUM")
                            nc.tensor.matmul(a2p[:], ntT[:], wa2[:], start=True, stop=True)
                            a2t = sp.tile([128, 256], BF16, tag="a2t")
                            nc.vector.tensor_copy(a2t[:], a2p[:])
                            nc.sync.dma_start(a2_loc[gw * WIN:(gw + 1) * WIN, :], a2t[:])

            nc.gpsimd.collective_compute(
                "AllGather", AOP.bypass, replica_groups=[list(range(NC))],
                ins=[tab1_loc.opt()], outs=[tab1.opt()])
            if debug:
                nc.sync.dma_start(dbg["x1"][:], tab1[:])

            # ======================= CONV 2 =======================
            if phases >= 2:
              with tc.tile_pool(name="c2sb", bufs=2) as sb:
                  c2b = [[load_vec(t_c2b[i, mb], sb, f"c2b{i}{mb}") for mb in range(2)]
                         for i in range(2)]
                  c2gn = [[[load_vec(t_c2gn[i, j, mb], sb, f"c2gn{i}{j}{mb}")
                            for mb in range(2)] for j in range(3)] for i in range(2)]
                  # ---- pass 1: z1 = A2[dst] (one-hot expand) + WB2 @ x1[src] ----
                  sA = sb.tile([128, 2], F32, tag="c2sA")
                  qA = sb.tile([128, 2], F32, tag="c2qA")
                  nc.vector.memset(sA[:], 0.0)
                  nc.vector.memset(qA[:], 0.0)
                  with tc.tile_pool(name="g2g", bufs=2) as g2, \
                       tc.tile_pool(name="g2q", bufs=1) as gqp, \
                       tc.tile_pool(name="g2z", bufs=2) as zwp, \
                       tc.tile_pool(name="g2ps", bufs=2, space="PSUM") as ps, \
                       tc.tile_pool(name="g2tp", bufs=2, space="PSUM") as ps_tp:
                      wbs = []
                      for mo in range(2):
                          wtb = sb.tile([128, 128], BF16, tag=f"c2wb{mo}")
                          nc.sync.dma_start(wtb[:], t_c2wb[mo])
                          wbs.append(wtb)
                      for b in range(NBLK):
                          ixj = g2.tile([128, NCHUNK], mybir.dt.int32, tag="ixj")
                          nc.sync.dma_start(ixj[:], t_xj[:, b * NCHUNK:(b + 1) * NCHUNK])
                          gxj = gqp.tile([128, NCHUNK * 128], BF16, tag="gxj")
                          for ch in range(NCHUNK):
                              nc.gpsimd.indirect_dma_start(
                                  out=gxj[:, ch * 128:(ch + 1) * 128],
                                  out_offset=None,
                                  in_=tab1[:],
                                  in_offset=bass.IndirectOffsetOnAxis(
                                      ap=ixj[:, ch:ch + 1], axis=0))
                          ohTb = g2.tile([128, BLK], BF16, tag="ohTb")
                          nc.sync.dma_start(ohTb[:], t_ohT[:, b * BLK:(b + 1) * BLK])
                          zsb = [zwp.tile([128, BLK], BF16, tag=f"zsb{h}", name=f"zsb{h}")
                                 for h in range(2)]
                          for w in range(NW_BLK):
                              gw = b * NW_BLK + w
                              a2w = g2.tile([128, 256], BF16, tag="a2w")
                              nc.sync.dma_start(a2w[:], a2_loc[gw * WIN:(gw + 1) * WIN, :])
                              tpp = ps_tp.tile([128, 512], BF16, tag="xtp", space="PSUM")
                              for cb in range(B):
                                  nc.tensor.transpose(
                                      tpp[:, cb * 128:(cb + 1) * 128],
                                      gxj[:, (w * B + cb) * 128:(w * B + cb + 1) * 128],
                                      ident[:])
                              xjT = g2.tile([128, 512], BF16, tag="xjT")
                              nc.vector.tensor_copy(xjT[:], tpp[:])
                              for h in range(2):
                                  zp = ps.tile([128, 512], F32, tag="zp")
                                  nc.tensor.matmul(zp[:], a2w[:, h * 128:(h + 1) * 128],
                                                   ohTb[:, w * 512:(w + 1) * 512],
                                                   start=True, stop=False)
                                  nc.tensor.matmul(zp[:], wbs[h][:], xjT[:],
                                                   start=False, stop=True)
                                  nc.scalar.copy(zsb[h][:, w * 512:(w + 1) * 512], zp[:])
                          for h in range(2):
                              acc_stats(zsb[h][:], sA[:, h:h + 1], qA[:, h:h + 1],
                                        sb, n=BLK)
                              nc.sync.dma_start(z_scr[0][h, :, b * BLK:(b + 1) * BLK],
                                                zsb[h][:])
                  stg = allreduce_stats(sA, qA, 2, sb)
                  A1, C1 = affine_from_stats(stg, 2, c2b[0], c2gn[0], sb)

                  # ---- layer 2 ----
                  s2 = sb.tile([128, 2], F32, tag="c2s2")
                  q2 = sb.tile([128, 2], F32, tag="c2q2")
                  nc.vector.memset(s2[:], 0.0)
                  nc.vector.memset(q2[:], 0.0)
                  zlast = [None, None]
                  with tc.tile_pool(name="c2mid", bufs=2) as mp, \
                       tc.tile_pool(name="c2ps", bufs=2, space="PSUM") as ps:
                      w2s = []
                      for ki in range(2):
                          for mo in range(2):
                              w = sb.tile([128, 128], BF16, tag=f"c2w2{ki}{mo}")
                              nc.sync.dma_start(w[:], t_c2w2[ki, mo])
                              w2s.append(w)
                      for b in range(NBLK):
                          h1 = []
                          for mb in range(2):
                              z = mp.tile([128, BLK], BF16, tag=f"c2z1r{mb}")
                              nc.sync.dma_start(z[:], z_scr[0][mb, :, b * BLK:(b + 1) * BLK])
                              hh = mp.tile([128, BLK], BF16, tag=f"c2h1{mb}")
                              nc.vector.tensor_scalar(hh[:], z[:], A1[mb], C1[mb],
                                                      AOP.mult, AOP.add)
                              nc.vector.tensor_scalar(hh[:], hh[:], 0.0, None, AOP.max)
                              h1.append(hh)
                          for mo in range(2):
                              zw = mp.tile([128, BLK], BF16, tag=f"c2z2w{mo}")
                              for s in range(NSEG):
                                  zp = ps.tile([128, 512], F32, tag="c2zp")
                                  for ki in range(2):
                                      nc.tensor.matmul(zp[:], w2s[ki * 2 + mo][:],
                                                       h1[ki][:, s * 512:(s + 1) * 512],
                                                       start=(ki == 0), stop=(ki == 1))
                                  nc.scalar.copy(zw[:, s * 512:(s + 1) * 512], zp[:])
                              acc_stats(zw[:], s2[:, mo:mo + 1], q2[:, mo:mo + 1],
                                        sb, n=BLK)
                              nc.sync.dma_start(z_scr[1][mo, :, b * BLK:(b + 1) * BLK], zw[:])
                              zlast[mo] = zw
                      zsent = []
                      for mo in range(2):
                          zc = sb.tile([128, 1], F32, tag=f"c2zs{mo}")
                          nc.vector.tensor_copy(zc[:], zlast[mo][:, BLK - 1:BLK])
                          zsent.append(zc[:])
                  sentinel_correct(s2, q2, zsent, 2, sb)
                  stg2 = allreduce_stats(s2, q2, 2, sb)
                  A2, C2 = affine_from_stats(stg2, 2, c2b[1], c2gn[1], sb)

                  # ---- scatter + A3/B3 tables ----
                  with tc.tile_pool(name="s2sb", bufs=2) as sp, \
                       tc.tile_pool(name="s2oh", bufs=2) as ohp, \
                       tc.tile_pool(name="s2tp", bufs=2, space="PSUM") as ps_tp, \
                       tc.tile_pool(name="s2sc", bufs=2, space="PSUM") as ps_sc:
                      wab3 = sp.tile([128, 1024], BF16, tag="wab3")
                      for ki in range(2):
                          nc.sync.dma_start(wab3[:, ki * 512:ki * 512 + 256], t_wa3[ki])
                          nc.sync.dma_start(wab3[:, ki * 512 + 256:ki * 512 + 512],
                                            t_wb3[ki])
                      for b in range(NBLK):
                          hs = []
                          for mb in range(2):
                              z = sp.tile([128, BLK], BF16, tag=f"s2z{mb}")
                              nc.sync.dma_start(z[:], z_scr[1][mb, :, b * BLK:(b + 1) * BLK])
                              h = sp.tile([128, BLK], BF16, tag=f"s2h{mb}")
                              nc.scalar.activation(h[:], z[:], AFT.Relu,
                                                   bias=C2[mb], scale=A2[mb])
                              hs.append(h)
                          ohb = ohp.tile([128, BLK], BF16, tag="ohb2")
                          nc.sync.dma_start(ohb[:], t_oh[:, b * BLK:(b + 1) * BLK])
                          for w in range(NW_BLK):
                              gw = b * NW_BLK + w
                              tpp = ps_tp.tile([128, 1024], BF16, tag="tpp2", space="PSUM")
                              for cb in range(B):
                                  for mb in range(2):
                                      nc.tensor.transpose(
                                          tpp[:, (cb * 2 + mb) * 128:(cb * 2 + mb + 1) * 128],
                                          hs[mb][:, (w * B + cb) * 128:(w * B + cb + 1) * 128],
                                          ident[:])
                              hE = sp.tile([128, 1024], BF16, tag="hE2")
                              nc.vector.tensor_copy(hE[:], tpp[:])
                              sc = ps_sc.tile([128, 256], F32, tag="sc2", space="PSUM")
                              for cb in range(B):
                                  nc.tensor.matmul(
                                      sc[:], ohb[:, (w * B + cb) * 128:(w * B + cb + 1) * 128],
                                      hE[:, cb * 256:(cb + 1) * 256],
                                      start=(cb == 0), stop=(cb == B - 1))
                              nt = sp.tile([128, 256], BF16, tag="nt2")
                              nc.vector.tensor_scalar(nt[:], sc[:], invcnt[:, gw:gw + 1],
                                                      None, AOP.mult)
                              if debug:
                                  nc.sync.dma_start(dbg["x2"][gw * WIN:(gw + 1) * WIN, :], nt[:])
                              ntp = ps_tp.tile([128, 256], BF16, tag="ntp2", space="PSUM")
                              for ki in range(2):
                                  nc.tensor.transpose(ntp[:, ki * 128:(ki + 1) * 128],
                                                      nt[:, ki * 128:(ki + 1) * 128],
                                                      ident[:])
                              ntT = sp.tile([128, 256], BF16, tag="ntT2")
                              nc.vector.tensor_copy(ntT[:], ntp[:])
                              abp = ps_sc.tile([128, 512], F32, tag="abp", space="PSUM")
                              for ki in range(2):
                                  nc.tensor.matmul(abp[:], ntT[:, ki * 128:(ki + 1) * 128],
                                                   wab3[:, ki * 512:(ki + 1) * 512],
                                                   start=(ki == 0), stop=(ki == 1))
                              abt = sp.tile([128, 512], BF16, tag="abt")
                              nc.vector.tensor_copy(abt[:], abp[:])
                              nc.sync.dma_start(a3_loc[gw * WIN:(gw + 1) * WIN, :],
                                                abt[:, 0:256])
                              nc.sync.dma_start(b3_loc[gw * WIN:(gw + 1) * WIN, :],
                                                abt[:, 256:512])

            nc.gpsimd.collective_compute(
                "AllGather", AOP.bypass, replica_groups=[list(range(NC))],
                ins=[b3_loc.opt()], outs=[b3_full.opt()])

            # ======================= CONV 3 =======================
            if phases >= 3:
              with tc.tile_pool(name="c3sb", bufs=2) as sb:
                  c3b = [load_vec(t_c3b[mb], sb, f"c3b{mb}") for mb in range(2)]
                  c3gn = [[load_vec(t_c3gn[j, mb], sb, f"c3gn{j}{mb}") for mb in range(2)]
                          for j in range(3)]
                  G1 = sb.tile([128, ZW], F32, tag="G1")
                  G2 = sb.tile([128, ZW], F32, tag="G2")
                  nc.vector.memset(G1[:], 0.0)
                  nc.vector.memset(G2[:], 0.0)
                  # ---- pass 1: z = A3[dst] + B3[src]; Gram stats; spill z ----
                  with tc.tile_pool(name="c3g", bufs=2) as g3, \
                       tc.tile_pool(name="c3q", bufs=1) as gqp3, \
                       tc.tile_pool(name="c3zt", bufs=2) as ztp, \
                       tc.tile_pool(name="c3ps", bufs=4, space="PSUM") as ps, \
                       tc.tile_pool(name="c3gp", bufs=2, space="PSUM") as psg:
                      for b in range(NBLK):
                          ixj = g3.tile([128, NCHUNK], mybir.dt.int32, tag="ixj3")
                          nc.sync.dma_start(ixj[:], t_xj[:, b * NCHUNK:(b + 1) * NCHUNK])
                          gb = gqp3.tile([128, NCHUNK * 256], BF16, tag="gb3")
                          for ch in range(NCHUNK):
                              nc.gpsimd.indirect_dma_start(
                                  out=gb[:, ch * 256:(ch + 1) * 256],
                                  out_offset=None,
                                  in_=b3_full[:],
                                  in_offset=bass.IndirectOffsetOnAxis(
                                      ap=ixj[:, ch:ch + 1], axis=0))
                          zt = ztp.tile([128, NCHUNK * ZW], BF16, tag="zt")
                          ones_ap = zt[:].rearrange("p (c k) -> p c k", k=ZW)[:, :, 256:257]
                          nc.vector.memset(ones_ap, 1.0)
                          ohTb = g3.tile([128, BLK], BF16, tag="ohTb3")
                          nc.sync.dma_start(ohTb[:], t_ohT[:, b * BLK:(b + 1) * BLK])
                          for w in range(NW_BLK):
                              gw = b * NW_BLK + w
                              a3w = g3.tile([128, 256], BF16, tag="a3w")
                              nc.sync.dma_start(a3w[:], a3_loc[gw * WIN:(gw + 1) * WIN, :])
                              for cb in range(B):
                                  ch = w * B + cb
                                  ap_ = ps.tile([128, 256], F32, tag="aexp")
                                  nc.tensor.matmul(ap_[:],
                                                   ohTb[:, ch * 128:(ch + 1) * 128],
                                                   a3w[:], start=True, stop=True)
                                  nc.vector.tensor_tensor(
                                      zt[:, ch * ZW:ch * ZW + 256],
                                      gb[:, ch * 256:(ch + 1) * 256],
                                      ap_[:], op=AOP.add)
                          G1p = psg.tile([128, ZW], F32, tag="G1p", space="PSUM")
                          G2p = psg.tile([128, ZW], F32, tag="G2p", space="PSUM")
                          for ch in range(NCHUNK):
                              nc.tensor.matmul(G1p[:], zt[:, ch * ZW:ch * ZW + 128],
                                               zt[:, ch * ZW:ch * ZW + ZW],
                                               start=(ch == 0), stop=(ch == NCHUNK - 1),
                                               skip_group_check=True)
                              nc.tensor.matmul(G2p[:], zt[:, ch * ZW + 128:ch * ZW + 256],
                                               zt[:, ch * ZW:ch * ZW + ZW],
                                               start=(ch == 0), stop=(ch == NCHUNK - 1),
                                               skip_group_check=True)
                          nc.vector.tensor_tensor(G1[:], G1[:], G1p[:], op=AOP.add)
                          nc.vector.tensor_tensor(G2[:], G2[:], G2p[:], op=AOP.add)
                          nc.sync.dma_start(
                              z3_scr[:, b * NCHUNK * ZW:(b + 1) * NCHUNK * ZW], zt[:])
                  # stats: sums = G[:,256]; sumsq = diag
                  sA = sb.tile([128, 2], F32, tag="c3sA")
                  qA = sb.tile([128, 2], F32, tag="c3qA")
                  nc.vector.tensor_copy(sA[:, 0:1], G1[:, 256:257])
                  nc.vector.tensor_copy(sA[:, 1:2], G2[:, 256:257])
                  dtmp = sb.tile([128, 128], F32, tag="dtmp")
                  nc.vector.tensor_tensor(dtmp[:], G1[:, 0:128], eyef[:], op=AOP.mult)
                  nc.vector.reduce_sum(out=qA[:, 0:1], in_=dtmp[:], axis=AX.X)
                  nc.vector.tensor_tensor(dtmp[:], G2[:, 128:256], eyef[:], op=AOP.mult)
                  nc.vector.reduce_sum(out=qA[:, 1:2], in_=dtmp[:], axis=AX.X)
                  stg = allreduce_stats(sA, qA, 2, sb)
                  A1, C1 = affine_from_stats(stg, 2, c3b, c3gn, sb)
                  # broadcast affine rows: a_bc/c_bc [128, ZW] bf16
                  a_bc = sb.tile([128, ZW], BF16, tag="a_bc")
                  c_bc = sb.tile([128, ZW], BF16, tag="c_bc")
                  with tc.tile_pool(name="c3bp", bufs=2, space="PSUM") as psb:
                      for dst_t, vals in ((a_bc, A1), (c_bc, C1)):
                          nc.vector.memset(dst_t[:], 0.0)
                          for mb in range(2):
                              tp = psb.tile([128, 128], F32, tag="bcp", space="PSUM")
                              nc.tensor.transpose(
                                  tp[:], vals[mb][:, 0:1].to_broadcast([128, 128]),
                                  eyef[:])
                              nc.vector.tensor_copy(dst_t[:, mb * 128:(mb + 1) * 128], tp[:])
                  # ---- pass 2: h = relu(a*z + c); scatter ----
                  with tc.tile_pool(name="c3p2", bufs=2) as p2, \
                       tc.tile_pool(name="c3oh", bufs=2) as ohp, \
                       tc.tile_pool(name="c3sc", bufs=2, space="PSUM") as ps_sc:
                      for b in range(NBLK):
                          zt = p2.tile([128, NCHUNK * ZW], BF16, tag="zt2")
                          nc.sync.dma_start(
                              zt[:], z3_scr[:, b * NCHUNK * ZW:(b + 1) * NCHUNK * ZW])
                          h = p2.tile([128, NCHUNK * ZW], BF16, tag="h3b")
                          nc.vector.tensor_tensor(
                              out=h[:].rearrange("p (c k) -> p c k", k=ZW),
                              in0=zt[:].rearrange("p (c k) -> p c k", k=ZW),
                              in1=a_bc[:, None, :].to_broadcast((128, NCHUNK, ZW)),
                              op=AOP.mult)
                          nc.vector.tensor_tensor(
                              out=h[:].rearrange("p (c k) -> p c k", k=ZW),
                              in0=h[:].rearrange("p (c k) -> p c k", k=ZW),
                              in1=c_bc[:, None, :].to_broadcast((128, NCHUNK, ZW)),
                              op=AOP.add)
                          nc.scalar.activation(h[:], h[:], AFT.Relu)
                          ohb = ohp.tile([128, BLK], BF16, tag="ohb3")
                          nc.sync.dma_start(ohb[:], t_oh[:, b * BLK:(b + 1) * BLK])
                          for w in range(NW_BLK):
                              gw = b * NW_BLK + w
                              sc = ps_sc.tile([128, 256], F32, tag="sc3", space="PSUM")
                              for cb in range(B):
                                  ch = w * B + cb
                                  nc.tensor.matmul(
                                      sc[:], ohb[:, ch * 128:(ch + 1) * 128],
                                      h[:, ch * ZW:ch * ZW + 256],
                                      start=(cb == 0), stop=(cb == B - 1))
                              nt = p2.tile([128, 256], BF16, tag="nt3")
                              nc.vector.tensor_scalar(nt[:], sc[:], invcnt[:, gw:gw + 1],
                                                      None, AOP.mult)
                              nc.sync.dma_start(tab3_loc[gw * WIN:(gw + 1) * WIN, :], nt[:])

            if debug:
                nc.sync.dma_start(dbg["x3"][:], tab3_loc[:])

            # ======================= POOL + HEAD =======================
            if phases >= 4:
              with tc.tile_pool(name="p_sb", bufs=2) as sb, \
                 tc.tile_pool(name="p_ps", bufs=2, space="PSUM") as ps:
                  for gw in range(8):
                      pidx = sb.tile([128, Bg], mybir.dt.int32, tag="p_idx")
                      nc.sync.dma_start(pidx[:], t_pidx[gw])
                      poh = sb.tile([128, Bg * 128], BF16, tag="p_poh")
                      nc.sync.dma_start(poh[:],
                                        t_poh[:, gw * Bg * 128:(gw + 1) * Bg * 128])
                      gp = sb.tile([128, Bg * 256], BF16, tag="p_gp")
                      for c in range(Bg):
                          nc.gpsimd.indirect_dma_start(
                              out=gp[:, c * 256:(c + 1) * 256], out_offset=None,
                              in_=tab3_loc[:],
                              in_offset=bass.IndirectOffsetOnAxis(
                                  ap=pidx[:, c:c + 1], axis=0))
                      pp = ps.tile([128, 256], F32, tag="p_pp", space="PSUM")
                      for c in range(Bg):
                          nc.tensor.matmul(pp[:], poh[:, c * 128:(c + 1) * 128],
                                           gp[:, c * 256:(c + 1) * 256],
                                           start=(c == 0), stop=(c == Bg - 1))
                      pf = sb.tile([128, 256], F32, tag="p_pf")
                      nc.vector.tensor_copy(pf[:], pp[:])
                      nc.sync.dma_start(pool_in[gw * 128:(gw + 1) * 128, :], pf[:])
                  nc.gpsimd.collective_compute(
                      "AllReduce", AOP.add, replica_groups=[list(range(NC))],
                      ins=[pool_in.opt()], outs=[pool_out.opt()])
                  if debug:
                      nc.sync.dma_start(dbg["pool"][:], pool_out[:])

                  invg = sb.tile([128, 8], F32, tag="p_invg")
                  nc.sync.dma_start(invg[:], t_invg[:])
                  lw1 = []
                  for ki in range(2):
                      for mo in range(2):
                          w = sb.tile([128, 128], BF16, tag=f"p_lw1{ki}{mo}")
                          nc.sync.dma_start(w[:], t_lw1[ki, mo])
                          lw1.append(w)
                  lw2 = []
                  for ki in range(2):
                      w = sb.tile([128, 2], BF16, tag=f"p_lw2{ki}")
                      nc.sync.dma_start(w[:], t_lw2[ki])
                      lw2.append(w)
                  lb1 = [load_vec(t_lb1[mb], sb, f"p_lb1{mb}") for mb in range(2)]
                  lb2 = sb.tile([2, 1], F32, tag="p_lb2")
                  nc.sync.dma_start(lb2[:], t_lb2[:])
                  ofin = sb.tile([2, 1024], F32, tag="p_out")
                  for gw in range(8):
                      g = sb.tile([128, 256], F32, tag="p_g")
                      nc.sync.dma_start(g[:], pool_out[gw * 128:(gw + 1) * 128, :])
                      gm = sb.tile([128, 256], BF16, tag="p_gm")
                      nc.vector.tensor_scalar(gm[:], g[:], invg[:, gw:gw + 1], None, AOP.mult)
                      gT = sb.tile([128, 2 * 128], BF16, tag="p_gT")
                      for kb in range(2):
                          tp = ps.tile([128, 128], BF16, tag="p_tp", space="PSUM")
                          nc.tensor.transpose(tp[:], gm[:, kb * 128:(kb + 1) * 128], ident[:])
                          nc.vector.tensor_copy(gT[:, kb * 128:(kb + 1) * 128], tp[:])
                      hT = sb.tile([128, 2 * 128], BF16, tag="p_hT")
                      for mo in range(2):
                          hp = ps.tile([128, 128], F32, tag="p_hp", space="PSUM")
                          for ki in range(2):
                              nc.tensor.matmul(hp[:], lw1[ki * 2 + mo][:],
                                               gT[:, ki * 128:(ki + 1) * 128],
                                               start=(ki == 0), stop=(ki == 1))
                          nc.scalar.activation(hT[:, mo * 128:(mo + 1) * 128], hp[:],
                                               AFT.Relu, bias=lb1[mo])
                      op_ = ps.tile([2, 128], F32, tag="p_op", space="PSUM")
                      for ki in range(2):
                          nc.tensor.matmul(op_[:], lw2[ki][:],
                                           hT[:, ki * 128:(ki + 1) * 128],
                                           start=(ki == 0), stop=(ki == 1))
                      nc.vector.tensor_scalar(ofin[:, gw * 128:(gw + 1) * 128],
                                              op_[:], lb2[:], None, AOP.add)
                  nc.sync.dma_start(o_out[:], ofin[:, :N_GRAPHS])

    nc.compile()
    return nc


# ============================ entry point ============================


def kernel(**inputs):
    x = np.asarray(inputs["x"], dtype=np.float32)
    edge_index = np.asarray(inputs["edge_index"])
    batch = np.asarray(inputs["batch"])

    meta = _pack(edge_index, batch)
    Bg = meta["Bg"]

    import os as _os
    phases = int(_os.environ.get("KPHASES", "4"))
    key = ("mod", Bg, phases, _DEBUG[0])
    if key not in _cache:
        _cache[key] = _build(Bg, debug=bool(inputs.get("_debug", False)) or _DEBUG[0],
                             phases=phases)
    nc = _cache[key]

    # ---- per-core input arrays ----
    src = np.asarray(edge_index[0], dtype=np.int64)
    dst = np.asarray(edge_index[1], dtype=np.int64)

    # conv1 msgT: [core, 48, E_PAD//2] bf16; edge e<EHALF -> rows 0..9 col e,
    # e>=EHALF -> rows 32..41 col e-EHALF
    xi_v = x[dst]
    xj_v = x[src]
    msg = np.concatenate([xi_v, xj_v - xi_v], axis=1)       # [E, 10]
    msg_full = np.zeros((NC, E_PAD, 10), dtype=np.float32)
    ec, pos = meta["ec"], meta["pos"]
    msg_full[ec, pos] = msg[meta["eorder"]]
    msgT = np.zeros((NC, 48, EHALF), dtype=ml_dtypes.bfloat16)
    msgT[:, :10, :] = _bf(msg_full[:, :EHALF].transpose(0, 2, 1))
    msgT[:, 32:42, :] = _bf(msg_full[:, EHALF:].transpose(0, 2, 1))

    # one-hot tables from dstwin
    dstwin = meta["dstwin"]  # [NC, E_PAD], float; -1 for padding
    dw = dstwin.reshape(NC, NCH_ALL, 128).astype(np.int32)
    nn_ = np.arange(128, dtype=np.int32)
    oh_in = np.empty((NC, 128, E_PAD), dtype=ml_dtypes.bfloat16)
    ohT_in = np.empty((NC, 128, E_PAD), dtype=ml_dtypes.bfloat16)
    for c in range(NC):
        m = (dw[c][:, :, None] == nn_[None, None, :])     # [392, 128e, 128n]
        oh_in[c] = m.transpose(1, 0, 2).reshape(128, E_PAD).astype(ml_dtypes.bfloat16)
        ohT_in[c] = m.transpose(2, 0, 1).reshape(128, E_PAD).astype(ml_dtypes.bfloat16)

    invcnt_in = np.ascontiguousarray(
        meta["inv_cnt"].reshape(NC, NWIN, 128).transpose(0, 2, 1)).astype(np.float32)
    padcnt_in = np.repeat(meta["pad_cnt"][:, None], 128, axis=1)[:, :, None].astype(np.float32)

    ident_in = np.eye(128, dtype=np.float32).astype(ml_dtypes.bfloat16)
    eyef_in = np.eye(128, dtype=np.float32)

    xj_in = np.ascontiguousarray(
        meta["xj_glob"].reshape(NC, NCH_ALL, 128).transpose(0, 2, 1)).astype(np.int32)

    # weights
    c1w = np.zeros((3, 128, 128), dtype=ml_dtypes.bfloat16)
    c1w[0, :10, :] = _bf(inputs["c1_w1"])
    c1w[0, 32:42, :] = _bf(inputs["c1_w1"])
    c1w[1] = _bf(inputs["c1_w2"])
    c1w[2] = _bf(inputs["c1_w3"])
    c1b = np.stack([np.asarray(inputs[f"c1_b{i}"], dtype=np.float32).reshape(128, 1)
                    for i in (1, 2, 3)])
    c1gn = np.stack([np.asarray(inputs[f"c1_gn{i}"], dtype=np.float32).reshape(3, 128, 1)
                     for i in (1, 2, 3)])

    w2a = np.asarray(inputs["c2_w1"], dtype=np.float32)   # [256, 256]
    WA2 = w2a[:128] - w2a[128:]
    WB2 = w2a[128:]
    wa2 = _bf(WA2)                                        # [128, 256]
    c2wb = _tile_w(WB2)[0]                                # [2(mo), 128, 128]
    c2w2 = _tile_w(np.asarray(inputs["c2_w2"], dtype=np.float32))  # [2,2,128,128]
    c2b = np.stack([np.asarray(inputs["c2_b1"], dtype=np.float32).reshape(2, 128, 1),
                    np.asarray(inputs["c2_b2"], dtype=np.float32).reshape(2, 128, 1)])
    c2gn = np.stack([np.asarray(inputs["c2_gn1"], dtype=np.float32).reshape(3, 2, 128, 1),
                     np.asarray(inputs["c2_gn2"], dtype=np.float32).reshape(3, 2, 128, 1)])

    w3a = np.asarray(inputs["c3_w1"], dtype=np.float32)   # [512, 256]
    WA3 = w3a[:256] - w3a[256:]
    WB3 = w3a[256:]
    wa3 = _bf(WA3).reshape(2, 128, 256)
    wb3 = _bf(WB3).reshape(2, 128, 256)
    c3b = np.asarray(inputs["c3_b1"], dtype=np.float32).reshape(2, 128, 1)
    c3gn = np.asarray(inputs["c3_gn1"], dtype=np.float32).reshape(3, 2, 128, 1)

    lw1 = _tile_w(np.asarray(inputs["lin_w1"], dtype=np.float32))
    lb1 = np.asarray(inputs["lin_b1"], dtype=np.float32).reshape(2, 128, 1)
    lw2_f = np.asarray(inputs["lin_w2"], dtype=np.float32)  # [256, 2]
    lw2 = np.stack([_bf(lw2_f[:128]), _bf(lw2_f[128:])])    # [2, 128, 2]
    lb2 = np.asarray(inputs["lin_b2"], dtype=np.float32).reshape(2, 1)

    pidx_in = np.ascontiguousarray(
        meta["pool_idx"].astype(np.int32).reshape(NC, 8, Bg, 128).transpose(0, 1, 3, 2))
    pgwl = meta["pool_gwl"].reshape(NC, 8, Bg, 128)        # [c, gw, cs, p]
    gg = np.arange(128, dtype=np.float32)
    poh_in = np.empty((NC, 128, 8 * Bg * 128), dtype=ml_dtypes.bfloat16)
    for c in range(NC):
        m = (pgwl[c][:, :, :, None] == gg[None, None, None, :])  # [8, Bg, 128p, 128g]
        poh_in[c] = m.transpose(2, 0, 1, 3).reshape(128, 8 * Bg * 128).astype(
            ml_dtypes.bfloat16)
    invg_in = np.broadcast_to(
        meta["inv_g"].reshape(8, 128).T[None], (NC, 128, 8)).astype(np.float32)
    invg_in = np.ascontiguousarray(invg_in)

    in_maps = []
    for c in range(NC):
        im = {
            "msgT": msgT[c],
            "xj_idx": xj_in[c],
            "pool_idx": pidx_in[c],
            "ohtab": oh_in[c],
            "ohTtab": ohT_in[c],
            "invcnt": invcnt_in[c],
            "padcnt": padcnt_in[c],
            "ident": ident_in,
            "eyef": eyef_in,
            "c1w": c1w, "c1b": c1b, "c1gn": c1gn,
            "wa2": wa2, "c2wb": c2wb, "c2w2": c2w2, "c2b": c2b, "c2gn": c2gn,
            "wa3": wa3, "wb3": wb3, "c3b": c3b, "c3gn": c3gn,
            "lw1": lw1, "lb1": lb1, "lw2": lw2, "lb2": lb2,
            "poolohtab": poh_in[c],
            "invg": invg_in[c],
        }
        in_maps.append(im)

    res = run_bass_kernel_spmd(nc, in_maps, core_ids=list(range(NC)),
                               trace=_TRACE[0])
    kernel.last_result = res
    kernel.last_meta = meta
    out = res.results[0]["out"]            # [2, 1000]
    return np.ascontiguousarray(out.T).astype(np.float32)


_DEBUG = [False]
_TRACE = [False]


# revision 20
# speedup vs baseline: 2.0106x; 1.0652x over previous
"""LundNetTagger GNN on 8 Trainium2 NeuronCores (Bass/Tile).

Self-contained: kernel(**inputs) -> np.ndarray [1000, 2] float32.

Strategy: nodes are assigned to 100352 "slots" (8 cores x 98 windows x 128),
packed so each window receives <= 512 edges. Edges live on the core owning
their dst slot, in window-major order padded to 4x128-edge chunks per window.
EdgeConv cat[xi, xj-xi] is folded into split weights WA = W[:C]-W[C:],
WB = W[C:].

conv1 runs feature-major from a host-packed message tensor, keeping z in SBUF.
conv2 layer1 expands the xi term from a per-node table A2 = x1 @ WA2 via
host-precomputed transposed one-hot masks (no per-edge xi matmuls) and gathers
xj rows of x1. conv3 is fully table-based: z_e = A3[dst] + B3[src] with
A3/B3 = x2 @ WA3 / x2 @ WB3 computed during conv2's scatter; the gather then
yields z directly in [edge, channel] layout, GraphNorm stats come from a
Gram-matrix matmul (sum + sum-of-squares in one accumulation), and the scatter
consumes [edge, channel] tiles with zero transposes. One-hot masks for
scatter/expansion/pool are host-precomputed bf16 tables. GraphNorm stats are
global AllReduces; mean-aggregation is a collision-free one-hot matmul scatter
into PSUM per window.
"""
import numpy as np
import ml_dtypes

import concourse.bass as bass
import concourse.tile as tile
from concourse import bacc, mybir
from concourse.bass_utils import run_bass_kernel_spmd

BF16 = mybir.dt.bfloat16
F32 = mybir.dt.float32
AOP = mybir.AluOpType
AFT = mybir.ActivationFunctionType
AX = mybir.AxisListType

N_NODES = 100000
N_EDGES = 400000
N_GRAPHS = 1000
NC = 8
WIN = 128
NWIN = 98
SPC = WIN * NWIN          # 12544
NSLOTS = SPC * NC         # 100352
QUAD = NSLOTS // 4        # 25088
B = 4                     # chunks per window
EPW = B * WIN             # 512
E_PAD = NWIN * EPW        # 50176
EPS = 1e-5

NW_BLK = 7
BLK = NW_BLK * EPW        # 3584
NBLK = NWIN // NW_BLK     # 14
NCHUNK = BLK // 128       # 28
NSEG = BLK // 512         # 7
NCH_ALL = E_PAD // 128    # 392
ZW = 257                  # z3 chunk width (256 channels + ones column)


_cache = {}


# ============================ host-side packing ============================

def _pack(edge_index, batch):
    src = np.asarray(edge_index[0], dtype=np.int64)
    dst = np.asarray(edge_index[1], dtype=np.int64)
    batch = np.asarray(batch, dtype=np.int64)
    cnt = np.bincount(dst, minlength=N_NODES)

    nvirt = NSLOTS - N_NODES
    cnt_all = np.concatenate([cnt, np.zeros(nvirt, dtype=cnt.dtype)])
    order = np.argsort(-cnt_all, kind="stable")
    GW = NWIN * NC
    rounds = NSLOTS // GW
    win_of_rank = np.empty(NSLOTS, dtype=np.int64)
    for r in range(rounds):
        seg = np.arange(GW) if r % 2 == 0 else np.arange(GW - 1, -1, -1)
        win_of_rank[r * GW:(r + 1) * GW] = seg
    win_of_node = np.empty(NSLOTS, dtype=np.int64)
    win_of_node[order] = win_of_rank
    wsum = np.bincount(win_of_node, weights=cnt_all.astype(np.float64),
                       minlength=GW).astype(np.int64)

    cap = EPW
    members_of = [list(np.where(win_of_node == w)[0]) for w in range(GW)]
    for _ in range(2000):
        over = np.where(wsum > cap)[0]
        if len(over) == 0:
            break
        w = int(over[0])
        # smallest-count >0 node in w
        mem = members_of[w]
        cs = [(int(cnt_all[n]), n) for n in mem if cnt_all[n] > 0]
        cs.sort()
        moved = False
        for c1, n in cs:
            # find target window with a smaller-count node to swap
            worder2 = np.argsort(wsum)
            for tw in worder2[:64]:
                tw = int(tw)
                if tw == w:
                    continue
                tmem = members_of[tw]
                best = None
                for m in tmem:
                    c2 = int(cnt_all[m])
                    if c2 < c1 and wsum[tw] + c1 - c2 <= cap:
                        if best is None or c2 < best[0]:
                            best = (c2, m)
                        if c2 == 0:
                            break
                if best is not None:
                    c2, m = best
                    members_of[tw].remove(m)
                    members_of[tw].append(n)
                    members_of[w].remove(n)
                    members_of[w].append(m)
                    win_of_node[n] = tw
                    win_of_node[m] = w
                    wsum[tw] += c1 - c2
                    wsum[w] -= c1 - c2
                    moved = True
                    break
            if moved:
                break
        if not moved:
            raise RuntimeError("packing fixup stuck")
    assert wsum.max() <= cap, f"window packing failed: max={wsum.max()}"

    worder = np.argsort(-wsum, kind="stable")
    core_load = np.zeros(NC, dtype=np.int64)
    core_nwin = np.zeros(NC, dtype=np.int64)
    core_of_win = np.empty(GW, dtype=np.int64)
    for w in worder:
        cands = np.where(core_nwin < NWIN)[0]
        c = cands[np.argmin(core_load[cands])]
        core_of_win[w] = c
        core_load[c] += wsum[w]
        core_nwin[c] += 1

    win_lists = [[] for _ in range(NC)]
    for w in range(GW):
        win_lists[core_of_win[w]].append(w)
    for c in range(NC):
        wl = win_lists[c]
        j = int(np.argmin(wsum[wl]))
        assert wsum[wl[j]] < cap, "no sentinel room"
        wl[j], wl[-1] = wl[-1], wl[j]

    slot_of_node = np.empty(NSLOTS, dtype=np.int64)
    for c in range(NC):
        for wi, w in enumerate(win_lists[c]):
            mem = np.sort(np.array(members_of[w], dtype=np.int64))
            assert len(mem) == WIN
            slot_of_node[mem] = c * SPC + wi * WIN + np.arange(WIN)
    node_of_slot = np.empty(NSLOTS, dtype=np.int64)
    node_of_slot[slot_of_node] = np.arange(NSLOTS)
    cnt_of_slot = cnt_all[node_of_slot]

    qzero = []
    for q in range(4):
        z = np.where(cnt_of_slot[q * QUAD:(q + 1) * QUAD] == 0)[0]
        assert len(z) > 0
        qzero.append(int(z[0]))  # local to quadrant
    czero = []
    for c in range(NC):
        z = np.where(cnt_of_slot[c * SPC:(c + 1) * SPC] == 0)[0]
        assert len(z) > 0
        czero.append(int(z[0]))  # local to core

    dslot = slot_of_node[dst]
    sslot = slot_of_node[src]
    ecore = dslot // SPC
    ewin = (dslot % SPC) // WIN
    key = ecore * (NWIN * WIN) + ewin * WIN + (dslot % WIN)
    eorder = np.argsort(key, kind="stable")
    dsl, ssl = dslot[eorder], sslot[eorder]
    ec, ew = ecore[eorder], ewin[eorder]

    cw = ec * NWIN + ew
    cw_cnt = np.bincount(cw, minlength=NC * NWIN)
    assert cw_cnt.max() <= EPW

    xi_idx = np.zeros((NC, E_PAD), dtype=np.int64)
    xj_idx = np.zeros((NC, E_PAD), dtype=np.int64)
    dstwin = np.full((NC, E_PAD), -1.0, dtype=np.float32)
    valid = np.zeros((NC, E_PAD), dtype=bool)

    ofs = (np.arange(NC * NWIN) % NWIN) * EPW
    start = np.concatenate([[0], np.cumsum(cw_cnt)[:-1]])
    within = np.arange(N_EDGES) - start[cw]
    pos = ofs[cw] + within
    xi_idx[ec, pos] = dsl % SPC
    xj_idx[ec, pos] = ssl
    dstwin[ec, pos] = (dsl % WIN).astype(np.float32)
    valid[ec, pos] = True
    for c in range(NC):
        xi_idx[c, ~valid[c]] = czero[c]
    pad_cnt = (~valid).sum(axis=1).astype(np.float32)
    assert np.all(~valid[:, -1]), "sentinel column must be padding"

    gzero = qzero[0]  # global slot with zero row
    xj_glob = np.where(valid, xj_idx, gzero).astype(np.int32)

    inv_cnt = (1.0 / np.maximum(cnt_of_slot.reshape(NC, SPC), 1.0)).astype(np.float32)

    g_of_slot = np.full(NSLOTS, -1, dtype=np.int64)
    real = node_of_slot < N_NODES
    g_of_slot[real] = batch[node_of_slot[real]]
    NGW = 8
    Bg = 0
    pools = [[None] * NGW for _ in range(NC)]
    for c in range(NC):
        gl = g_of_slot[c * SPC:(c + 1) * SPC]
        for gw in range(NGW):
            m = np.where((gl >= gw * 128) & (gl < (gw + 1) * 128))[0]
            pools[c][gw] = m
            Bg = max(Bg, (len(m) + 127) // 128)
    NPG = Bg * 128
    pool_idx = np.zeros((NC, NGW, NPG), dtype=np.int16)
    pool_gwl = np.full((NC, NGW, NPG), -1.0, dtype=np.float32)
    for c in range(NC):
        for gw in range(NGW):
            m = pools[c][gw]
            pool_idx[c, gw, :len(m)] = m.astype(np.int16)
            pool_idx[c, gw, len(m):] = czero[c]
            pool_gwl[c, gw, :len(m)] = (g_of_slot[c * SPC + m] - gw * 128).astype(np.float32)

    gcnt = np.bincount(batch, minlength=N_GRAPHS).astype(np.float32)
    inv_g = np.zeros(1024, dtype=np.float32)
    inv_g[:N_GRAPHS] = 1.0 / np.maximum(gcnt, 1.0)

    return dict(slot_of_node=slot_of_node, node_of_slot=node_of_slot,
                xj_glob=xj_glob, dstwin=dstwin, pad_cnt=pad_cnt,
                inv_cnt=inv_cnt, valid=valid, eorder=eorder, ec=ec, pos=pos,
                pool_idx=pool_idx, pool_gwl=pool_gwl, inv_g=inv_g, Bg=Bg,
                qzero=qzero)


def _wrap_idx(a):
    """[.., n] int -> [.., 128, n//16]: element i -> partition i%16 col i//16,
    replicated to 8 groups of 16 partitions."""
    n = a.shape[-1]
    assert n % 16 == 0
    w = a.reshape(*a.shape[:-1], n // 16, 16)
    w = np.swapaxes(w, -1, -2)
    w = np.broadcast_to(w[..., None, :, :], (*a.shape[:-1], 8, 16, n // 16))
    return np.ascontiguousarray(w).reshape(*a.shape[:-1], 128, n // 16).astype(np.int16)


def _bf(x):
    return np.ascontiguousarray(np.asarray(x, dtype=np.float32)).astype(ml_dtypes.bfloat16)


def _tile_w(w):
    K, M = w.shape
    nk, nm = (K + 127) // 128, (M + 127) // 128
    out = np.zeros((nk, nm, 128, 128), dtype=ml_dtypes.bfloat16)
    for i in range(nk):
        for j in range(nm):
            blk = np.asarray(w, dtype=np.float32)[i * 128:(i + 1) * 128, j * 128:(j + 1) * 128]
            out[i, j, :blk.shape[0], :blk.shape[1]] = _bf(blk)
    return out


# ============================ device kernel ============================

EHALF = E_PAD // 2        # 25088
NSEG_H = EHALF // 512     # 49


def _build(Bg, debug=False, phases=4):
    nc = bacc.Bacc("TRN2", target_bir_lowering=False, debug=False, num_devices=NC)

    def din(name, shape, dt):
        return nc.dram_tensor(name, shape, dt, kind="ExternalInput").ap()

    t_msgT = din("msgT", [48, EHALF], BF16)
    t_xj = din("xj_idx", [128, NCH_ALL], mybir.dt.int32)
    t_pidx = din("pool_idx", [8, 128, Bg], mybir.dt.int32)
    t_oh = din("ohtab", [128, E_PAD], BF16)
    t_ohT = din("ohTtab", [128, E_PAD], BF16)
    t_invcnt = din("invcnt", [128, NWIN], F32)
    t_padcnt = din("padcnt", [128, 1], F32)
    t_ident = din("ident", [128, 128], BF16)
    t_eyef = din("eyef", [128, 128], F32)
    t_c1w = din("c1w", [3, 128, 128], BF16)
    t_c1b = din("c1b", [3, 128, 1], F32)
    t_c1gn = din("c1gn", [3, 3, 128, 1], F32)
    t_wa2 = din("wa2", [128, 256], BF16)
    t_c2wb = din("c2wb", [2, 128, 128], BF16)
    t_c2w2 = din("c2w2", [2, 2, 128, 128], BF16)
    t_c2b = din("c2b", [2, 2, 128, 1], F32)
    t_c2gn = din("c2gn", [2, 3, 2, 128, 1], F32)
    t_wa3 = din("wa3", [2, 128, 256], BF16)
    t_wb3 = din("wb3", [2, 128, 256], BF16)
    t_c3b = din("c3b", [2, 128, 1], F32)
    t_c3gn = din("c3gn", [3, 2, 128, 1], F32)
    t_lw1 = din("lw1", [2, 2, 128, 128], BF16)
    t_lb1 = din("lb1", [2, 128, 1], F32)
    t_lw2 = din("lw2", [2, 128, 2], BF16)
    t_lb2 = din("lb2", [2, 1], F32)
    t_poh = din("poolohtab", [128, 8 * Bg * 128], BF16)
    t_invg = din("invg", [128, 8], F32)

    o_out = nc.dram_tensor("out", [2, N_GRAPHS], F32, kind="ExternalOutput").ap()
    dbg = {}
    if debug:
        dbg["x1"] = nc.dram_tensor("dbg_x1", [NSLOTS, 128], BF16, kind="ExternalOutput").ap()
        dbg["x2"] = nc.dram_tensor("dbg_x2", [SPC, 256], BF16, kind="ExternalOutput").ap()
        dbg["x3"] = nc.dram_tensor("dbg_x3", [SPC, 256], BF16, kind="ExternalOutput").ap()
        dbg["pool"] = nc.dram_tensor("dbg_pool", [1024, 256], F32, kind="ExternalOutput").ap()

    with tile.TileContext(nc) as tc:
        with tc.tile_pool(name="dram", bufs=1, space="DRAM") as dram, \
             tc.tile_pool(name="cp", bufs=1) as cp:
            z_scr = [dram.tile([2, 128, E_PAD], BF16, tag=f"zscr{i}", name=f"zscr{i}") for i in range(2)]
            z3_scr = dram.tile([128, NCH_ALL * ZW], BF16)
            tab1_loc = dram.tile([SPC, 128], BF16)
            tab1 = dram.tile([NSLOTS, 128], BF16)
            a2_loc = dram.tile([SPC, 256], BF16)
            a3_loc = dram.tile([SPC, 256], BF16)
            b3_loc = dram.tile([SPC, 256], BF16)
            b3_full = dram.tile([NSLOTS, 256], BF16)
            tab3_loc = dram.tile([SPC, 256], BF16)
            st_in = dram.tile([128, 8], F32)
            st_out = dram.tile([128, 8], F32)
            pool_in = dram.tile([1024, 256], F32)
            pool_out = dram.tile([1024, 256], F32)

            ident = cp.tile([128, 128], BF16)
            nc.sync.dma_start(ident[:], t_ident[:])
            eyef = cp.tile([128, 128], F32)
            nc.sync.dma_start(eyef[:], t_eyef[:])
            invcnt = cp.tile([128, NWIN], F32)
            nc.sync.dma_start(invcnt[:], t_invcnt[:])
            padcnt = cp.tile([128, 1], F32)
            nc.sync.dma_start(padcnt[:], t_padcnt[:])

            # ---------- helpers ----------
            def allreduce_stats(s_acc, q_acc, n_mb, sb):
                st = sb.tile([128, 8], F32, tag="st_")
                nc.vector.memset(st[:], 0.0)
                nc.vector.tensor_copy(st[:, 0:n_mb], s_acc[:])
                nc.vector.tensor_copy(st[:, 4:4 + n_mb], q_acc[:])
                nc.sync.dma_start(st_in[:], st[:])
                nc.gpsimd.collective_compute(
                    "AllReduce", AOP.add, replica_groups=[list(range(NC))],
                    ins=[st_in.opt()], outs=[st_out.opt()])
                stg = sb.tile([128, 8], F32, tag="stg_")
                nc.sync.dma_start(stg[:], st_out[:])
                return stg

            def affine_from_stats(stg, n_mb, b_lin, gn, sb):
                A, Cc = [], []
                for mb in range(n_mb):
                    s = stg[:, mb:mb + 1]
                    q = stg[:, 4 + mb:5 + mb]
                    g, bgn, ms = gn[0][mb], gn[1][mb], gn[2][mb]
                    bl = b_lin[mb]
                    m = sb.tile([128, 1], F32, tag="af_m")
                    nc.vector.tensor_scalar(m[:], s, 1.0 / N_EDGES, None, AOP.mult)
                    nc.vector.tensor_tensor(m[:], m[:], bl, op=AOP.add)
                    e2 = sb.tile([128, 1], F32, tag="af_e2")
                    nc.vector.tensor_scalar(e2[:], q, 1.0 / N_EDGES, None, AOP.mult)
                    tmp = sb.tile([128, 1], F32, tag="af_t")
                    nc.vector.tensor_tensor(tmp[:], m[:], bl, op=AOP.mult)
                    nc.vector.tensor_scalar(tmp[:], tmp[:], 2.0, None, AOP.mult)
                    nc.vector.tensor_tensor(e2[:], e2[:], tmp[:], op=AOP.add)
                    nc.vector.tensor_tensor(tmp[:], bl, bl, op=AOP.mult)
                    nc.vector.tensor_tensor(e2[:], e2[:], tmp[:], op=AOP.subtract)
                    msm = sb.tile([128, 1], F32, tag="af_msm")
                    nc.vector.tensor_tensor(msm[:], ms, m[:], op=AOP.mult)
                    var = sb.tile([128, 1], F32, tag="af_v")
                    nc.vector.tensor_tensor(var[:], msm[:], msm[:], op=AOP.mult)
                    nc.vector.tensor_tensor(tmp[:], msm[:], m[:], op=AOP.mult)
                    nc.vector.tensor_scalar(tmp[:], tmp[:], 2.0, None, AOP.mult)
                    nc.vector.tensor_tensor(var[:], var[:], tmp[:], op=AOP.subtract)
                    nc.vector.tensor_tensor(var[:], var[:], e2[:], op=AOP.add)
                    a = sb.tile([128, 1], F32, tag="af_a")
                    nc.vector.tensor_scalar(var[:], var[:], EPS, None, AOP.add)
                    nc.scalar.activation(a[:], var[:], AFT.Sqrt)
                    nc.vector.reciprocal(a[:], a[:])
                    nc.vector.tensor_tensor(a[:], a[:], g, op=AOP.mult)
                    cc = sb.tile([128, 1], F32, tag="af_c")
                    nc.vector.tensor_tensor(cc[:], bl, msm[:], op=AOP.subtract)
                    nc.vector.tensor_tensor(cc[:], cc[:], a[:], op=AOP.mult)
                    nc.vector.tensor_tensor(cc[:], cc[:], bgn, op=AOP.add)
                    A.append(a)
                    Cc.append(cc)
                return A, Cc

            sqscr = cp.tile([128, BLK], BF16)

            def acc_stats(ps_ap, s_col, q_col, sb, n=512, sq_scalar=False):
                t1 = sb.tile([128, 1], F32, tag="rs_t1")
                nc.vector.reduce_sum(out=t1[:], in_=ps_ap, axis=AX.X)
                nc.vector.tensor_tensor(s_col, s_col, t1[:], op=AOP.add)
                qa = sb.tile([128, 1], F32, tag="rs_qa")
                nc.scalar.activation(sqscr[:, :n], ps_ap, AFT.Square,
                                     accum_out=qa[:])
                nc.vector.tensor_tensor(q_col, q_col, qa[:], op=AOP.add)


            def sentinel_correct(s_acc, q_acc, zsent_cols, n_mb, sb):
                for mb in range(n_mb):
                    zs = zsent_cols[mb]
                    t1 = sb.tile([128, 1], F32, tag="sc_t1")
                    nc.vector.tensor_tensor(t1[:], zs, padcnt[:], op=AOP.mult)
                    nc.vector.tensor_tensor(s_acc[:, mb:mb + 1], s_acc[:, mb:mb + 1],
                                            t1[:], op=AOP.subtract)
                    nc.vector.tensor_tensor(t1[:], zs, zs, op=AOP.mult)
                    nc.vector.tensor_tensor(t1[:], t1[:], padcnt[:], op=AOP.mult)
                    nc.vector.tensor_tensor(q_acc[:, mb:mb + 1], q_acc[:, mb:mb + 1],
                                            t1[:], op=AOP.subtract)

            def load_vec(t_ap, sb, tag):
                v = sb.tile([128, 1], F32, tag=tag)
                nc.sync.dma_start(v[:], t_ap)
                return v[:]

            # ======================= CONV 1 =======================
            with tc.tile_pool(name="c1sb", bufs=2) as sb, \
                 tc.tile_pool(name="c1zb", bufs=1) as zbp:
                c1b = [[load_vec(t_c1b[i], sb, f"c1b{i}")] for i in range(3)]
                c1gn = [[[load_vec(t_c1gn[i, j], sb, f"c1gn{i}{j}")] for j in range(3)]
                        for i in range(3)]
                zbuf = zbp.tile([128, E_PAD], BF16)
                with tc.tile_pool(name="c1big", bufs=2) as bp, \
                     tc.tile_pool(name="c1ps", bufs=2, space="PSUM") as ps, \
                     tc.tile_pool(name="msgp", bufs=1) as msgp:
                    c1w = []
                    for i in range(3):
                        w = sb.tile([128, 128], BF16, tag=f"c1w{i}")
                        nc.sync.dma_start(w[:], t_c1w[i])
                        c1w.append(w)
                    msgT = msgp.tile([48, EHALF], BF16)
                    nc.sync.dma_start(msgT[:], t_msgT[:])

                    def z1_psum(h, s):
                        zp = ps.tile([128, 512], F32, tag="zp")
                        nc.tensor.matmul(zp[:], c1w[0][32 * h:32 * h + 10, :],
                                         msgT[32 * h:32 * h + 10, s * 512:(s + 1) * 512],
                                         start=True, stop=True)
                        return zp

                    s1 = sb.tile([128, 1], F32, tag="s1")
                    q1 = sb.tile([128, 1], F32, tag="q1")
                    nc.vector.memset(s1[:], 0.0)
                    nc.vector.memset(q1[:], 0.0)
                    for h in range(2):
                        for s in range(NSEG_H):
                            zp = z1_psum(h, s)
                            acc_stats(zp[:], s1[:, 0:1], q1[:, 0:1], sb,
                                      sq_scalar=True)
                    stg = allreduce_stats(s1, q1, 1, sb)
                    A1, C1 = affine_from_stats(stg, 1, c1b[0], c1gn[0], sb)

                    # L2: recompute z1, relu, z2 = W2 @ h1 -> zbuf (SBUF), batched stats
                    s2 = sb.tile([128, 1], F32, tag="s2")
                    q2 = sb.tile([128, 1], F32, tag="q2")
                    nc.vector.memset(s2[:], 0.0)
                    nc.vector.memset(q2[:], 0.0)
                    for h in range(2):
                        for b in range(NSEG_H // 7):
                            h1 = bp.tile([128, BLK], BF16, tag="h1")
                            for s in range(7):
                                zp = z1_psum(h, b * 7 + s)
                                nc.scalar.activation(h1[:, s * 512:(s + 1) * 512], zp[:],
                                                     AFT.Relu, bias=C1[0], scale=A1[0])
                            col0 = h * EHALF + b * BLK
                            for s in range(7):
                                zp = ps.tile([128, 512], F32, tag="zp")
                                nc.tensor.matmul(zp[:], c1w[1][:],
                                                 h1[:, s * 512:(s + 1) * 512],
                                                 start=True, stop=True)
                                nc.scalar.copy(
                                    zbuf[:, col0 + s * 512:col0 + (s + 1) * 512], zp[:])
                            acc_stats(zbuf[:, col0:col0 + BLK], s2[:, 0:1], q2[:, 0:1],
                                      sb, n=BLK)
                    zs2 = sb.tile([128, 1], F32, tag="zs2")
                    nc.vector.tensor_copy(zs2[:], zbuf[:, E_PAD - 1:E_PAD])
                    sentinel_correct(s2, q2, [zs2[:]], 1, sb)
                    stg2 = allreduce_stats(s2, q2, 1, sb)
                    A2, C2 = affine_from_stats(stg2, 1, c1b[1], c1gn[1], sb)

                    # L3: h2 = relu(aff(z2)), z3 = W3 @ h2 -> zbuf in place
                    s3 = sb.tile([128, 1], F32, tag="s3")
                    q3 = sb.tile([128, 1], F32, tag="q3")
                    nc.vector.memset(s3[:], 0.0)
                    nc.vector.memset(q3[:], 0.0)
                    for b in range(NBLK):
                        h2 = bp.tile([128, BLK], BF16, tag="h2")
                        nc.scalar.activation(h2[:], zbuf[:, b * BLK:(b + 1) * BLK],
                                             AFT.Relu, bias=C2[0], scale=A2[0])
                        for s in range(7):
                            zp = ps.tile([128, 512], F32, tag="zp")
                            nc.tensor.matmul(zp[:], c1w[2][:],
                                             h2[:, s * 512:(s + 1) * 512],
                                             start=True, stop=True)
                            nc.scalar.copy(
                                zbuf[:, b * BLK + s * 512:b * BLK + (s + 1) * 512], zp[:])
                        acc_stats(zbuf[:, b * BLK:(b + 1) * BLK], s3[:, 0:1], q3[:, 0:1],
                                  sb, n=BLK)
                    zs3 = sb.tile([128, 1], F32, tag="zs3")
                    nc.vector.tensor_copy(zs3[:], zbuf[:, E_PAD - 1:E_PAD])
                    sentinel_correct(s3, q3, [zs3[:]], 1, sb)
                    stg3 = allreduce_stats(s3, q3, 1, sb)
                    A3, C3 = affine_from_stats(stg3, 1, c1b[2], c1gn[2], sb)

                # scatter: h3 = relu(aff(z3)); transpose; one-hot matmul; + A2 table
                with tc.tile_pool(name="s1sb", bufs=2) as sp, \
                     tc.tile_pool(name="s1oh", bufs=2) as ohp, \
                     tc.tile_pool(name="s1tp", bufs=2, space="PSUM") as ps_tp, \
                     tc.tile_pool(name="s1sc", bufs=2, space="PSUM") as ps_sc:
                    wa2 = sp.tile([128, 256], BF16, tag="wa2")
                    nc.sync.dma_start(wa2[:], t_wa2[:])
                    for b in range(NBLK):
                        h3 = sp.tile([128, BLK], BF16, tag="h3")
                        nc.scalar.activation(h3[:], zbuf[:, b * BLK:(b + 1) * BLK],
                                             AFT.Relu, bias=C3[0], scale=A3[0])
                        ohb = ohp.tile([128, BLK], BF16, tag="ohb")
                        nc.sync.dma_start(ohb[:], t_oh[:, b * BLK:(b + 1) * BLK])
                        for w in range(NW_BLK):
                            gw = b * NW_BLK + w
                            tpp = ps_tp.tile([128, 512], BF16, tag="tpp", space="PSUM")
                            for cb in range(B):
                                nc.tensor.transpose(
                                    tpp[:, cb * 128:(cb + 1) * 128],
                                    h3[:, (w * B + cb) * 128:(w * B + cb + 1) * 128],
                                    ident[:])
                            hE = sp.tile([128, 512], BF16, tag="hE")
                            nc.vector.tensor_copy(hE[:], tpp[:])
                            sc = ps_sc.tile([128, 128], F32, tag="sc", space="PSUM")
                            for cb in range(B):
                                nc.tensor.matmul(
                                    sc[:], ohb[:, (w * B + cb) * 128:(w * B + cb + 1) * 128],
                                    hE[:, cb * 128:(cb + 1) * 128],
                                    start=(cb == 0), stop=(cb == B - 1))
                            nt = sp.tile([128, 128], BF16, tag="nt")
                            nc.vector.tensor_scalar(nt[:], sc[:], invcnt[:, gw:gw + 1],
                                                    None, AOP.mult)
                            nc.sync.dma_start(tab1_loc[gw * WIN:(gw + 1) * WIN, :], nt[:])
                            # A2 table: ntT then (x1_win) @ WA2
                            ntp = ps_tp.tile([128, 128], BF16, tag="ntp", space="PSUM")
                            nc.tensor.transpose(ntp[:], nt[:], ident[:])
                            ntT = sp.tile([128, 128], BF16, tag="ntT")
                            nc.vector.tensor_copy(ntT[:], ntp[:])
                            a2p = ps_sc.tile([128, 256], F32, tag="a2p", space="PSUM")
                            nc.tensor.matmul(a2p[:], ntT[:], wa2[:], start=True, stop=True)
                            a2t = sp.tile([128, 256], BF16, tag="a2t")
                            nc.vector.tensor_copy(a2t[:], a2p[:])
                            nc.sync.dma_start(a2_loc[gw * WIN:(gw + 1) * WIN, :], a2t[:])

            nc.gpsimd.collective_compute(
                "AllGather", AOP.bypass, replica_groups=[list(range(NC))],
                ins=[tab1_loc.opt()], outs=[tab1.opt()])
            if debug:
                nc.sync.dma_start(dbg["x1"][:], tab1[:])

            # ======================= CONV 2 =======================
            if phases >= 2:
              with tc.tile_pool(name="c2sb", bufs=2) as sb:
                  c2b = [[load_vec(t_c2b[i, mb], sb, f"c2b{i}{mb}") for mb in range(2)]
                         for i in range(2)]
                  c2gn = [[[load_vec(t_c2gn[i, j, mb], sb, f"c2gn{i}{j}{mb}")
                            for mb in range(2)] for j in range(3)] for i in range(2)]
                  # ---- pass 1: z1 = A2[dst] (one-hot expand) + WB2 @ x1[src] ----
                  sA = sb.tile([128, 2], F32, tag="c2sA")
                  qA = sb.tile([128, 2], F32, tag="c2qA")
                  nc.vector.memset(sA[:], 0.0)
                  nc.vector.memset(qA[:], 0.0)
                  with tc.tile_pool(name="g2g", bufs=2) as g2, \
                       tc.tile_pool(name="g2q", bufs=2) as gqp, \
                       tc.tile_pool(name="g2z", bufs=2) as zwp, \
                       tc.tile_pool(name="g2ps", bufs=2, space="PSUM") as ps, \
                       tc.tile_pool(name="g2tp", bufs=2, space="PSUM") as ps_tp:
                      wbs = []
                      for mo in range(2):
                          wtb = sb.tile([128, 128], BF16, tag=f"c2wb{mo}")
                          nc.sync.dma_start(wtb[:], t_c2wb[mo])
                          wbs.append(wtb)
                      for b in range(NBLK):
                          ixj = g2.tile([128, NCHUNK], mybir.dt.int32, tag="ixj")
                          nc.sync.dma_start(ixj[:], t_xj[:, b * NCHUNK:(b + 1) * NCHUNK])
                          gxj = gqp.tile([128, NCHUNK * 128], BF16, tag="gxj")
                          for ch in range(NCHUNK):
                              nc.gpsimd.indirect_dma_start(
                                  out=gxj[:, ch * 128:(ch + 1) * 128],
                                  out_offset=None,
                                  in_=tab1[:],
                                  in_offset=bass.IndirectOffsetOnAxis(
                                      ap=ixj[:, ch:ch + 1], axis=0))
                          ohTb = g2.tile([128, BLK], BF16, tag="ohTb")
                          nc.sync.dma_start(ohTb[:], t_ohT[:, b * BLK:(b + 1) * BLK])
                          zsb = [zwp.tile([128, BLK], BF16, tag=f"zsb{h}", name=f"zsb{h}")
                                 for h in range(2)]
                          for w in range(NW_BLK):
                              gw = b * NW_BLK + w
                              a2w = g2.tile([128, 256], BF16, tag="a2w")
                              nc.sync.dma_start(a2w[:], a2_loc[gw * WIN:(gw + 1) * WIN, :])
                              tpp = ps_tp.tile([128, 512], BF16, tag="xtp", space="PSUM")
                              for cb in range(B):
                                  nc.tensor.transpose(
                                      tpp[:, cb * 128:(cb + 1) * 128],
                                      gxj[:, (w * B + cb) * 128:(w * B + cb + 1) * 128],
                                      ident[:])
                              xjT = g2.tile([128, 512], BF16, tag="xjT")
                              nc.vector.tensor_copy(xjT[:], tpp[:])
                              for h in range(2):
                                  zp = ps.tile([128, 512], F32, tag="zp")
                                  nc.tensor.matmul(zp[:], a2w[:, h * 128:(h + 1) * 128],
                                                   ohTb[:, w * 512:(w + 1) * 512],
                                                   start=True, stop=False)
                                  nc.tensor.matmul(zp[:], wbs[h][:], xjT[:],
                                                   start=False, stop=True)
                                  nc.scalar.copy(zsb[h][:, w * 512:(w + 1) * 512], zp[:])
                          for h in range(2):
                              acc_stats(zsb[h][:], sA[:, h:h + 1], qA[:, h:h + 1],
                                        sb, n=BLK)
                              nc.sync.dma_start(z_scr[0][h, :, b * BLK:(b + 1) * BLK],
                                                zsb[h][:])
                  stg = allreduce_stats(sA, qA, 2, sb)
                  A1, C1 = affine_from_stats(stg, 2, c2b[0], c2gn[0], sb)

                  # ---- layer 2 ----
                  s2 = sb.tile([128, 2], F32, tag="c2s2")
                  q2 = sb.tile([128, 2], F32, tag="c2q2")
                  nc.vector.memset(s2[:], 0.0)
                  nc.vector.memset(q2[:], 0.0)
                  zlast = [None, None]
                  with tc.tile_pool(name="c2mid", bufs=2) as mp, \
                       tc.tile_pool(name="c2ps", bufs=2, space="PSUM") as ps:
                      w2s = []
                      for ki in range(2):
                          for mo in range(2):
                              w = sb.tile([128, 128], BF16, tag=f"c2w2{ki}{mo}")
                              nc.sync.dma_start(w[:], t_c2w2[ki, mo])
                              w2s.append(w)
                      for b in range(NBLK):
                          h1 = []
                          for mb in range(2):
                              z = mp.tile([128, BLK], BF16, tag=f"c2z1r{mb}")
                              nc.sync.dma_start(z[:], z_scr[0][mb, :, b * BLK:(b + 1) * BLK])
                              hh = mp.tile([128, BLK], BF16, tag=f"c2h1{mb}")
                              nc.vector.tensor_scalar(hh[:], z[:], A1[mb], C1[mb],
                                                      AOP.mult, AOP.add)
                              nc.vector.tensor_scalar(hh[:], hh[:], 0.0, None, AOP.max)
                              h1.append(hh)
                          for mo in range(2):
                              zw = mp.tile([128, BLK], BF16, tag=f"c2z2w{mo}")
                              for s in range(NSEG):
                                  zp = ps.tile([128, 512], F32, tag="c2zp")
                                  for ki in range(2):
                                      nc.tensor.matmul(zp[:], w2s[ki * 2 + mo][:],
                                                       h1[ki][:, s * 512:(s + 1) * 512],
                                                       start=(ki == 0), stop=(ki == 1))
                                  nc.scalar.copy(zw[:, s * 512:(s + 1) * 512], zp[:])
                              acc_stats(zw[:], s2[:, mo:mo + 1], q2[:, mo:mo + 1],
                                        sb, n=BLK)
                              nc.sync.dma_start(z_scr[1][mo, :, b * BLK:(b + 1) * BLK], zw[:])
                              zlast[mo] = zw
                      zsent = []
                      for mo in range(2):
                          zc = sb.tile([128, 1], F32, tag=f"c2zs{mo}")
                          nc.vector.tensor_copy(zc[:], zlast[mo][:, BLK - 1:BLK])
                          zsent.append(zc[:])
                  sentinel_correct(s2, q2, zsent, 2, sb)
                  stg2 = allreduce_stats(s2, q2, 2, sb)
                  A2, C2 = affine_from_stats(stg2, 2, c2b[1], c2gn[1], sb)

                  # ---- scatter + A3/B3 tables ----
                  with tc.tile_pool(name="s2sb", bufs=2) as sp, \
                       tc.tile_pool(name="s2oh", bufs=2) as ohp, \
                       tc.tile_pool(name="s2tp", bufs=2, space="PSUM") as ps_tp, \
                       tc.tile_pool(name="s2sc", bufs=2, space="PSUM") as ps_sc:
                      wab3 = sp.tile([128, 1024], BF16, tag="wab3")
                      for ki in range(2):
                          nc.sync.dma_start(wab3[:, ki * 512:ki * 512 + 256], t_wa3[ki])
                          nc.sync.dma_start(wab3[:, ki * 512 + 256:ki * 512 + 512],
                                            t_wb3[ki])
                      for b in range(NBLK):
                          hs = []
                          for mb in range(2):
                              z = sp.tile([128, BLK], BF16, tag=f"s2z{mb}")
                              nc.sync.dma_start(z[:], z_scr[1][mb, :, b * BLK:(b + 1) * BLK])
                              h = sp.tile([128, BLK], BF16, tag=f"s2h{mb}")
                              nc.scalar.activation(h[:], z[:], AFT.Relu,
                                                   bias=C2[mb], scale=A2[mb])
                              hs.append(h)
                          ohb = ohp.tile([128, BLK], BF16, tag="ohb2")
                          nc.sync.dma_start(ohb[:], t_oh[:, b * BLK:(b + 1) * BLK])
                          for w in range(NW_BLK):
                              gw = b * NW_BLK + w
                              tpp = ps_tp.tile([128, 1024], BF16, tag="tpp2", space="PSUM")
                              for cb in range(B):
                                  for mb in range(2):
                                      nc.tensor.transpose(
                                          tpp[:, (cb * 2 + mb) * 128:(cb * 2 + mb + 1) * 128],
                                          hs[mb][:, (w * B + cb) * 128:(w * B + cb + 1) * 128],
                                          ident[:])
                              hE = sp.tile([128, 1024], BF16, tag="hE2")
                              nc.vector.tensor_copy(hE[:], tpp[:])
                              sc = ps_sc.tile([128, 256], F32, tag="sc2", space="PSUM")
                              for cb in range(B):
                                  nc.tensor.matmul(
                                      sc[:], ohb[:, (w * B + cb) * 128:(w * B + cb + 1) * 128],
                                      hE[:, cb * 256:(cb + 1) * 256],
                                      start=(cb == 0), stop=(cb == B - 1))
                              nt = sp.tile([128, 256], BF16, tag="nt2")
                              nc.vector.tensor_scalar(nt[:], sc[:], invcnt[:, gw:gw + 1],
                                                      None, AOP.mult)
                              if debug:
                                  nc.sync.dma_start(dbg["x2"][gw * WIN:(gw + 1) * WIN, :], nt[:])
                              ntp = ps_tp.tile([128, 256], BF16, tag="ntp2", space="PSUM")
                              for ki in range(2):
                                  nc.tensor.transpose(ntp[:, ki * 128:(ki + 1) * 128],
                                                      nt[:, ki * 128:(ki + 1) * 128],
                                                      ident[:])
                              ntT = sp.tile([128, 256], BF16, tag="ntT2")
                              nc.vector.tensor_copy(ntT[:], ntp[:])
                              abp = ps_sc.tile([128, 512], F32, tag="abp", space="PSUM")
                              for ki in range(2):
                                  nc.tensor.matmul(abp[:], ntT[:, ki * 128:(ki + 1) * 128],
                                                   wab3[:, ki * 512:(ki + 1) * 512],
                                                   start=(ki == 0), stop=(ki == 1))
                              abt = sp.tile([128, 512], BF16, tag="abt")
                              nc.vector.tensor_copy(abt[:], abp[:])
                              nc.sync.dma_start(a3_loc[gw * WIN:(gw + 1) * WIN, :],
                                                abt[:, 0:256])
                              nc.sync.dma_start(b3_loc[gw * WIN:(gw + 1) * WIN, :],
                                                abt[:, 256:512])

            nc.gpsimd.collective_compute(
                "AllGather", AOP.bypass, replica_groups=[list(range(NC))],
                ins=[b3_loc.opt()], outs=[b3_full.opt()])

            # ======================= CONV 3 =======================
            if phases >= 3:
              with tc.tile_pool(name="c3sb", bufs=2) as sb:
                  c3b = [load_vec(t_c3b[mb], sb, f"c3b{mb}") for mb in range(2)]
                  c3gn = [[load_vec(t_c3gn[j, mb], sb, f"c3gn{j}{mb}") for mb in range(2)]
                          for j in range(3)]
                  G1 = sb.tile([128, ZW], F32, tag="G1")
                  G2 = sb.tile([128, ZW], F32, tag="G2")
                  nc.vector.memset(G1[:], 0.0)
                  nc.vector.memset(G2[:], 0.0)
                  # ---- pass 1: z = A3[dst] + B3[src]; Gram stats; spill z ----
                  with tc.tile_pool(name="c3g", bufs=2) as g3, \
                       tc.tile_pool(name="c3q", bufs=2) as gqp3, \
                       tc.tile_pool(name="c3zt", bufs=2) as ztp, \
                       tc.tile_pool(name="c3ps", bufs=4, space="PSUM") as ps, \
                       tc.tile_pool(name="c3gp", bufs=2, space="PSUM") as psg:
                      for b in range(NBLK):
                          ixj = g3.tile([128, NCHUNK], mybir.dt.int32, tag="ixj3")
                          nc.sync.dma_start(ixj[:], t_xj[:, b * NCHUNK:(b + 1) * NCHUNK])
                          gb = gqp3.tile([128, NCHUNK * 256], BF16, tag="gb3")
                          for ch in range(NCHUNK):
                              nc.gpsimd.indirect_dma_start(
                                  out=gb[:, ch * 256:(ch + 1) * 256],
                                  out_offset=None,
                                  in_=b3_full[:],
                                  in_offset=bass.IndirectOffsetOnAxis(
                                      ap=ixj[:, ch:ch + 1], axis=0))
                          zt = ztp.tile([128, NCHUNK * ZW], BF16, tag="zt")
                          ones_ap = zt[:].rearrange("p (c k) -> p c k", k=ZW)[:, :, 256:257]
                          nc.vector.memset(ones_ap, 1.0)
                          ohTb = g3.tile([128, BLK], BF16, tag="ohTb3")
                          nc.sync.dma_start(ohTb[:], t_ohT[:, b * BLK:(b + 1) * BLK])
                          for w in range(NW_BLK):
                              gw = b * NW_BLK + w
                              a3w = g3.tile([128, 256], BF16, tag="a3w")
                              nc.sync.dma_start(a3w[:], a3_loc[gw * WIN:(gw + 1) * WIN, :])
                              for cb in range(B):
                                  ch = w * B + cb
                                  ap_ = ps.tile([128, 256], F32, tag="aexp")
                                  nc.tensor.matmul(ap_[:],
                                                   ohTb[:, ch * 128:(ch + 1) * 128],
                                                   a3w[:], start=True, stop=True)
                                  nc.vector.tensor_tensor(
                                      zt[:, ch * ZW:ch * ZW + 256],
                                      gb[:, ch * 256:(ch + 1) * 256],
                                      ap_[:], op=AOP.add)
                          G1p = psg.tile([128, ZW], F32, tag="G1p", space="PSUM")
                          G2p = psg.tile([128, ZW], F32, tag="G2p", space="PSUM")
                          for ch in range(NCHUNK):
                              nc.tensor.matmul(G1p[:], zt[:, ch * ZW:ch * ZW + 128],
                                               zt[:, ch * ZW:ch * ZW + ZW],
                                               start=(ch == 0), stop=(ch == NCHUNK - 1),
                                               skip_group_check=True)
                              nc.tensor.matmul(G2p[:], zt[:, ch * ZW + 128:ch * ZW + 256],
                                               zt[:, ch * ZW:ch * ZW + ZW],
                                               start=(ch == 0), stop=(ch == NCHUNK - 1),
                                               skip_group_check=True)
                          nc.vector.tensor_tensor(G1[:], G1[:], G1p[:], op=AOP.add)
                          nc.vector.tensor_tensor(G2[:], G2[:], G2p[:], op=AOP.add)
                          nc.sync.dma_start(
                              z3_scr[:, b * NCHUNK * ZW:(b + 1) * NCHUNK * ZW], zt[:])
                  # stats: sums = G[:,256]; sumsq = diag
                  sA = sb.tile([128, 2], F32, tag="c3sA")
                  qA = sb.tile([128, 2], F32, tag="c3qA")
                  nc.vector.tensor_copy(sA[:, 0:1], G1[:, 256:257])
                  nc.vector.tensor_copy(sA[:, 1:2], G2[:, 256:257])
                  dtmp = sb.tile([128, 128], F32, tag="dtmp")
                  nc.vector.tensor_tensor(dtmp[:], G1[:, 0:128], eyef[:], op=AOP.mult)
                  nc.vector.reduce_sum(out=qA[:, 0:1], in_=dtmp[:], axis=AX.X)
                  nc.vector.tensor_tensor(dtmp[:], G2[:, 128:256], eyef[:], op=AOP.mult)
                  nc.vector.reduce_sum(out=qA[:, 1:2], in_=dtmp[:], axis=AX.X)
                  stg = allreduce_stats(sA, qA, 2, sb)
                  A1, C1 = affine_from_stats(stg, 2, c3b, c3gn, sb)
                  # broadcast affine rows: a_bc/c_bc [128, ZW] bf16
                  a_bc = sb.tile([128, ZW], BF16, tag="a_bc")
                  c_bc = sb.tile([128, ZW], BF16, tag="c_bc")
                  with tc.tile_pool(name="c3bp", bufs=2, space="PSUM") as psb:
                      for dst_t, vals in ((a_bc, A1), (c_bc, C1)):
                          nc.vector.memset(dst_t[:], 0.0)
                          for mb in range(2):
                              tp = psb.tile([128, 128], F32, tag="bcp", space="PSUM")
                              nc.tensor.transpose(
                                  tp[:], vals[mb][:, 0:1].to_broadcast([128, 128]),
                                  eyef[:])
                              nc.vector.tensor_copy(dst_t[:, mb * 128:(mb + 1) * 128], tp[:])
                  # ---- pass 2: h = relu(a*z + c); scatter ----
                  with tc.tile_pool(name="c3p2", bufs=2) as p2, \
                       tc.tile_pool(name="c3oh", bufs=2) as ohp, \
                       tc.tile_pool(name="c3sc", bufs=2, space="PSUM") as ps_sc:
                      for b in range(NBLK):
                          zt = p2.tile([128, NCHUNK * ZW], BF16, tag="zt2")
                          nc.sync.dma_start(
                              zt[:], z3_scr[:, b * NCHUNK * ZW:(b + 1) * NCHUNK * ZW])
                          h = p2.tile([128, NCHUNK * ZW], BF16, tag="h3b")
                          nc.vector.tensor_tensor(
                              out=h[:].rearrange("p (c k) -> p c k", k=ZW),
                              in0=zt[:].rearrange("p (c k) -> p c k", k=ZW),
                              in1=a_bc[:, None, :].to_broadcast((128, NCHUNK, ZW)),
                              op=AOP.mult)
                          nc.vector.tensor_tensor(
                              out=h[:].rearrange("p (c k) -> p c k", k=ZW),
                              in0=h[:].rearrange("p (c k) -> p c k", k=ZW),
                              in1=c_bc[:, None, :].to_broadcast((128, NCHUNK, ZW)),
                              op=AOP.add)
                          nc.scalar.activation(h[:], h[:], AFT.Relu)
                          ohb = ohp.tile([128, BLK], BF16, tag="ohb3")
                          nc.sync.dma_start(ohb[:], t_oh[:, b * BLK:(b + 1) * BLK])
                          for w in range(NW_BLK):
                              gw = b * NW_BLK + w
                              sc = ps_sc.tile([128, 256], F32, tag="sc3", space="PSUM")
                              for cb in range(B):
                                  ch = w * B + cb
                                  nc.tensor.matmul(
                                      sc[:], ohb[:, ch * 128:(ch + 1) * 128],
                                      h[:, ch * ZW:ch * ZW + 256],
                                      start=(cb == 0), stop=(cb == B - 1))
                              nt = p2.tile([128, 256], BF16, tag="nt3")
                              nc.vector.tensor_scalar(nt[:], sc[:], invcnt[:, gw:gw + 1],
                                                      None, AOP.mult)
                              nc.sync.dma_start(tab3_loc[gw * WIN:(gw + 1) * WIN, :], nt[:])

            if debug:
                nc.sync.dma_start(dbg["x3"][:], tab3_loc[:])

            # ======================= POOL + HEAD =======================
            if phases >= 4:
              with tc.tile_pool(name="p_sb", bufs=2) as sb, \
                 tc.tile_pool(name="p_ps", bufs=2, space="PSUM") as ps:
                  for gw in range(8):
                      pidx = sb.tile([128, Bg], mybir.dt.int32, tag="p_idx")
                      nc.sync.dma_start(pidx[:], t_pidx[gw])
                      poh = sb.tile([128, Bg * 128], BF16, tag="p_poh")
                      nc.sync.dma_start(poh[:],
                                        t_poh[:, gw * Bg * 128:(gw + 1) * Bg * 128])
                      gp = sb.tile([128, Bg * 256], BF16, tag="p_gp")
                      for c in range(Bg):
                          nc.gpsimd.indirect_dma_start(
                              out=gp[:, c * 256:(c + 1) * 256], out_offset=None,
                              in_=tab3_loc[:],
                              in_offset=bass.IndirectOffsetOnAxis(
                                  ap=pidx[:, c:c + 1], axis=0))
                      pp = ps.tile([128, 256], F32, tag="p_pp", space="PSUM")
                      for c in range(Bg):
                          nc.tensor.matmul(pp[:], poh[:, c * 128:(c + 1) * 128],
                                           gp[:, c * 256:(c + 1) * 256],
                                           start=(c == 0), stop=(c == Bg - 1))
                      pf = sb.tile([128, 256], F32, tag="p_pf")
                      nc.vector.tensor_copy(pf[:], pp[:])
                      nc.sync.dma_start(pool_in[gw * 128:(gw + 1) * 128, :], pf[:])
                  nc.gpsimd.collective_compute(
                      "AllReduce", AOP.add, replica_groups=[list(range(NC))],
                      ins=[pool_in.opt()], outs=[pool_out.opt()])
                  if debug:
                      nc.sync.dma_start(dbg["pool"][:], pool_out[:])

                  invg = sb.tile([128, 8], F32, tag="p_invg")
                  nc.sync.dma_start(invg[:], t_invg[:])
                  lw1 = []
                  for ki in range(2):
                      for mo in range(2):
                          w = sb.tile([128, 128], BF16, tag=f"p_lw1{ki}{mo}")
                          nc.sync.dma_start(w[:], t_lw1[ki, mo])
                          lw1.append(w)
                  lw2 = []
                  for ki in range(2):
                      w = sb.tile([128, 2], BF16, tag=f"p_lw2{ki}")
                      nc.sync.dma_start(w[:], t_lw2[ki])
                      lw2.append(w)
                  lb1 = [load_vec(t_lb1[mb], sb, f"p_lb1{mb}") for mb in range(2)]
                  lb2 = sb.tile([2, 1], F32, tag="p_lb2")
                  nc.sync.dma_start(lb2[:], t_lb2[:])
                  ofin = sb.tile([2, 1024], F32, tag="p_out")
                  for gw in range(8):
                      g = sb.tile([128, 256], F32, tag="p_g")
                      nc.sync.dma_start(g[:], pool_out[gw * 128:(gw + 1) * 128, :])
                      gm = sb.tile([128, 256], BF16, tag="p_gm")
                      nc.vector.tensor_scalar(gm[:], g[:], invg[:, gw:gw + 1], None, AOP.mult)
                      gT = sb.tile([128, 2 * 128], BF16, tag="p_gT")
                      for kb in range(2):
                          tp = ps.tile([128, 128], BF16, tag="p_tp", space="PSUM")
                          nc.tensor.transpose(tp[:], gm[:, kb * 128:(kb + 1) * 128], ident[:])
                          nc.vector.tensor_copy(gT[:, kb * 128:(kb + 1) * 128], tp[:])
                      hT = sb.tile([128, 2 * 128], BF16, tag="p_hT")
                      for mo in range(2):
                          hp = ps.tile([128, 128], F32, tag="p_hp", space="PSUM")
                          for ki in range(2):
                              nc.tensor.matmul(hp[:], lw1[ki * 2 + mo][:],
                                               gT[:, ki * 128:(ki + 1) * 128],
                                               start=(ki == 0), stop=(ki == 1))
                          nc.scalar.activation(hT[:, mo * 128:(mo + 1) * 128], hp[:],
                                               AFT.Relu, bias=lb1[mo])
                      op_ = ps.tile([2, 128], F32, tag="p_op", space="PSUM")
                      for ki in range(2):
                          nc.tensor.matmul(op_[:], lw2[ki][:],
                                           hT[:, ki * 128:(ki + 1) * 128],
                                           start=(ki == 0), stop=(ki == 1))
                      nc.vector.tensor_scalar(ofin[:, gw * 128:(gw + 1) * 128],
                                              op_[:], lb2[:], None, AOP.add)
                  nc.sync.dma_start(o_out[:], ofin[:, :N_GRAPHS])

    nc.compile()
    return nc


# ============================ entry point ============================


def kernel(**inputs):
    x = np.asarray(inputs["x"], dtype=np.float32)
    edge_index = np.asarray(inputs["edge_index"])
    batch = np.asarray(inputs["batch"])

    meta = _pack(edge_index, batch)
    Bg = meta["Bg"]

    import os as _os
    phases = int(_os.environ.get("KPHASES", "4"))
    key = ("mod", Bg, phases, _DEBUG[0])
    if key not in _cache:
        _cache[key] = _build(Bg, debug=bool(inputs.get("_debug", False)) or _DEBUG[0],
                             phases=phases)
    nc = _cache[key]

    # ---- per-core input arrays ----
    src = np.asarray(edge_index[0], dtype=np.int64)
    dst = np.asarray(edge_index[1], dtype=np.int64)

    # conv1 msgT: [core, 48, E_PAD//2] bf16; edge e<EHALF -> rows 0..9 col e,
    # e>=EHALF -> rows 32..41 col e-EHALF
    xi_v = x[dst]
    xj_v = x[src]
    msg = np.concatenate([xi_v, xj_v - xi_v], axis=1)       # [E, 10]
    msg_full = np.zeros((NC, E_PAD, 10), dtype=np.float32)
    ec, pos = meta["ec"], meta["pos"]
    msg_full[ec, pos] = msg[meta["eorder"]]
    msgT = np.zeros((NC, 48, EHALF), dtype=ml_dtypes.bfloat16)
    msgT[:, :10, :] = _bf(msg_full[:, :EHALF].transpose(0, 2, 1))
    msgT[:, 32:42, :] = _bf(msg_full[:, EHALF:].transpose(0, 2, 1))

    # one-hot tables from dstwin
    dstwin = meta["dstwin"]  # [NC, E_PAD], float; -1 for padding
    dw = dstwin.reshape(NC, NCH_ALL, 128).astype(np.int32)
    nn_ = np.arange(128, dtype=np.int32)
    oh_in = np.empty((NC, 128, E_PAD), dtype=ml_dtypes.bfloat16)
    ohT_in = np.empty((NC, 128, E_PAD), dtype=ml_dtypes.bfloat16)
    for c in range(NC):
        m = (dw[c][:, :, None] == nn_[None, None, :])     # [392, 128e, 128n]
        oh_in[c] = m.transpose(1, 0, 2).reshape(128, E_PAD).astype(ml_dtypes.bfloat16)
        ohT_in[c] = m.transpose(2, 0, 1).reshape(128, E_PAD).astype(ml_dtypes.bfloat16)

    invcnt_in = np.ascontiguousarray(
        meta["inv_cnt"].reshape(NC, NWIN, 128).transpose(0, 2, 1)).astype(np.float32)
    padcnt_in = np.repeat(meta["pad_cnt"][:, None], 128, axis=1)[:, :, None].astype(np.float32)

    ident_in = np.eye(128, dtype=np.float32).astype(ml_dtypes.bfloat16)
    eyef_in = np.eye(128, dtype=np.float32)

    xj_in = np.ascontiguousarray(
        meta["xj_glob"].reshape(NC, NCH_ALL, 128).transpose(0, 2, 1)).astype(np.int32)

    # weights
    c1w = np.zeros((3, 128, 128), dtype=ml_dtypes.bfloat16)
    c1w[0, :10, :] = _bf(inputs["c1_w1"])
    c1w[0, 32:42, :] = _bf(inputs["c1_w1"])
    c1w[1] = _bf(inputs["c1_w2"])
    c1w[2] = _bf(inputs["c1_w3"])
    c1b = np.stack([np.asarray(inputs[f"c1_b{i}"], dtype=np.float32).reshape(128, 1)
                    for i in (1, 2, 3)])
    c1gn = np.stack([np.asarray(inputs[f"c1_gn{i}"], dtype=np.float32).reshape(3, 128, 1)
                     for i in (1, 2, 3)])

    w2a = np.asarray(inputs["c2_w1"], dtype=np.float32)   # [256, 256]
    WA2 = w2a[:128] - w2a[128:]
    WB2 = w2a[128:]
    wa2 = _bf(WA2)                                        # [128, 256]
    c2wb = _tile_w(WB2)[0]                                # [2(mo), 128, 128]
    c2w2 = _tile_w(np.asarray(inputs["c2_w2"], dtype=np.float32))  # [2,2,128,128]
    c2b = np.stack([np.asarray(inputs["c2_b1"], dtype=np.float32).reshape(2, 128, 1),
                    np.asarray(inputs["c2_b2"], dtype=np.float32).reshape(2, 128, 1)])
    c2gn = np.stack([np.asarray(inputs["c2_gn1"], dtype=np.float32).reshape(3, 2, 128, 1),
                     np.asarray(inputs["c2_gn2"], dtype=np.float32).reshape(3, 2, 128, 1)])

    w3a = np.asarray(inputs["c3_w1"], dtype=np.float32)   # [512, 256]
    WA3 = w3a[:256] - w3a[256:]
    WB3 = w3a[256:]
    wa3 = _bf(WA3).reshape(2, 128, 256)
    wb3 = _bf(WB3).reshape(2, 128, 256)
    c3b = np.asarray(inputs["c3_b1"], dtype=np.float32).reshape(2, 128, 1)
    c3gn = np.asarray(inputs["c3_gn1"], dtype=np.float32).reshape(3, 2, 128, 1)

    lw1 = _tile_w(np.asarray(inputs["lin_w1"], dtype=np.float32))
    lb1 = np.asarray(inputs["lin_b1"], dtype=np.float32).reshape(2, 128, 1)
    lw2_f = np.asarray(inputs["lin_w2"], dtype=np.float32)  # [256, 2]
    lw2 = np.stack([_bf(lw2_f[:128]), _bf(lw2_f[128:])])    # [2, 128, 2]
    lb2 = np.asarray(inputs["lin_b2"], dtype=np.float32).reshape(2, 1)

    pidx_in = np.ascontiguousarray(
        meta["pool_idx"].astype(np.int32).reshape(NC, 8, Bg, 128).transpose(0, 1, 3, 2))
    pgwl = meta["pool_gwl"].reshape(NC, 8, Bg, 128)        # [c, gw, cs, p]
    gg = np.arange(128, dtype=np.float32)
    poh_in = np.empty((NC, 128, 8 * Bg * 128), dtype=ml_dtypes.bfloat16)
    for c in range(NC):
        m = (pgwl[c][:, :, :, None] == gg[None, None, None, :])  # [8, Bg, 128p, 128g]
        poh_in[c] = m.transpose(2, 0, 1, 3).reshape(128, 8 * Bg * 128).astype(
            ml_dtypes.bfloat16)
    invg_in = np.broadcast_to(
        meta["inv_g"].reshape(8, 128).T[None], (NC, 128, 8)).astype(np.float32)
    invg_in = np.ascontiguousarray(invg_in)

    in_maps = []
    for c in range(NC):
        im = {
            "msgT": msgT[c],
            "xj_idx": xj_in[c],
            "pool_idx": pidx_in[c],
            "ohtab": oh_in[c],
            "ohTtab": ohT_in[c],
            "invcnt": invcnt_in[c],
            "padcnt": padcnt_in[c],
            "ident": ident_in,
            "eyef": eyef_in,
            "c1w": c1w, "c1b": c1b, "c1gn": c1gn,
            "wa2": wa2, "c2wb": c2wb, "c2w2": c2w2, "c2b": c2b, "c2gn": c2gn,
            "wa3": wa3, "wb3": wb3, "c3b": c3b, "c3gn": c3gn,
            "lw1": lw1, "lb1": lb1, "lw2": lw2, "lb2": lb2,
            "poolohtab": poh_in[c],
            "invg": invg_in[c],
        }
        in_maps.append(im)

    res = run_bass_kernel_spmd(nc, in_maps, core_ids=list(range(NC)),
                               trace=_TRACE[0])
    kernel.last_result = res
    kernel.last_meta = meta
    out = res.results[0]["out"]            # [2, 1000]
    return np.ascontiguousarray(out.T).astype(np.float32)


_DEBUG = [False]
_TRACE = [False]


# revision 21
# speedup vs baseline: 2.0448x; 1.0170x over previous
"""LundNetTagger GNN on 8 Trainium2 NeuronCores (Bass/Tile).

Self-contained: kernel(**inputs) -> np.ndarray [1000, 2] float32.

Strategy: nodes are assigned to 100352 "slots" (8 cores x 98 windows x 128),
packed so each window receives <= 512 edges. Edges live on the core owning
their dst slot, in window-major order padded to 4x128-edge chunks per window.
EdgeConv cat[xi, xj-xi] is folded into split weights WA = W[:C]-W[C:],
WB = W[C:].

conv1 runs feature-major from a host-packed message tensor, keeping z in SBUF.
conv2 layer1 expands the xi term from a per-node table A2 = x1 @ WA2 via
host-precomputed transposed one-hot masks (no per-edge xi matmuls) and gathers
xj rows of x1. conv3 is fully table-based: z_e = A3[dst] + B3[src] with
A3/B3 = x2 @ WA3 / x2 @ WB3 computed during conv2's scatter; the gather then
yields z directly in [edge, channel] layout, GraphNorm stats come from a
Gram-matrix matmul (sum + sum-of-squares in one accumulation), and the scatter
consumes [edge, channel] tiles with zero transposes. One-hot masks for
scatter/expansion/pool are host-precomputed bf16 tables. GraphNorm stats are
global AllReduces; mean-aggregation is a collision-free one-hot matmul scatter
into PSUM per window.
"""
import numpy as np
import ml_dtypes

import concourse.bass as bass
import concourse.tile as tile
from concourse import bacc, mybir
from concourse.bass_utils import run_bass_kernel_spmd

BF16 = mybir.dt.bfloat16
F32 = mybir.dt.float32
AOP = mybir.AluOpType
AFT = mybir.ActivationFunctionType
AX = mybir.AxisListType

N_NODES = 100000
N_EDGES = 400000
N_GRAPHS = 1000
NC = 8
WIN = 128
NWIN = 98
SPC = WIN * NWIN          # 12544
NSLOTS = SPC * NC         # 100352
QUAD = NSLOTS // 4        # 25088
B = 4                     # chunks per window
EPW = B * WIN             # 512
E_PAD = NWIN * EPW        # 50176
EPS = 1e-5

NW_BLK = 7
BLK = NW_BLK * EPW        # 3584
NBLK = NWIN // NW_BLK     # 14
NCHUNK = BLK // 128       # 28
NSEG = BLK // 512         # 7
NCH_ALL = E_PAD // 128    # 392
ZW = 257                  # z3 chunk width (256 channels + ones column)


_cache = {}


# ============================ host-side packing ============================

def _pack(edge_index, batch):
    src = np.asarray(edge_index[0], dtype=np.int64)
    dst = np.asarray(edge_index[1], dtype=np.int64)
    batch = np.asarray(batch, dtype=np.int64)
    cnt = np.bincount(dst, minlength=N_NODES)

    nvirt = NSLOTS - N_NODES
    cnt_all = np.concatenate([cnt, np.zeros(nvirt, dtype=cnt.dtype)])
    order = np.argsort(-cnt_all, kind="stable")
    GW = NWIN * NC
    rounds = NSLOTS // GW
    win_of_rank = np.empty(NSLOTS, dtype=np.int64)
    for r in range(rounds):
        seg = np.arange(GW) if r % 2 == 0 else np.arange(GW - 1, -1, -1)
        win_of_rank[r * GW:(r + 1) * GW] = seg
    win_of_node = np.empty(NSLOTS, dtype=np.int64)
    win_of_node[order] = win_of_rank
    wsum = np.bincount(win_of_node, weights=cnt_all.astype(np.float64),
                       minlength=GW).astype(np.int64)

    cap = EPW
    members_of = [list(np.where(win_of_node == w)[0]) for w in range(GW)]
    for _ in range(2000):
        over = np.where(wsum > cap)[0]
        if len(over) == 0:
            break
        w = int(over[0])
        # smallest-count >0 node in w
        mem = members_of[w]
        cs = [(int(cnt_all[n]), n) for n in mem if cnt_all[n] > 0]
        cs.sort()
        moved = False
        for c1, n in cs:
            # find target window with a smaller-count node to swap
            worder2 = np.argsort(wsum)
            for tw in worder2[:64]:
                tw = int(tw)
                if tw == w:
                    continue
                tmem = members_of[tw]
                best = None
                for m in tmem:
                    c2 = int(cnt_all[m])
                    if c2 < c1 and wsum[tw] + c1 - c2 <= cap:
                        if best is None or c2 < best[0]:
                            best = (c2, m)
                        if c2 == 0:
                            break
                if best is not None:
                    c2, m = best
                    members_of[tw].remove(m)
                    members_of[tw].append(n)
                    members_of[w].remove(n)
                    members_of[w].append(m)
                    win_of_node[n] = tw
                    win_of_node[m] = w
                    wsum[tw] += c1 - c2
                    wsum[w] -= c1 - c2
                    moved = True
                    break
            if moved:
                break
        if not moved:
            raise RuntimeError("packing fixup stuck")
    assert wsum.max() <= cap, f"window packing failed: max={wsum.max()}"

    worder = np.argsort(-wsum, kind="stable")
    core_load = np.zeros(NC, dtype=np.int64)
    core_nwin = np.zeros(NC, dtype=np.int64)
    core_of_win = np.empty(GW, dtype=np.int64)
    for w in worder:
        cands = np.where(core_nwin < NWIN)[0]
        c = cands[np.argmin(core_load[cands])]
        core_of_win[w] = c
        core_load[c] += wsum[w]
        core_nwin[c] += 1

    win_lists = [[] for _ in range(NC)]
    for w in range(GW):
        win_lists[core_of_win[w]].append(w)
    for c in range(NC):
        wl = win_lists[c]
        j = int(np.argmin(wsum[wl]))
        assert wsum[wl[j]] < cap, "no sentinel room"
        wl[j], wl[-1] = wl[-1], wl[j]

    slot_of_node = np.empty(NSLOTS, dtype=np.int64)
    for c in range(NC):
        for wi, w in enumerate(win_lists[c]):
            mem = np.sort(np.array(members_of[w], dtype=np.int64))
            assert len(mem) == WIN
            slot_of_node[mem] = c * SPC + wi * WIN + np.arange(WIN)
    node_of_slot = np.empty(NSLOTS, dtype=np.int64)
    node_of_slot[slot_of_node] = np.arange(NSLOTS)
    cnt_of_slot = cnt_all[node_of_slot]

    qzero = []
    for q in range(4):
        z = np.where(cnt_of_slot[q * QUAD:(q + 1) * QUAD] == 0)[0]
        assert len(z) > 0
        qzero.append(int(z[0]))  # local to quadrant
    czero = []
    for c in range(NC):
        z = np.where(cnt_of_slot[c * SPC:(c + 1) * SPC] == 0)[0]
        assert len(z) > 0
        czero.append(int(z[0]))  # local to core

    dslot = slot_of_node[dst]
    sslot = slot_of_node[src]
    ecore = dslot // SPC
    ewin = (dslot % SPC) // WIN
    key = ecore * (NWIN * WIN) + ewin * WIN + (dslot % WIN)
    eorder = np.argsort(key, kind="stable")
    dsl, ssl = dslot[eorder], sslot[eorder]
    ec, ew = ecore[eorder], ewin[eorder]

    cw = ec * NWIN + ew
    cw_cnt = np.bincount(cw, minlength=NC * NWIN)
    assert cw_cnt.max() <= EPW

    xi_idx = np.zeros((NC, E_PAD), dtype=np.int64)
    xj_idx = np.zeros((NC, E_PAD), dtype=np.int64)
    dstwin = np.full((NC, E_PAD), -1.0, dtype=np.float32)
    valid = np.zeros((NC, E_PAD), dtype=bool)

    ofs = (np.arange(NC * NWIN) % NWIN) * EPW
    start = np.concatenate([[0], np.cumsum(cw_cnt)[:-1]])
    within = np.arange(N_EDGES) - start[cw]
    pos = ofs[cw] + within
    xi_idx[ec, pos] = dsl % SPC
    xj_idx[ec, pos] = ssl
    dstwin[ec, pos] = (dsl % WIN).astype(np.float32)
    valid[ec, pos] = True
    for c in range(NC):
        xi_idx[c, ~valid[c]] = czero[c]
    pad_cnt = (~valid).sum(axis=1).astype(np.float32)
    assert np.all(~valid[:, -1]), "sentinel column must be padding"

    gzero = qzero[0]  # global slot with zero row
    xj_glob = np.where(valid, xj_idx, gzero).astype(np.int32)

    inv_cnt = (1.0 / np.maximum(cnt_of_slot.reshape(NC, SPC), 1.0)).astype(np.float32)

    g_of_slot = np.full(NSLOTS, -1, dtype=np.int64)
    real = node_of_slot < N_NODES
    g_of_slot[real] = batch[node_of_slot[real]]
    NGW = 8
    Bg = 0
    pools = [[None] * NGW for _ in range(NC)]
    for c in range(NC):
        gl = g_of_slot[c * SPC:(c + 1) * SPC]
        for gw in range(NGW):
            m = np.where((gl >= gw * 128) & (gl < (gw + 1) * 128))[0]
            pools[c][gw] = m
            Bg = max(Bg, (len(m) + 127) // 128)
    NPG = Bg * 128
    pool_idx = np.zeros((NC, NGW, NPG), dtype=np.int16)
    pool_gwl = np.full((NC, NGW, NPG), -1.0, dtype=np.float32)
    for c in range(NC):
        for gw in range(NGW):
            m = pools[c][gw]
            pool_idx[c, gw, :len(m)] = m.astype(np.int16)
            pool_idx[c, gw, len(m):] = czero[c]
            pool_gwl[c, gw, :len(m)] = (g_of_slot[c * SPC + m] - gw * 128).astype(np.float32)

    gcnt = np.bincount(batch, minlength=N_GRAPHS).astype(np.float32)
    inv_g = np.zeros(1024, dtype=np.float32)
    inv_g[:N_GRAPHS] = 1.0 / np.maximum(gcnt, 1.0)

    return dict(slot_of_node=slot_of_node, node_of_slot=node_of_slot,
                xj_glob=xj_glob, dstwin=dstwin, pad_cnt=pad_cnt,
                inv_cnt=inv_cnt, valid=valid, eorder=eorder, ec=ec, pos=pos,
                pool_idx=pool_idx, pool_gwl=pool_gwl, inv_g=inv_g, Bg=Bg,
                qzero=qzero)


def _wrap_idx(a):
    """[.., n] int -> [.., 128, n//16]: element i -> partition i%16 col i//16,
    replicated to 8 groups of 16 partitions."""
    n = a.shape[-1]
    assert n % 16 == 0
    w = a.reshape(*a.shape[:-1], n // 16, 16)
    w = np.swapaxes(w, -1, -2)
    w = np.broadcast_to(w[..., None, :, :], (*a.shape[:-1], 8, 16, n // 16))
    return np.ascontiguousarray(w).reshape(*a.shape[:-1], 128, n // 16).astype(np.int16)


# AllGather chunking: windows split into 4 groups; gathered tables are laid
# out chunk-major [chunk][core][local slot] so each chunk can AllGather as soon
# as its windows are written.
WCH = [25, 25, 24, 24]
CUMW = [0, 25, 50, 74]
CSL = [w * WIN for w in WCH]            # slots per chunk per core
CUMS = [c * WIN for c in CUMW]          # slot offset of chunk within core
GOFF = [NC * c for c in CUMS]           # global row offset of chunk


def _remap_slot(g):
    g = np.asarray(g, dtype=np.int64)
    c = g // SPC
    s = g % SPC
    w = s // WIN
    k = np.digitize(w, CUMW) - 1
    cs = np.asarray(CSL, dtype=np.int64)[k]
    cum = np.asarray(CUMS, dtype=np.int64)[k]
    return np.asarray(GOFF, dtype=np.int64)[k] + c * cs + (s - cum)


def _bf(x):
    return np.ascontiguousarray(np.asarray(x, dtype=np.float32)).astype(ml_dtypes.bfloat16)


def _tile_w(w):
    K, M = w.shape
    nk, nm = (K + 127) // 128, (M + 127) // 128
    out = np.zeros((nk, nm, 128, 128), dtype=ml_dtypes.bfloat16)
    for i in range(nk):
        for j in range(nm):
            blk = np.asarray(w, dtype=np.float32)[i * 128:(i + 1) * 128, j * 128:(j + 1) * 128]
            out[i, j, :blk.shape[0], :blk.shape[1]] = _bf(blk)
    return out


# ============================ device kernel ============================

EHALF = E_PAD // 2        # 25088
NSEG_H = EHALF // 512     # 49


def _build(Bg, debug=False, phases=4):
    nc = bacc.Bacc("TRN2", target_bir_lowering=False, debug=False, num_devices=NC)

    def din(name, shape, dt):
        return nc.dram_tensor(name, shape, dt, kind="ExternalInput").ap()

    t_msgT = din("msgT", [48, EHALF], BF16)
    t_xj = din("xj_idx", [128, NCH_ALL], mybir.dt.int32)
    t_pidx = din("pool_idx", [8, 128, Bg], mybir.dt.int32)
    t_oh = din("ohtab", [128, E_PAD], BF16)
    t_ohT = din("ohTtab", [128, E_PAD], BF16)
    t_invcnt = din("invcnt", [128, NWIN], F32)
    t_padcnt = din("padcnt", [128, 1], F32)
    t_ident = din("ident", [128, 128], BF16)
    t_eyef = din("eyef", [128, 128], F32)
    t_c1w = din("c1w", [3, 128, 128], BF16)
    t_c1b = din("c1b", [3, 128, 1], F32)
    t_c1gn = din("c1gn", [3, 3, 128, 1], F32)
    t_wa2 = din("wa2", [128, 256], BF16)
    t_c2wb = din("c2wb", [2, 128, 128], BF16)
    t_c2w2 = din("c2w2", [2, 2, 128, 128], BF16)
    t_c2b = din("c2b", [2, 2, 128, 1], F32)
    t_c2gn = din("c2gn", [2, 3, 2, 128, 1], F32)
    t_wa3 = din("wa3", [2, 128, 256], BF16)
    t_wb3 = din("wb3", [2, 128, 256], BF16)
    t_c3b = din("c3b", [2, 128, 1], F32)
    t_c3gn = din("c3gn", [3, 2, 128, 1], F32)
    t_lw1 = din("lw1", [2, 2, 128, 128], BF16)
    t_lb1 = din("lb1", [2, 128, 1], F32)
    t_lw2 = din("lw2", [2, 128, 2], BF16)
    t_lb2 = din("lb2", [2, 1], F32)
    t_poh = din("poolohtab", [128, 8 * Bg * 128], BF16)
    t_invg = din("invg", [128, 8], F32)

    o_out = nc.dram_tensor("out", [2, N_GRAPHS], F32, kind="ExternalOutput").ap()
    dbg = {}
    if debug:
        dbg["x1"] = nc.dram_tensor("dbg_x1", [NSLOTS, 128], BF16, kind="ExternalOutput").ap()
        dbg["x2"] = nc.dram_tensor("dbg_x2", [SPC, 256], BF16, kind="ExternalOutput").ap()
        dbg["x3"] = nc.dram_tensor("dbg_x3", [SPC, 256], BF16, kind="ExternalOutput").ap()
        dbg["pool"] = nc.dram_tensor("dbg_pool", [1024, 256], F32, kind="ExternalOutput").ap()

    with tile.TileContext(nc) as tc:
        with tc.tile_pool(name="dram", bufs=1, space="DRAM") as dram, \
             tc.tile_pool(name="cp", bufs=1) as cp:
            z_scr = [dram.tile([2, 128, E_PAD], BF16, tag=f"zscr{i}", name=f"zscr{i}") for i in range(2)]
            z3_scr = dram.tile([128, NCH_ALL * ZW], BF16)
            tab1_loc = dram.tile([SPC, 128], BF16)
            tab1 = dram.tile([NSLOTS, 128], BF16)
            a2_loc = dram.tile([SPC, 256], BF16)
            a3_loc = dram.tile([SPC, 256], BF16)
            b3_loc = dram.tile([SPC, 256], BF16)
            b3_full = dram.tile([NSLOTS, 256], BF16)
            tab3_loc = dram.tile([SPC, 256], BF16)
            st_in = dram.tile([128, 8], F32)
            st_out = dram.tile([128, 8], F32)
            pool_in = dram.tile([1024, 256], F32)
            pool_out = dram.tile([1024, 256], F32)

            ident = cp.tile([128, 128], BF16)
            nc.sync.dma_start(ident[:], t_ident[:])
            eyef = cp.tile([128, 128], F32)
            nc.sync.dma_start(eyef[:], t_eyef[:])
            invcnt = cp.tile([128, NWIN], F32)
            nc.sync.dma_start(invcnt[:], t_invcnt[:])
            padcnt = cp.tile([128, 1], F32)
            nc.sync.dma_start(padcnt[:], t_padcnt[:])

            # ---------- helpers ----------
            def allreduce_stats(s_acc, q_acc, n_mb, sb):
                st = sb.tile([128, 8], F32, tag="st_")
                nc.vector.memset(st[:], 0.0)
                nc.vector.tensor_copy(st[:, 0:n_mb], s_acc[:])
                nc.vector.tensor_copy(st[:, 4:4 + n_mb], q_acc[:])
                nc.sync.dma_start(st_in[:], st[:])
                nc.gpsimd.collective_compute(
                    "AllReduce", AOP.add, replica_groups=[list(range(NC))],
                    ins=[st_in.opt()], outs=[st_out.opt()])
                stg = sb.tile([128, 8], F32, tag="stg_")
                nc.sync.dma_start(stg[:], st_out[:])
                return stg

            def affine_from_stats(stg, n_mb, b_lin, gn, sb):
                A, Cc = [], []
                for mb in range(n_mb):
                    s = stg[:, mb:mb + 1]
                    q = stg[:, 4 + mb:5 + mb]
                    g, bgn, ms = gn[0][mb], gn[1][mb], gn[2][mb]
                    bl = b_lin[mb]
                    m = sb.tile([128, 1], F32, tag="af_m")
                    nc.vector.tensor_scalar(m[:], s, 1.0 / N_EDGES, None, AOP.mult)
                    nc.vector.tensor_tensor(m[:], m[:], bl, op=AOP.add)
                    e2 = sb.tile([128, 1], F32, tag="af_e2")
                    nc.vector.tensor_scalar(e2[:], q, 1.0 / N_EDGES, None, AOP.mult)
                    tmp = sb.tile([128, 1], F32, tag="af_t")
                    nc.vector.tensor_tensor(tmp[:], m[:], bl, op=AOP.mult)
                    nc.vector.tensor_scalar(tmp[:], tmp[:], 2.0, None, AOP.mult)
                    nc.vector.tensor_tensor(e2[:], e2[:], tmp[:], op=AOP.add)
                    nc.vector.tensor_tensor(tmp[:], bl, bl, op=AOP.mult)
                    nc.vector.tensor_tensor(e2[:], e2[:], tmp[:], op=AOP.subtract)
                    msm = sb.tile([128, 1], F32, tag="af_msm")
                    nc.vector.tensor_tensor(msm[:], ms, m[:], op=AOP.mult)
                    var = sb.tile([128, 1], F32, tag="af_v")
                    nc.vector.tensor_tensor(var[:], msm[:], msm[:], op=AOP.mult)
                    nc.vector.tensor_tensor(tmp[:], msm[:], m[:], op=AOP.mult)
                    nc.vector.tensor_scalar(tmp[:], tmp[:], 2.0, None, AOP.mult)
                    nc.vector.tensor_tensor(var[:], var[:], tmp[:], op=AOP.subtract)
                    nc.vector.tensor_tensor(var[:], var[:], e2[:], op=AOP.add)
                    a = sb.tile([128, 1], F32, tag="af_a")
                    nc.vector.tensor_scalar(var[:], var[:], EPS, None, AOP.add)
                    nc.scalar.activation(a[:], var[:], AFT.Sqrt)
                    nc.vector.reciprocal(a[:], a[:])
                    nc.vector.tensor_tensor(a[:], a[:], g, op=AOP.mult)
                    cc = sb.tile([128, 1], F32, tag="af_c")
                    nc.vector.tensor_tensor(cc[:], bl, msm[:], op=AOP.subtract)
                    nc.vector.tensor_tensor(cc[:], cc[:], a[:], op=AOP.mult)
                    nc.vector.tensor_tensor(cc[:], cc[:], bgn, op=AOP.add)
                    A.append(a)
                    Cc.append(cc)
                return A, Cc

            sqscr = cp.tile([128, BLK], BF16)

            def acc_stats(ps_ap, s_col, q_col, sb, n=512, sq_scalar=False):
                t1 = sb.tile([128, 1], F32, tag="rs_t1")
                nc.vector.reduce_sum(out=t1[:], in_=ps_ap, axis=AX.X)
                nc.vector.tensor_tensor(s_col, s_col, t1[:], op=AOP.add)
                qa = sb.tile([128, 1], F32, tag="rs_qa")
                nc.scalar.activation(sqscr[:, :n], ps_ap, AFT.Square,
                                     accum_out=qa[:])
                nc.vector.tensor_tensor(q_col, q_col, qa[:], op=AOP.add)


            def sentinel_correct(s_acc, q_acc, zsent_cols, n_mb, sb):
                for mb in range(n_mb):
                    zs = zsent_cols[mb]
                    t1 = sb.tile([128, 1], F32, tag="sc_t1")
                    nc.vector.tensor_tensor(t1[:], zs, padcnt[:], op=AOP.mult)
                    nc.vector.tensor_tensor(s_acc[:, mb:mb + 1], s_acc[:, mb:mb + 1],
                                            t1[:], op=AOP.subtract)
                    nc.vector.tensor_tensor(t1[:], zs, zs, op=AOP.mult)
                    nc.vector.tensor_tensor(t1[:], t1[:], padcnt[:], op=AOP.mult)
                    nc.vector.tensor_tensor(q_acc[:, mb:mb + 1], q_acc[:, mb:mb + 1],
                                            t1[:], op=AOP.subtract)

            def load_vec(t_ap, sb, tag):
                v = sb.tile([128, 1], F32, tag=tag)
                nc.sync.dma_start(v[:], t_ap)
                return v[:]

            # ======================= CONV 1 =======================
            with tc.tile_pool(name="c1sb", bufs=2) as sb, \
                 tc.tile_pool(name="c1zb", bufs=1) as zbp:
                c1b = [[load_vec(t_c1b[i], sb, f"c1b{i}")] for i in range(3)]
                c1gn = [[[load_vec(t_c1gn[i, j], sb, f"c1gn{i}{j}")] for j in range(3)]
                        for i in range(3)]
                zbuf = zbp.tile([128, E_PAD], BF16)
                with tc.tile_pool(name="c1big", bufs=2) as bp, \
                     tc.tile_pool(name="c1ps", bufs=2, space="PSUM") as ps, \
                     tc.tile_pool(name="msgp", bufs=1) as msgp:
                    c1w = []
                    for i in range(3):
                        w = sb.tile([128, 128], BF16, tag=f"c1w{i}")
                        nc.sync.dma_start(w[:], t_c1w[i])
                        c1w.append(w)
                    msgT = msgp.tile([48, EHALF], BF16)
                    nc.sync.dma_start(msgT[:], t_msgT[:])

                    def z1_psum(h, s):
                        zp = ps.tile([128, 512], F32, tag="zp")
                        nc.tensor.matmul(zp[:], c1w[0][32 * h:32 * h + 10, :],
                                         msgT[32 * h:32 * h + 10, s * 512:(s + 1) * 512],
                                         start=True, stop=True)
                        return zp

                    s1 = sb.tile([128, 1], F32, tag="s1")
                    q1 = sb.tile([128, 1], F32, tag="q1")
                    nc.vector.memset(s1[:], 0.0)
                    nc.vector.memset(q1[:], 0.0)
                    for h in range(2):
                        for s in range(NSEG_H):
                            zp = z1_psum(h, s)
                            acc_stats(zp[:], s1[:, 0:1], q1[:, 0:1], sb,
                                      sq_scalar=True)
                    stg = allreduce_stats(s1, q1, 1, sb)
                    A1, C1 = affine_from_stats(stg, 1, c1b[0], c1gn[0], sb)

                    # L2: recompute z1, relu, z2 = W2 @ h1 -> zbuf (SBUF), batched stats
                    s2 = sb.tile([128, 1], F32, tag="s2")
                    q2 = sb.tile([128, 1], F32, tag="q2")
                    nc.vector.memset(s2[:], 0.0)
                    nc.vector.memset(q2[:], 0.0)
                    for h in range(2):
                        for b in range(NSEG_H // 7):
                            h1 = bp.tile([128, BLK], BF16, tag="h1")
                            for s in range(7):
                                zp = z1_psum(h, b * 7 + s)
                                nc.scalar.activation(h1[:, s * 512:(s + 1) * 512], zp[:],
                                                     AFT.Relu, bias=C1[0], scale=A1[0])
                            col0 = h * EHALF + b * BLK
                            for s in range(7):
                                zp = ps.tile([128, 512], F32, tag="zp")
                                nc.tensor.matmul(zp[:], c1w[1][:],
                                                 h1[:, s * 512:(s + 1) * 512],
                                                 start=True, stop=True)
                                nc.scalar.copy(
                                    zbuf[:, col0 + s * 512:col0 + (s + 1) * 512], zp[:])
                            acc_stats(zbuf[:, col0:col0 + BLK], s2[:, 0:1], q2[:, 0:1],
                                      sb, n=BLK)
                    zs2 = sb.tile([128, 1], F32, tag="zs2")
                    nc.vector.tensor_copy(zs2[:], zbuf[:, E_PAD - 1:E_PAD])
                    sentinel_correct(s2, q2, [zs2[:]], 1, sb)
                    stg2 = allreduce_stats(s2, q2, 1, sb)
                    A2, C2 = affine_from_stats(stg2, 1, c1b[1], c1gn[1], sb)

                    # L3: h2 = relu(aff(z2)), z3 = W3 @ h2 -> zbuf in place
                    s3 = sb.tile([128, 1], F32, tag="s3")
                    q3 = sb.tile([128, 1], F32, tag="q3")
                    nc.vector.memset(s3[:], 0.0)
                    nc.vector.memset(q3[:], 0.0)
                    for b in range(NBLK):
                        h2 = bp.tile([128, BLK], BF16, tag="h2")
                        nc.scalar.activation(h2[:], zbuf[:, b * BLK:(b + 1) * BLK],
                                             AFT.Relu, bias=C2[0], scale=A2[0])
                        for s in range(7):
                            zp = ps.tile([128, 512], F32, tag="zp")
                            nc.tensor.matmul(zp[:], c1w[2][:],
                                             h2[:, s * 512:(s + 1) * 512],
                                             start=True, stop=True)
                            nc.scalar.copy(
                                zbuf[:, b * BLK + s * 512:b * BLK + (s + 1) * 512], zp[:])
                        acc_stats(zbuf[:, b * BLK:(b + 1) * BLK], s3[:, 0:1], q3[:, 0:1],
                                  sb, n=BLK)
                    zs3 = sb.tile([128, 1], F32, tag="zs3")
                    nc.vector.tensor_copy(zs3[:], zbuf[:, E_PAD - 1:E_PAD])
                    sentinel_correct(s3, q3, [zs3[:]], 1, sb)
                    stg3 = allreduce_stats(s3, q3, 1, sb)
                    A3, C3 = affine_from_stats(stg3, 1, c1b[2], c1gn[2], sb)

                # scatter: h3 = relu(aff(z3)); transpose; one-hot matmul; + A2 table
                with tc.tile_pool(name="s1sb", bufs=2) as sp, \
                     tc.tile_pool(name="s1oh", bufs=2) as ohp, \
                     tc.tile_pool(name="s1tp", bufs=2, space="PSUM") as ps_tp, \
                     tc.tile_pool(name="s1sc", bufs=2, space="PSUM") as ps_sc:
                    wa2 = sp.tile([128, 256], BF16, tag="wa2")
                    nc.sync.dma_start(wa2[:], t_wa2[:])
                    for b in range(NBLK):
                        h3 = sp.tile([128, BLK], BF16, tag="h3")
                        nc.scalar.activation(h3[:], zbuf[:, b * BLK:(b + 1) * BLK],
                                             AFT.Relu, bias=C3[0], scale=A3[0])
                        ohb = ohp.tile([128, BLK], BF16, tag="ohb")
                        nc.sync.dma_start(ohb[:], t_oh[:, b * BLK:(b + 1) * BLK])
                        for w in range(NW_BLK):
                            gw = b * NW_BLK + w
                            tpp = ps_tp.tile([128, 512], BF16, tag="tpp", space="PSUM")
                            for cb in range(B):
                                nc.tensor.transpose(
                                    tpp[:, cb * 128:(cb + 1) * 128],
                                    h3[:, (w * B + cb) * 128:(w * B + cb + 1) * 128],
                                    ident[:])
                            hE = sp.tile([128, 512], BF16, tag="hE")
                            nc.vector.tensor_copy(hE[:], tpp[:])
                            sc = ps_sc.tile([128, 128], F32, tag="sc", space="PSUM")
                            for cb in range(B):
                                nc.tensor.matmul(
                                    sc[:], ohb[:, (w * B + cb) * 128:(w * B + cb + 1) * 128],
                                    hE[:, cb * 128:(cb + 1) * 128],
                                    start=(cb == 0), stop=(cb == B - 1))
                            nt = sp.tile([128, 128], BF16, tag="nt")
                            nc.vector.tensor_scalar(nt[:], sc[:], invcnt[:, gw:gw + 1],
                                                    None, AOP.mult)
                            nc.sync.dma_start(tab1_loc[gw * WIN:(gw + 1) * WIN, :], nt[:])
                            # A2 table: ntT then (x1_win) @ WA2
                            ntp = ps_tp.tile([128, 128], BF16, tag="ntp", space="PSUM")
                            nc.tensor.transpose(ntp[:], nt[:], ident[:])
                            ntT = sp.tile([128, 128], BF16, tag="ntT")
                            nc.vector.tensor_copy(ntT[:], ntp[:])
                            a2p = ps_sc.tile([128, 256], F32, tag="a2p", space="PSUM")
                            nc.tensor.matmul(a2p[:], ntT[:], wa2[:], start=True, stop=True)
                            a2t = sp.tile([128, 256], BF16, tag="a2t")
                            nc.vector.tensor_copy(a2t[:], a2p[:])
                            nc.sync.dma_start(a2_loc[gw * WIN:(gw + 1) * WIN, :], a2t[:])

            for k in range(4):
                nc.gpsimd.collective_compute(
                    "AllGather", AOP.bypass, replica_groups=[list(range(NC))],
                    ins=[tab1_loc[CUMS[k]:CUMS[k] + CSL[k], :].opt()],
                    outs=[tab1[GOFF[k]:GOFF[k] + NC * CSL[k], :].opt()])
            if debug:
                nc.sync.dma_start(dbg["x1"][:], tab1[:])

            # ======================= CONV 2 =======================
            if phases >= 2:
              with tc.tile_pool(name="c2sb", bufs=2) as sb:
                  c2b = [[load_vec(t_c2b[i, mb], sb, f"c2b{i}{mb}") for mb in range(2)]
                         for i in range(2)]
                  c2gn = [[[load_vec(t_c2gn[i, j, mb], sb, f"c2gn{i}{j}{mb}")
                            for mb in range(2)] for j in range(3)] for i in range(2)]
                  # ---- pass 1: z1 = A2[dst] (one-hot expand) + WB2 @ x1[src] ----
                  sA = sb.tile([128, 2], F32, tag="c2sA")
                  qA = sb.tile([128, 2], F32, tag="c2qA")
                  nc.vector.memset(sA[:], 0.0)
                  nc.vector.memset(qA[:], 0.0)
                  with tc.tile_pool(name="g2g", bufs=2) as g2, \
                       tc.tile_pool(name="g2q", bufs=2) as gqp, \
                       tc.tile_pool(name="g2z", bufs=2) as zwp, \
                       tc.tile_pool(name="g2ps", bufs=2, space="PSUM") as ps, \
                       tc.tile_pool(name="g2tp", bufs=2, space="PSUM") as ps_tp:
                      wbs = []
                      for mo in range(2):
                          wtb = sb.tile([128, 128], BF16, tag=f"c2wb{mo}")
                          nc.sync.dma_start(wtb[:], t_c2wb[mo])
                          wbs.append(wtb)
                      for b in range(NBLK):
                          ixj = g2.tile([128, NCHUNK], mybir.dt.int32, tag="ixj")
                          nc.sync.dma_start(ixj[:], t_xj[:, b * NCHUNK:(b + 1) * NCHUNK])
                          gxj = gqp.tile([128, NCHUNK * 128], BF16, tag="gxj")
                          for ch in range(NCHUNK):
                              nc.gpsimd.indirect_dma_start(
                                  out=gxj[:, ch * 128:(ch + 1) * 128],
                                  out_offset=None,
                                  in_=tab1[:],
                                  in_offset=bass.IndirectOffsetOnAxis(
                                      ap=ixj[:, ch:ch + 1], axis=0))
                          ohTb = g2.tile([128, BLK], BF16, tag="ohTb")
                          nc.sync.dma_start(ohTb[:], t_ohT[:, b * BLK:(b + 1) * BLK])
                          zsb = [zwp.tile([128, BLK], BF16, tag=f"zsb{h}", name=f"zsb{h}")
                                 for h in range(2)]
                          for w in range(NW_BLK):
                              gw = b * NW_BLK + w
                              a2w = g2.tile([128, 256], BF16, tag="a2w")
                              nc.sync.dma_start(a2w[:], a2_loc[gw * WIN:(gw + 1) * WIN, :])
                              tpp = ps_tp.tile([128, 512], BF16, tag="xtp", space="PSUM")
                              for cb in range(B):
                                  nc.tensor.transpose(
                                      tpp[:, cb * 128:(cb + 1) * 128],
                                      gxj[:, (w * B + cb) * 128:(w * B + cb + 1) * 128],
                                      ident[:])
                              xjT = g2.tile([128, 512], BF16, tag="xjT")
                              nc.vector.tensor_copy(xjT[:], tpp[:])
                              for h in range(2):
                                  zp = ps.tile([128, 512], F32, tag="zp")
                                  nc.tensor.matmul(zp[:], a2w[:, h * 128:(h + 1) * 128],
                                                   ohTb[:, w * 512:(w + 1) * 512],
                                                   start=True, stop=False)
                                  nc.tensor.matmul(zp[:], wbs[h][:], xjT[:],
                                                   start=False, stop=True)
                                  nc.scalar.copy(zsb[h][:, w * 512:(w + 1) * 512], zp[:])
                          for h in range(2):
                              acc_stats(zsb[h][:], sA[:, h:h + 1], qA[:, h:h + 1],
                                        sb, n=BLK)
                              nc.sync.dma_start(z_scr[0][h, :, b * BLK:(b + 1) * BLK],
                                                zsb[h][:])
                  stg = allreduce_stats(sA, qA, 2, sb)
                  A1, C1 = affine_from_stats(stg, 2, c2b[0], c2gn[0], sb)

                  # ---- layer 2 ----
                  s2 = sb.tile([128, 2], F32, tag="c2s2")
                  q2 = sb.tile([128, 2], F32, tag="c2q2")
                  nc.vector.memset(s2[:], 0.0)
                  nc.vector.memset(q2[:], 0.0)
                  zlast = [None, None]
                  with tc.tile_pool(name="c2mid", bufs=2) as mp, \
                       tc.tile_pool(name="c2ps", bufs=2, space="PSUM") as ps:
                      w2s = []
                      for ki in range(2):
                          for mo in range(2):
                              w = sb.tile([128, 128], BF16, tag=f"c2w2{ki}{mo}")
                              nc.sync.dma_start(w[:], t_c2w2[ki, mo])
                              w2s.append(w)
                      for b in range(NBLK):
                          h1 = []
                          for mb in range(2):
                              z = mp.tile([128, BLK], BF16, tag=f"c2z1r{mb}")
                              nc.sync.dma_start(z[:], z_scr[0][mb, :, b * BLK:(b + 1) * BLK])
                              hh = mp.tile([128, BLK], BF16, tag=f"c2h1{mb}")
                              nc.vector.tensor_scalar(hh[:], z[:], A1[mb], C1[mb],
                                                      AOP.mult, AOP.add)
                              nc.vector.tensor_scalar(hh[:], hh[:], 0.0, None, AOP.max)
                              h1.append(hh)
                          for mo in range(2):
                              zw = mp.tile([128, BLK], BF16, tag=f"c2z2w{mo}")
                              for s in range(NSEG):
                                  zp = ps.tile([128, 512], F32, tag="c2zp")
                                  for ki in range(2):
                                      nc.tensor.matmul(zp[:], w2s[ki * 2 + mo][:],
                                                       h1[ki][:, s * 512:(s + 1) * 512],
                                                       start=(ki == 0), stop=(ki == 1))
                                  nc.scalar.copy(zw[:, s * 512:(s + 1) * 512], zp[:])
                              acc_stats(zw[:], s2[:, mo:mo + 1], q2[:, mo:mo + 1],
                                        sb, n=BLK)
                              nc.sync.dma_start(z_scr[1][mo, :, b * BLK:(b + 1) * BLK], zw[:])
                              zlast[mo] = zw
                      zsent = []
                      for mo in range(2):
                          zc = sb.tile([128, 1], F32, tag=f"c2zs{mo}")
                          nc.vector.tensor_copy(zc[:], zlast[mo][:, BLK - 1:BLK])
                          zsent.append(zc[:])
                  sentinel_correct(s2, q2, zsent, 2, sb)
                  stg2 = allreduce_stats(s2, q2, 2, sb)
                  A2, C2 = affine_from_stats(stg2, 2, c2b[1], c2gn[1], sb)

                  # ---- scatter + A3/B3 tables ----
                  with tc.tile_pool(name="s2sb", bufs=2) as sp, \
                       tc.tile_pool(name="s2oh", bufs=2) as ohp, \
                       tc.tile_pool(name="s2tp", bufs=2, space="PSUM") as ps_tp, \
                       tc.tile_pool(name="s2sc", bufs=2, space="PSUM") as ps_sc:
                      wab3 = sp.tile([128, 1024], BF16, tag="wab3")
                      for ki in range(2):
                          nc.sync.dma_start(wab3[:, ki * 512:ki * 512 + 256], t_wa3[ki])
                          nc.sync.dma_start(wab3[:, ki * 512 + 256:ki * 512 + 512],
                                            t_wb3[ki])
                      for b in range(NBLK):
                          hs = []
                          for mb in range(2):
                              z = sp.tile([128, BLK], BF16, tag=f"s2z{mb}")
                              nc.sync.dma_start(z[:], z_scr[1][mb, :, b * BLK:(b + 1) * BLK])
                              h = sp.tile([128, BLK], BF16, tag=f"s2h{mb}")
                              nc.scalar.activation(h[:], z[:], AFT.Relu,
                                                   bias=C2[mb], scale=A2[mb])
                              hs.append(h)
                          ohb = ohp.tile([128, BLK], BF16, tag="ohb2")
                          nc.sync.dma_start(ohb[:], t_oh[:, b * BLK:(b + 1) * BLK])
                          for w in range(NW_BLK):
                              gw = b * NW_BLK + w
                              tpp = ps_tp.tile([128, 1024], BF16, tag="tpp2", space="PSUM")
                              for cb in range(B):
                                  for mb in range(2):
                                      nc.tensor.transpose(
                                          tpp[:, (cb * 2 + mb) * 128:(cb * 2 + mb + 1) * 128],
                                          hs[mb][:, (w * B + cb) * 128:(w * B + cb + 1) * 128],
                                          ident[:])
                              hE = sp.tile([128, 1024], BF16, tag="hE2")
                              nc.vector.tensor_copy(hE[:], tpp[:])
                              sc = ps_sc.tile([128, 256], F32, tag="sc2", space="PSUM")
                              for cb in range(B):
                                  nc.tensor.matmul(
                                      sc[:], ohb[:, (w * B + cb) * 128:(w * B + cb + 1) * 128],
                                      hE[:, cb * 256:(cb + 1) * 256],
                                      start=(cb == 0), stop=(cb == B - 1))
                              nt = sp.tile([128, 256], BF16, tag="nt2")
                              nc.vector.tensor_scalar(nt[:], sc[:], invcnt[:, gw:gw + 1],
                                                      None, AOP.mult)
                              if debug:
                                  nc.sync.dma_start(dbg["x2"][gw * WIN:(gw + 1) * WIN, :], nt[:])
                              ntp = ps_tp.tile([128, 256], BF16, tag="ntp2", space="PSUM")
                              for ki in range(2):
                                  nc.tensor.transpose(ntp[:, ki * 128:(ki + 1) * 128],
                                                      nt[:, ki * 128:(ki + 1) * 128],
                                                      ident[:])
                              ntT = sp.tile([128, 256], BF16, tag="ntT2")
                              nc.vector.tensor_copy(ntT[:], ntp[:])
                              abp = ps_sc.tile([128, 512], F32, tag="abp", space="PSUM")
                              for ki in range(2):
                                  nc.tensor.matmul(abp[:], ntT[:, ki * 128:(ki + 1) * 128],
                                                   wab3[:, ki * 512:(ki + 1) * 512],
                                                   start=(ki == 0), stop=(ki == 1))
                              abt = sp.tile([128, 512], BF16, tag="abt")
                              nc.vector.tensor_copy(abt[:], abp[:])
                              nc.sync.dma_start(a3_loc[gw * WIN:(gw + 1) * WIN, :],
                                                abt[:, 0:256])
                              nc.sync.dma_start(b3_loc[gw * WIN:(gw + 1) * WIN, :],
                                                abt[:, 256:512])

            for k in range(4):
                nc.gpsimd.collective_compute(
                    "AllGather", AOP.bypass, replica_groups=[list(range(NC))],
                    ins=[b3_loc[CUMS[k]:CUMS[k] + CSL[k], :].opt()],
                    outs=[b3_full[GOFF[k]:GOFF[k] + NC * CSL[k], :].opt()])

            # ======================= CONV 3 =======================
            if phases >= 3:
              with tc.tile_pool(name="c3sb", bufs=2) as sb:
                  c3b = [load_vec(t_c3b[mb], sb, f"c3b{mb}") for mb in range(2)]
                  c3gn = [[load_vec(t_c3gn[j, mb], sb, f"c3gn{j}{mb}") for mb in range(2)]
                          for j in range(3)]
                  G1 = sb.tile([128, ZW], F32, tag="G1")
                  G2 = sb.tile([128, ZW], F32, tag="G2")
                  nc.vector.memset(G1[:], 0.0)
                  nc.vector.memset(G2[:], 0.0)
                  # ---- pass 1: z = A3[dst] + B3[src]; Gram stats; spill z ----
                  with tc.tile_pool(name="c3g", bufs=2) as g3, \
                       tc.tile_pool(name="c3q", bufs=2) as gqp3, \
                       tc.tile_pool(name="c3zt", bufs=2) as ztp, \
                       tc.tile_pool(name="c3ps", bufs=4, space="PSUM") as ps, \
                       tc.tile_pool(name="c3gp", bufs=2, space="PSUM") as psg:
                      for b in range(NBLK):
                          ixj = g3.tile([128, NCHUNK], mybir.dt.int32, tag="ixj3")
                          nc.sync.dma_start(ixj[:], t_xj[:, b * NCHUNK:(b + 1) * NCHUNK])
                          gb = gqp3.tile([128, NCHUNK * 256], BF16, tag="gb3")
                          for ch in range(NCHUNK):
                              nc.gpsimd.indirect_dma_start(
                                  out=gb[:, ch * 256:(ch + 1) * 256],
                                  out_offset=None,
                                  in_=b3_full[:],
                                  in_offset=bass.IndirectOffsetOnAxis(
                                      ap=ixj[:, ch:ch + 1], axis=0))
                          zt = ztp.tile([128, NCHUNK * ZW], BF16, tag="zt")
                          ones_ap = zt[:].rearrange("p (c k) -> p c k", k=ZW)[:, :, 256:257]
                          nc.vector.memset(ones_ap, 1.0)
                          ohTb = g3.tile([128, BLK], BF16, tag="ohTb3")
                          nc.sync.dma_start(ohTb[:], t_ohT[:, b * BLK:(b + 1) * BLK])
                          for w in range(NW_BLK):
                              gw = b * NW_BLK + w
                              a3w = g3.tile([128, 256], BF16, tag="a3w")
                              nc.sync.dma_start(a3w[:], a3_loc[gw * WIN:(gw + 1) * WIN, :])
                              for cb in range(B):
                                  ch = w * B + cb
                                  ap_ = ps.tile([128, 256], F32, tag="aexp")
                                  nc.tensor.matmul(ap_[:],
                                                   ohTb[:, ch * 128:(ch + 1) * 128],
                                                   a3w[:], start=True, stop=True)
                                  nc.vector.tensor_tensor(
                                      zt[:, ch * ZW:ch * ZW + 256],
                                      gb[:, ch * 256:(ch + 1) * 256],
                                      ap_[:], op=AOP.add)
                          G1p = psg.tile([128, ZW], F32, tag="G1p", space="PSUM")
                          G2p = psg.tile([128, ZW], F32, tag="G2p", space="PSUM")
                          for ch in range(NCHUNK):
                              nc.tensor.matmul(G1p[:], zt[:, ch * ZW:ch * ZW + 128],
                                               zt[:, ch * ZW:ch * ZW + ZW],
                                               start=(ch == 0), stop=(ch == NCHUNK - 1),
                                               skip_group_check=True)
                              nc.tensor.matmul(G2p[:], zt[:, ch * ZW + 128:ch * ZW + 256],
                                               zt[:, ch * ZW:ch * ZW + ZW],
                                               start=(ch == 0), stop=(ch == NCHUNK - 1),
                                               skip_group_check=True)
                          nc.vector.tensor_tensor(G1[:], G1[:], G1p[:], op=AOP.add)
                          nc.vector.tensor_tensor(G2[:], G2[:], G2p[:], op=AOP.add)
                          nc.sync.dma_start(
                              z3_scr[:, b * NCHUNK * ZW:(b + 1) * NCHUNK * ZW], zt[:])
                  # stats: sums = G[:,256]; sumsq = diag
                  sA = sb.tile([128, 2], F32, tag="c3sA")
                  qA = sb.tile([128, 2], F32, tag="c3qA")
                  nc.vector.tensor_copy(sA[:, 0:1], G1[:, 256:257])
                  nc.vector.tensor_copy(sA[:, 1:2], G2[:, 256:257])
                  dtmp = sb.tile([128, 128], F32, tag="dtmp")
                  nc.vector.tensor_tensor(dtmp[:], G1[:, 0:128], eyef[:], op=AOP.mult)
                  nc.vector.reduce_sum(out=qA[:, 0:1], in_=dtmp[:], axis=AX.X)
                  nc.vector.tensor_tensor(dtmp[:], G2[:, 128:256], eyef[:], op=AOP.mult)
                  nc.vector.reduce_sum(out=qA[:, 1:2], in_=dtmp[:], axis=AX.X)
                  stg = allreduce_stats(sA, qA, 2, sb)
                  A1, C1 = affine_from_stats(stg, 2, c3b, c3gn, sb)
                  # broadcast affine rows: a_bc/c_bc [128, ZW] bf16
                  a_bc = sb.tile([128, ZW], BF16, tag="a_bc")
                  c_bc = sb.tile([128, ZW], BF16, tag="c_bc")
                  with tc.tile_pool(name="c3bp", bufs=2, space="PSUM") as psb:
                      for dst_t, vals in ((a_bc, A1), (c_bc, C1)):
                          nc.vector.memset(dst_t[:], 0.0)
                          for mb in range(2):
                              tp = psb.tile([128, 128], F32, tag="bcp", space="PSUM")
                              nc.tensor.transpose(
                                  tp[:], vals[mb][:, 0:1].to_broadcast([128, 128]),
                                  eyef[:])
                              nc.vector.tensor_copy(dst_t[:, mb * 128:(mb + 1) * 128], tp[:])
                  # ---- pass 2: h = relu(a*z + c); scatter ----
                  with tc.tile_pool(name="c3p2", bufs=2) as p2, \
                       tc.tile_pool(name="c3oh", bufs=2) as ohp, \
                       tc.tile_pool(name="c3sc", bufs=2, space="PSUM") as ps_sc:
                      for b in range(NBLK):
                          zt = p2.tile([128, NCHUNK * ZW], BF16, tag="zt2")
                          nc.sync.dma_start(
                              zt[:], z3_scr[:, b * NCHUNK * ZW:(b + 1) * NCHUNK * ZW])
                          h = p2.tile([128, NCHUNK * ZW], BF16, tag="h3b")
                          nc.vector.tensor_tensor(
                              out=h[:].rearrange("p (c k) -> p c k", k=ZW),
                              in0=zt[:].rearrange("p (c k) -> p c k", k=ZW),
                              in1=a_bc[:, None, :].to_broadcast((128, NCHUNK, ZW)),
                              op=AOP.mult)
                          nc.vector.tensor_tensor(
                              out=h[:].rearrange("p (c k) -> p c k", k=ZW),
                              in0=h[:].rearrange("p (c k) -> p c k", k=ZW),
                              in1=c_bc[:, None, :].to_broadcast((128, NCHUNK, ZW)),
                              op=AOP.add)
                          nc.scalar.activation(h[:], h[:], AFT.Relu)
                          ohb = ohp.tile([128, BLK], BF16, tag="ohb3")
                          nc.sync.dma_start(ohb[:], t_oh[:, b * BLK:(b + 1) * BLK])
                          for w in range(NW_BLK):
                              gw = b * NW_BLK + w
                              sc = ps_sc.tile([128, 256], F32, tag="sc3", space="PSUM")
                              for cb in range(B):
                                  ch = w * B + cb
                                  nc.tensor.matmul(
                                      sc[:], ohb[:, ch * 128:(ch + 1) * 128],
                                      h[:, ch * ZW:ch * ZW + 256],
                                      start=(cb == 0), stop=(cb == B - 1))
                              nt = p2.tile([128, 256], BF16, tag="nt3")
                              nc.vector.tensor_scalar(nt[:], sc[:], invcnt[:, gw:gw + 1],
                                                      None, AOP.mult)
                              nc.sync.dma_start(tab3_loc[gw * WIN:(gw + 1) * WIN, :], nt[:])

            if debug:
                nc.sync.dma_start(dbg["x3"][:], tab3_loc[:])

            # ======================= POOL + HEAD =======================
            if phases >= 4:
              with tc.tile_pool(name="p_sb", bufs=2) as sb, \
                 tc.tile_pool(name="p_ps", bufs=2, space="PSUM") as ps:
                  for gw in range(8):
                      pidx = sb.tile([128, Bg], mybir.dt.int32, tag="p_idx")
                      nc.sync.dma_start(pidx[:], t_pidx[gw])
                      poh = sb.tile([128, Bg * 128], BF16, tag="p_poh")
                      nc.sync.dma_start(poh[:],
                                        t_poh[:, gw * Bg * 128:(gw + 1) * Bg * 128])
                      gp = sb.tile([128, Bg * 256], BF16, tag="p_gp")
                      for c in range(Bg):
                          nc.gpsimd.indirect_dma_start(
                              out=gp[:, c * 256:(c + 1) * 256], out_offset=None,
                              in_=tab3_loc[:],
                              in_offset=bass.IndirectOffsetOnAxis(
                                  ap=pidx[:, c:c + 1], axis=0))
                      pp = ps.tile([128, 256], F32, tag="p_pp", space="PSUM")
                      for c in range(Bg):
                          nc.tensor.matmul(pp[:], poh[:, c * 128:(c + 1) * 128],
                                           gp[:, c * 256:(c + 1) * 256],
                                           start=(c == 0), stop=(c == Bg - 1))
                      pf = sb.tile([128, 256], F32, tag="p_pf")
                      nc.vector.tensor_copy(pf[:], pp[:])
                      nc.sync.dma_start(pool_in[gw * 128:(gw + 1) * 128, :], pf[:])
                  nc.gpsimd.collective_compute(
                      "AllReduce", AOP.add, replica_groups=[list(range(NC))],
                      ins=[pool_in.opt()], outs=[pool_out.opt()])
                  if debug:
                      nc.sync.dma_start(dbg["pool"][:], pool_out[:])

                  invg = sb.tile([128, 8], F32, tag="p_invg")
                  nc.sync.dma_start(invg[:], t_invg[:])
                  lw1 = []
                  for ki in range(2):
                      for mo in range(2):
                          w = sb.tile([128, 128], BF16, tag=f"p_lw1{ki}{mo}")
                          nc.sync.dma_start(w[:], t_lw1[ki, mo])
                          lw1.append(w)
                  lw2 = []
                  for ki in range(2):
                      w = sb.tile([128, 2], BF16, tag=f"p_lw2{ki}")
                      nc.sync.dma_start(w[:], t_lw2[ki])
                      lw2.append(w)
                  lb1 = [load_vec(t_lb1[mb], sb, f"p_lb1{mb}") for mb in range(2)]
                  lb2 = sb.tile([2, 1], F32, tag="p_lb2")
                  nc.sync.dma_start(lb2[:], t_lb2[:])
                  ofin = sb.tile([2, 1024], F32, tag="p_out")
                  for gw in range(8):
                      g = sb.tile([128, 256], F32, tag="p_g")
                      nc.sync.dma_start(g[:], pool_out[gw * 128:(gw + 1) * 128, :])
                      gm = sb.tile([128, 256], BF16, tag="p_gm")
                      nc.vector.tensor_scalar(gm[:], g[:], invg[:, gw:gw + 1], None, AOP.mult)
                      gT = sb.tile([128, 2 * 128], BF16, tag="p_gT")
                      for kb in range(2):
                          tp = ps.tile([128, 128], BF16, tag="p_tp", space="PSUM")
                          nc.tensor.transpose(tp[:], gm[:, kb * 128:(kb + 1) * 128], ident[:])
                          nc.vector.tensor_copy(gT[:, kb * 128:(kb + 1) * 128], tp[:])
                      hT = sb.tile([128, 2 * 128], BF16, tag="p_hT")
                      for mo in range(2):
                          hp = ps.tile([128, 128], F32, tag="p_hp", space="PSUM")
                          for ki in range(2):
                              nc.tensor.matmul(hp[:], lw1[ki * 2 + mo][:],
                                               gT[:, ki * 128:(ki + 1) * 128],
                                               start=(ki == 0), stop=(ki == 1))
                          nc.scalar.activation(hT[:, mo * 128:(mo + 1) * 128], hp[:],
                                               AFT.Relu, bias=lb1[mo])
                      op_ = ps.tile([2, 128], F32, tag="p_op", space="PSUM")
                      for ki in range(2):
                          nc.tensor.matmul(op_[:], lw2[ki][:],
                                           hT[:, ki * 128:(ki + 1) * 128],
                                           start=(ki == 0), stop=(ki == 1))
                      nc.vector.tensor_scalar(ofin[:, gw * 128:(gw + 1) * 128],
                                              op_[:], lb2[:], None, AOP.add)
                  nc.sync.dma_start(o_out[:], ofin[:, :N_GRAPHS])

    nc.compile()
    return nc


# ============================ entry point ============================


def kernel(**inputs):
    x = np.asarray(inputs["x"], dtype=np.float32)
    edge_index = np.asarray(inputs["edge_index"])
    batch = np.asarray(inputs["batch"])

    meta = _pack(edge_index, batch)
    Bg = meta["Bg"]

    import os as _os
    phases = int(_os.environ.get("KPHASES", "4"))
    key = ("mod", Bg, phases, _DEBUG[0])
    if key not in _cache:
        _cache[key] = _build(Bg, debug=bool(inputs.get("_debug", False)) or _DEBUG[0],
                             phases=phases)
    nc = _cache[key]

    # ---- per-core input arrays ----
    src = np.asarray(edge_index[0], dtype=np.int64)
    dst = np.asarray(edge_index[1], dtype=np.int64)

    # conv1 msgT: [core, 48, E_PAD//2] bf16; edge e<EHALF -> rows 0..9 col e,
    # e>=EHALF -> rows 32..41 col e-EHALF
    xi_v = x[dst]
    xj_v = x[src]
    msg = np.concatenate([xi_v, xj_v - xi_v], axis=1)       # [E, 10]
    msg_full = np.zeros((NC, E_PAD, 10), dtype=np.float32)
    ec, pos = meta["ec"], meta["pos"]
    msg_full[ec, pos] = msg[meta["eorder"]]
    msgT = np.zeros((NC, 48, EHALF), dtype=ml_dtypes.bfloat16)
    msgT[:, :10, :] = _bf(msg_full[:, :EHALF].transpose(0, 2, 1))
    msgT[:, 32:42, :] = _bf(msg_full[:, EHALF:].transpose(0, 2, 1))

    # one-hot tables from dstwin
    dstwin = meta["dstwin"]  # [NC, E_PAD], float; -1 for padding
    dw = dstwin.reshape(NC, NCH_ALL, 128).astype(np.int32)
    nn_ = np.arange(128, dtype=np.int32)
    oh_in = np.empty((NC, 128, E_PAD), dtype=ml_dtypes.bfloat16)
    ohT_in = np.empty((NC, 128, E_PAD), dtype=ml_dtypes.bfloat16)
    for c in range(NC):
        m = (dw[c][:, :, None] == nn_[None, None, :])     # [392, 128e, 128n]
        oh_in[c] = m.transpose(1, 0, 2).reshape(128, E_PAD).astype(ml_dtypes.bfloat16)
        ohT_in[c] = m.transpose(2, 0, 1).reshape(128, E_PAD).astype(ml_dtypes.bfloat16)

    invcnt_in = np.ascontiguousarray(
        meta["inv_cnt"].reshape(NC, NWIN, 128).transpose(0, 2, 1)).astype(np.float32)
    padcnt_in = np.repeat(meta["pad_cnt"][:, None], 128, axis=1)[:, :, None].astype(np.float32)

    ident_in = np.eye(128, dtype=np.float32).astype(ml_dtypes.bfloat16)
    eyef_in = np.eye(128, dtype=np.float32)

    xj_rm = _remap_slot(meta["xj_glob"])
    xj_in = np.ascontiguousarray(
        xj_rm.reshape(NC, NCH_ALL, 128).transpose(0, 2, 1)).astype(np.int32)

    # weights
    c1w = np.zeros((3, 128, 128), dtype=ml_dtypes.bfloat16)
    c1w[0, :10, :] = _bf(inputs["c1_w1"])
    c1w[0, 32:42, :] = _bf(inputs["c1_w1"])
    c1w[1] = _bf(inputs["c1_w2"])
    c1w[2] = _bf(inputs["c1_w3"])
    c1b = np.stack([np.asarray(inputs[f"c1_b{i}"], dtype=np.float32).reshape(128, 1)
                    for i in (1, 2, 3)])
    c1gn = np.stack([np.asarray(inputs[f"c1_gn{i}"], dtype=np.float32).reshape(3, 128, 1)
                     for i in (1, 2, 3)])

    w2a = np.asarray(inputs["c2_w1"], dtype=np.float32)   # [256, 256]
    WA2 = w2a[:128] - w2a[128:]
    WB2 = w2a[128:]
    wa2 = _bf(WA2)                                        # [128, 256]
    c2wb = _tile_w(WB2)[0]                                # [2(mo), 128, 128]
    c2w2 = _tile_w(np.asarray(inputs["c2_w2"], dtype=np.float32))  # [2,2,128,128]
    c2b = np.stack([np.asarray(inputs["c2_b1"], dtype=np.float32).reshape(2, 128, 1),
                    np.asarray(inputs["c2_b2"], dtype=np.float32).reshape(2, 128, 1)])
    c2gn = np.stack([np.asarray(inputs["c2_gn1"], dtype=np.float32).reshape(3, 2, 128, 1),
                     np.asarray(inputs["c2_gn2"], dtype=np.float32).reshape(3, 2, 128, 1)])

    w3a = np.asarray(inputs["c3_w1"], dtype=np.float32)   # [512, 256]
    WA3 = w3a[:256] - w3a[256:]
    WB3 = w3a[256:]
    wa3 = _bf(WA3).reshape(2, 128, 256)
    wb3 = _bf(WB3).reshape(2, 128, 256)
    c3b = np.asarray(inputs["c3_b1"], dtype=np.float32).reshape(2, 128, 1)
    c3gn = np.asarray(inputs["c3_gn1"], dtype=np.float32).reshape(3, 2, 128, 1)

    lw1 = _tile_w(np.asarray(inputs["lin_w1"], dtype=np.float32))
    lb1 = np.asarray(inputs["lin_b1"], dtype=np.float32).reshape(2, 128, 1)
    lw2_f = np.asarray(inputs["lin_w2"], dtype=np.float32)  # [256, 2]
    lw2 = np.stack([_bf(lw2_f[:128]), _bf(lw2_f[128:])])    # [2, 128, 2]
    lb2 = np.asarray(inputs["lin_b2"], dtype=np.float32).reshape(2, 1)

    pidx_in = np.ascontiguousarray(
        meta["pool_idx"].astype(np.int32).reshape(NC, 8, Bg, 128).transpose(0, 1, 3, 2))
    pgwl = meta["pool_gwl"].reshape(NC, 8, Bg, 128)        # [c, gw, cs, p]
    gg = np.arange(128, dtype=np.float32)
    poh_in = np.empty((NC, 128, 8 * Bg * 128), dtype=ml_dtypes.bfloat16)
    for c in range(NC):
        m = (pgwl[c][:, :, :, None] == gg[None, None, None, :])  # [8, Bg, 128p, 128g]
        poh_in[c] = m.transpose(2, 0, 1, 3).reshape(128, 8 * Bg * 128).astype(
            ml_dtypes.bfloat16)
    invg_in = np.broadcast_to(
        meta["inv_g"].reshape(8, 128).T[None], (NC, 128, 8)).astype(np.float32)
    invg_in = np.ascontiguousarray(invg_in)

    in_maps = []
    for c in range(NC):
        im = {
            "msgT": msgT[c],
            "xj_idx": xj_in[c],
            "pool_idx": pidx_in[c],
            "ohtab": oh_in[c],
            "ohTtab": ohT_in[c],
            "invcnt": invcnt_in[c],
            "padcnt": padcnt_in[c],
            "ident": ident_in,
            "eyef": eyef_in,
            "c1w": c1w, "c1b": c1b, "c1gn": c1gn,
            "wa2": wa2, "c2wb": c2wb, "c2w2": c2w2, "c2b": c2b, "c2gn": c2gn,
            "wa3": wa3, "wb3": wb3, "c3b": c3b, "c3gn": c3gn,
            "lw1": lw1, "lb1": lb1, "lw2": lw2, "lb2": lb2,
            "poolohtab": poh_in[c],
            "invg": invg_in[c],
        }
        in_maps.append(im)

    res = run_bass_kernel_spmd(nc, in_maps, core_ids=list(range(NC)),
                               trace=_TRACE[0])
    kernel.last_result = res
    kernel.last_meta = meta
    out = res.results[0]["out"]            # [2, 1000]
    return np.ascontiguousarray(out.T).astype(np.float32)


_DEBUG = [False]
_TRACE = [False]
